# revision 1
# baseline (speedup 1.0000x reference)
"""Trainium2 Bass kernel for nn_Graph_CNN_Feat_Mesh (Chebyshev GNN decoder).

Strategy (per-core, data-parallel over batch B=256 -> 32/core):
  - All spmms are dense matmuls on the tensor engine (PE) in bf16:
      y = A + L @ (B + L @ (2C)),  A/B/C = feature-space linears of the input.
    L is densified on host; for up4-preceded layers the replication is folded
    into LU = L @ U (contracting the small pre-upsample vertex space).
  - B and A linear terms accumulate directly into the spmm PSUM.
  - Activations live in packed F-layout [(j,Fin) partitions, (b//G)*Vsp + v]
    between layers; the per-layer linear emits V-layout directly; one PE
    transpose per layer returns to F-layout.
  - BatchNorm (training mode, global batch stats) is exact: per-core partial
    sums are AllReduce'd across the 8 cores in-kernel; scale/shift+relu fused
    into one ScalarE activation per layer.
  - FC head (2048->512->5120) runs in fp32; graph layers use bf16 operands
    with fp32 PSUM accumulation.
"""

import numpy as np

B = 256
NCORES = 8
BL = B // NCORES  # 32
EPS = 1e-5

_CACHE = {}


def _split_W(W):
    W = np.asarray(W, np.float32)
    return W[:, 0::3], W[:, 1::3], W[:, 2::3]


def _dense_L(rows, cols, vals, V):
    L = np.zeros((V, V), np.float32)
    np.add.at(L, (np.asarray(rows), np.asarray(cols)), np.asarray(vals, np.float32))
    return L


def _pad_rows(a, m):
    if a.shape[0] % m == 0:
        return a
    p = m - a.shape[0] % m
    return np.concatenate([a, np.zeros((p,) + a.shape[1:], a.dtype)], 0)


class _LCfg:
    def __init__(self, name, Vsp, V, Fin, Fout, up4, bn):
        self.name = name
        self.Vsp = Vsp      # source vertex space of C-linear (pre-up4)
        self.V = V          # output vertex count
        self.Fin = Fin
        self.Fout = Fout
        self.G = 128 // Fin          # batches packed on partitions at input
        self.nG = BL // self.G
        self.GF = self.G * Fout      # N of one B/C/A-linear matmul
        self.Gp = 128 // Fout if Fout in (32, 64) else None
        self.nGp = BL // self.Gp if self.Gp else None
        self.up4 = up4
        self.bn = bn
        self.nVt = (V + 127) // 128
        self.nVsp = (Vsp + 127) // 128
        self.BF = BL * Fout          # free width of V-layout per vtile

    def vts(self, t):
        return min(128, self.V - t * 128)

    def sps(self, s):
        return min(128, self.Vsp - s * 128)


CFGS = [
    _LCfg("c0", 80, 320, 64, 64, True, True),
    _LCfg("c1", 320, 320, 64, 32, False, True),
    _LCfg("c2", 320, 1280, 32, 32, True, True),
    _LCfg("c3", 1280, 1280, 32, 3, False, False),
]


def _wbd(W, G, Fin, Fout, which):
    """Block-diagonal rhs weight [128, G*Fout] for the fused linear.
    which: 'A' -> W0 - W2, 'B' -> W1, 'C' -> 2*W2.  col = j*Fout + c."""
    W0, W1, W2 = _split_W(W)
    M = {"A": W0 - W2, "B": W1, "C": 2.0 * W2}[which]  # [Fout, Fin]
    out = np.zeros((128, G * Fout), np.float32)
    for j in range(G):
        out[j * Fin:(j + 1) * Fin, j * Fout:(j + 1) * Fout] = M.T
    return out


def _build_host(inputs):
    import ml_dtypes
    bf = ml_dtypes.bfloat16
    f32 = np.float32
    d = {}
    d["xT"] = np.ascontiguousarray(np.asarray(inputs["x"], f32).T)  # [2048, 256]
    d["fc1wT"] = np.ascontiguousarray(np.asarray(inputs["fc1_w"], f32).T)
    d["fc1b"] = np.ascontiguousarray(
        np.asarray(inputs["fc1_b"], f32).reshape(4, 128).T)  # [128,4]
    d["fc2wT"] = np.ascontiguousarray(np.asarray(inputs["fc2_w"], f32).T)

    L1 = _dense_L(inputs["L1_rows"], inputs["L1_cols"], inputs["L1_vals"], 320)
    L2 = _dense_L(inputs["L2_rows"], inputs["L2_cols"], inputs["L2_vals"], 1280)
    U1 = np.repeat(np.eye(80, dtype=f32), 4, axis=0)    # [320, 80]
    U2 = np.repeat(np.eye(320, dtype=f32), 4, axis=0)   # [1280, 320]
    d["LU0"] = _pad_rows(np.ascontiguousarray((L1 @ U1).T), 128).astype(bf)  # [128,320]
    d["LT1"] = _pad_rows(np.ascontiguousarray(L1.T), 128).astype(bf)         # [384,320]
    d["LU2"] = _pad_rows(np.ascontiguousarray((L2 @ U2).T), 128).astype(bf)  # [384,1280]
    d["LT2"] = np.ascontiguousarray(L2.T).astype(bf)                         # [1280,1280]

    Wn = {"c0": "cl0_w", "c1": "cl1_w", "c2": "cl2_w", "c3": "cl3_w"}
    for cfg in CFGS:
        W = np.asarray(inputs[Wn[cfg.name]], f32)
        for which in "ABC":
            d[f"W{which}_{cfg.name}"] = _wbd(
                W, cfg.G, cfg.Fin, cfg.Fout, which).astype(bf)
    d["b3"] = np.asarray(inputs["cl3_b"], f32).copy()

    for i, (g, b) in enumerate([("bn0_g", "bn0_b"), ("bn1_g", "bn1_b"),
                                ("bn2_g", "bn2_b")]):
        gb = np.concatenate([np.asarray(inputs[g], f32),
                             np.asarray(inputs[b], f32)])
        d[f"gb{i}"] = np.ascontiguousarray(gb[None, :])  # [1, 2F]

    for F, nm in [(64, "sel64"), (32, "sel32")]:
        Gp = 128 // F
        sel = np.zeros((128, F), f32)
        for j in range(Gp):
            sel[j * F:(j + 1) * F] += np.eye(F, dtype=f32)
        d[nm] = sel
    return d


def _build_nc(b3_imm):
    import sys
    for p in ("/opt/trn_rl_repo", "/opt/trn_rl_repo/concourse"):
        if p not in sys.path:
            sys.path.insert(0, p)
    import concourse.bass as bass  # noqa
    import concourse.mybir as mybir
    import concourse.tile as tile
    from concourse import bacc
    from concourse.masks import make_identity

    f32 = mybir.dt.float32
    bf16 = mybir.dt.bfloat16
    AF = mybir.ActivationFunctionType
    ALU = mybir.AluOpType

    nc = bacc.Bacc(None, target_bir_lowering=False)

    xT = nc.dram_tensor("xT", [2048, BL], f32, kind="ExternalInput")
    fc1wT = nc.dram_tensor("fc1wT", [2048, 512], f32, kind="ExternalInput")
    fc1b = nc.dram_tensor("fc1b", [128, 4], f32, kind="ExternalInput")
    fc2wT = nc.dram_tensor("fc2wT", [512, 5120], f32, kind="ExternalInput")
    LU0 = nc.dram_tensor("LU0", [128, 320], bf16, kind="ExternalInput")
    LT1 = nc.dram_tensor("LT1", [384, 320], bf16, kind="ExternalInput")
    LU2 = nc.dram_tensor("LU2", [384, 1280], bf16, kind="ExternalInput")
    LT2 = nc.dram_tensor("LT2", [1280, 1280], bf16, kind="ExternalInput")
    Wt = {}
    for cfg in CFGS:
        for w in "ABC":
            Wt[f"{w}{cfg.name}"] = nc.dram_tensor(
                f"W{w}_{cfg.name}", [128, cfg.GF], bf16, kind="ExternalInput")
    gbs = [nc.dram_tensor(f"gb{i}", [1, 2 * F], f32, kind="ExternalInput")
           for i, F in enumerate([64, 32, 32])]
    sel64 = nc.dram_tensor("sel64", [128, 64], f32, kind="ExternalInput")
    sel32 = nc.dram_tensor("sel32", [128, 32], f32, kind="ExternalInput")
    ydram = nc.dram_tensor("y", [BL, 1280 * 3], f32, kind="ExternalOutput")

    with tile.TileContext(nc) as tc:
        with (
            tc.tile_pool(name="const", bufs=1) as constp,
            tc.tile_pool(name="wpool", bufs=1) as wpool,
            tc.tile_pool(name="poolA", bufs=2) as poolA,
            tc.tile_pool(name="poolB", bufs=2) as poolB,
            tc.tile_pool(name="poolC", bufs=1) as poolC,
            tc.tile_pool(name="misc", bufs=1) as miscp,
            tc.tile_pool(name="outp", bufs=3) as outp,
            tc.tile_pool(name="pslin", bufs=2, space="PSUM") as pslin,
            tc.tile_pool(name="psbig", bufs=2, space="PSUM") as psbig,
            tc.tile_pool(name="pstr", bufs=2, space="PSUM") as pstr,
            tc.tile_pool(name="dram", bufs=1, space="DRAM") as dramp,
        ):
            # ---- constants ----
            ident_b = constp.tile([128, 128], bf16, tag="identb")
            make_identity(nc, ident_b[:])
            ident_f = constp.tile([128, 128], f32, tag="identf")
            make_identity(nc, ident_f[:])
            sel_sb = {64: constp.tile([128, 64], f32, tag="sel64", name="sel64sb"),
                      32: constp.tile([128, 32], f32, tag="sel32", name="sel32sb")}
            nc.sync.dma_start(sel_sb[64][:], sel64[:])
            nc.sync.dma_start(sel_sb[32][:], sel32[:])
            gb_sb = []
            for i, F in enumerate([64, 32, 32]):
                t = constp.tile([1, 2 * F], f32, tag=f"gb{i}")
                nc.sync.dma_start(t[:], gbs[i][:])
                gb_sb.append(t)
            fc1b_sb = constp.tile([128, 4], f32, tag="fc1b")
            nc.sync.dma_start(fc1b_sb[:], fc1b[:])
            eps_t = constp.tile([1, 1], f32, tag="eps")
            nc.gpsimd.memset(eps_t[:], EPS)

            # L matrices + cheby weights
            LUT, LT = {}, {}
            t = wpool.tile([128, 320], bf16, tag="LU0")
            nc.sync.dma_start(t[:], LU0[0:128, :])
            LUT["c0"] = t
            t = wpool.tile([128, 3 * 320], bf16, tag="LT1")
            for s in range(3):
                nc.sync.dma_start(t[:, s * 320:(s + 1) * 320],
                                  LT1[s * 128:(s + 1) * 128, :])
            LT["c0"] = LT["c1"] = LUT["c1"] = t
            t = wpool.tile([128, 3 * 1280], bf16, tag="LU2")
            for s in range(3):
                nc.sync.dma_start(t[:, s * 1280:(s + 1) * 1280],
                                  LU2[s * 128:(s + 1) * 128, :])
            LUT["c2"] = t
            t = wpool.tile([128, 10 * 1280], bf16, tag="LT2")
            for s in range(10):
                nc.sync.dma_start(t[:, s * 1280:(s + 1) * 1280],
                                  LT2[s * 128:(s + 1) * 128, :])
            LT["c2"] = LT["c3"] = LUT["c3"] = t
            W_sb = {}
            for cfg in CFGS:
                for w in "ABC":
                    ti = wpool.tile([128, cfg.GF], bf16, tag=f"W{w}{cfg.name}")
                    nc.sync.dma_start(ti[:], Wt[f"{w}{cfg.name}"][:])
                    W_sb[f"{w}{cfg.name}"] = ti

            # ================= FC head (fp32) =================
            xT_sb = miscp.tile([128, 16 * BL], f32, tag="xT")
            for kt in range(16):
                nc.sync.dma_start(xT_sb[:, kt * BL:(kt + 1) * BL],
                                  xT[kt * 128:(kt + 1) * 128, :])
            fc1w_sb = poolA.tile([128, 16 * 512], f32, tag="A")
            for kt in range(16):
                nc.sync.dma_start(fc1w_sb[:, kt * 512:(kt + 1) * 512],
                                  fc1wT[kt * 128:(kt + 1) * 128, :])

            h1T = miscp.tile([128, 4 * BL], f32, tag="h1T")
            ps1 = pslin.tile([128, 4 * BL], f32, tag="lin")
            for mt in range(4):
                for kt in range(16):
                    nc.tensor.matmul(
                        ps1[:, mt * BL:(mt + 1) * BL],
                        fc1w_sb[:, kt * 512 + mt * 128: kt * 512 + (mt + 1) * 128],
                        xT_sb[:, kt * BL:(kt + 1) * BL],
                        start=(kt == 0), stop=(kt == 15))
                nc.scalar.activation(
                    h1T[:, mt * BL:(mt + 1) * BL], ps1[:, mt * BL:(mt + 1) * BL],
                    AF.Relu, bias=fc1b_sb[:, mt:mt + 1])

            # fc2 streamed in 4 column-chunks of 1280 (10 m-tiles each).
            # psum partition = (v0%2)*64+f, col = mi*BL+b ; channels c = v0*64+f.
            # dest: XF0[(b%2)*64+f, (b//2)*80 + v0],  v0 = 2*(mc*10+mi)+p0
            XF0 = poolC.tile([128, 16 * 80], bf16, tag="XF0")
            for mc in range(4):
                wch = poolB.tile([128, 4 * 1280], f32, tag="B")
                for kt in range(4):
                    nc.sync.dma_start(
                        wch[:, kt * 1280:(kt + 1) * 1280],
                        fc2wT[kt * 128:(kt + 1) * 128,
                              mc * 1280:(mc + 1) * 1280])
                ps2 = psbig.tile([128, 10 * BL], f32, tag="big")
                for mi in range(10):
                    for kt in range(4):
                        nc.tensor.matmul(
                            ps2[:, mi * BL:(mi + 1) * BL],
                            wch[:, kt * 1280 + mi * 128: kt * 1280 + (mi + 1) * 128],
                            h1T[:, kt * BL:(kt + 1) * BL],
                            start=(kt == 0), stop=(kt == 3))
                src4 = ps2[:].rearrange("p (i g j) -> p i g j", g=16, j=2)
                dst4 = XF0[:].rearrange("p (g u q) -> p g u q", u=40, q=2)
                for p0 in range(2):
                    for j in range(2):
                        nc.scalar.activation(
                            dst4[j * 64:(j + 1) * 64, :,
                                 mc * 10:(mc + 1) * 10, p0]
                            .rearrange("p g i -> p i g"),
                            src4[p0 * 64:(p0 + 1) * 64, :, :, j],
                            AF.Copy)

            # ================= cheby layers =================
            XF_cur = XF0
            ar_idx = 0

            for li, cfg in enumerate(CFGS):
                V, Vsp, F = cfg.V, cfg.Vsp, cfg.Fout
                BF = cfg.BF
                last = cfg.name == "c3"

                # --- replicate input for B/A linears if up4 ---
                if cfg.up4:
                    XFrep = poolA.tile([128, cfg.nG * V], bf16, tag="A")
                    s_r = XF_cur[:].rearrange("p (g w) -> p g w", w=Vsp)
                    d_r = XFrep[:].rearrange("p (g w r) -> p g w r", w=Vsp, r=4)
                    for r in range(4):
                        nc.vector.tensor_copy(d_r[:, :, :, r], s_r)
                else:
                    XFrep = XF_cur

                # --- C linear (in Vsp space) ---
                XC = poolC.tile([128, cfg.nVsp * BL * F], bf16, tag="XC")
                gpack = max(1, 512 // cfg.GF)
                for s in range(cfg.nVsp):
                    ssz = cfg.sps(s)
                    for g0 in range(0, cfg.nG, gpack):
                        gn = min(gpack, cfg.nG - g0)
                        pc = pslin.tile([128, 512], f32, tag="lin")
                        for gi in range(gn):
                            g = g0 + gi
                            nc.tensor.matmul(
                                pc[:ssz, gi * cfg.GF:(gi + 1) * cfg.GF],
                                XF_cur[:, g * Vsp + s * 128:
                                       g * Vsp + s * 128 + ssz],
                                W_sb[f"C{cfg.name}"][:],
                                start=True, stop=True)
                        nc.scalar.activation(
                            XC[:ssz, s * BL * F + g0 * cfg.GF:
                               s * BL * F + (g0 + gn) * cfg.GF],
                            pc[:ssz, :gn * cfg.GF], AF.Copy)

                # --- inner = LU @ (2C) + B ;  y = L @ inner + A ---
                Xin = poolB.tile([128, cfg.nVt * BF], bf16, tag="B")
                ytile = poolC.tile([128, cfg.nVt * BF], bf16, tag="YT")
                for phase in range(2):
                    srcL = LUT[cfg.name] if phase == 0 else LT[cfg.name]
                    nS = cfg.nVsp if phase == 0 else cfg.nVt
                    ssizes = ([cfg.sps(s) for s in range(nS)] if phase == 0
                              else [cfg.vts(s) for s in range(nS)])
                    rhs = XC if phase == 0 else Xin
                    rhs_w = BL * F if phase == 0 else BF
                    Wacc = W_sb[f"B{cfg.name}" if phase == 0 else f"A{cfg.name}"]
                    dst = Xin if phase == 0 else ytile
                    for t in range(cfg.nVt):
                        vsz = cfg.vts(t)
                        for pc0 in range(0, BF, 1024):
                            pw = min(1024, BF - pc0)
                            pi = psbig.tile([128, max(pw, 512)], f32, tag="big")
                            for nk in range(0, pw, 512):
                                n0 = pc0 + nk
                                n1 = min(n0 + 512, pc0 + pw)
                                for s in range(nS):
                                    ssz = ssizes[s]
                                    nc.tensor.matmul(
                                        pi[:vsz, n0 - pc0:n1 - pc0],
                                        srcL[:ssz, s * V + t * 128:
                                             s * V + t * 128 + vsz],
                                        rhs[:ssz, s * rhs_w + n0:
                                            s * rhs_w + n1],
                                        start=(s == 0), stop=False,
                                        skip_group_check=True)
                                for g in range(n0 // cfg.GF,
                                               (n1 + cfg.GF - 1) // cfg.GF):
                                    nc.tensor.matmul(
                                        pi[:vsz, g * cfg.GF - pc0:
                                           (g + 1) * cfg.GF - pc0],
                                        XFrep[:, g * V + t * 128:
                                              g * V + t * 128 + vsz],
                                        Wacc[:],
                                        start=False, stop=True,
                                        skip_group_check=True)
                            if last and phase == 1:
                                # reorder (b,fo) -> (fo,b) for output staging
                                nc.vector.tensor_copy(
                                    dst[:vsz, t * BF + pc0: t * BF + pc0 + pw]
                                    .rearrange("p (c b) -> p c b", b=BL),
                                    pi[:vsz, :pw]
                                    .rearrange("p (b c) -> p c b", c=3))
                            elif phase == 0:
                                nc.scalar.activation(
                                    dst[:vsz, t * BF + pc0: t * BF + pc0 + pw],
                                    pi[:vsz, :pw], AF.Copy)
                            else:
                                nc.vector.tensor_copy(
                                    dst[:vsz, t * BF + pc0: t * BF + pc0 + pw],
                                    pi[:vsz, :pw])

                if not last:
                    # --- back-transpose to packed F-layout of next level ---
                    Gp, nGp = cfg.Gp, cfg.nGp
                    XFn = poolA.tile([128, nGp * V], bf16, tag="A")
                    for t in range(cfg.nVt):
                        vsz = cfg.vts(t)
                        for q0 in range(0, nGp, 4):
                            qn = min(4, nGp - q0)
                            pt = pstr.tile([128, 512], bf16, tag="tr")
                            for qi in range(qn):
                                gp = q0 + qi
                                nc.tensor.transpose(
                                    pt[:, qi * 128: qi * 128 + vsz],
                                    ytile[:vsz, t * BF + gp * 128:
                                          t * BF + (gp + 1) * 128],
                                    ident_b[:vsz, :vsz])
                            dstv = XFn[:].rearrange("p (g v) -> p g v", v=V)
                            nc.scalar.activation(
                                dstv[:, q0:q0 + qn, t * 128:t * 128 + vsz],
                                pt[:].rearrange("p (q v) -> p q v", v=128)
                                [:, :qn, :vsz],
                                AF.Copy)

                    # --- BN stats (bf16 pre-BN values) -> AllReduce -> s,t ---
                    FD = nGp * V
                    nch = (FD + 511) // 512
                    bnst = miscp.tile([128, nch * 6], f32, tag="bnst")
                    for ch in range(nch):
                        c0_, c1_ = ch * 512, min((ch + 1) * 512, FD)
                        nc.vector.bn_stats(
                            bnst[:, ch * 6:(ch + 1) * 6], XFn[:, c0_:c1_])
                    aggr = miscp.tile([128, 2], f32, tag="aggr")
                    nc.vector.bn_aggr(
                        aggr[:], bnst[:].rearrange("p (c s) -> p c s", s=6))
                    part = miscp.tile([128, 2], f32, tag="part")
                    nc.vector.tensor_tensor(
                        out=part[:, 1:2], in0=aggr[:, 0:1], in1=aggr[:, 0:1],
                        op=ALU.mult)
                    nc.vector.tensor_tensor(
                        out=part[:, 1:2], in0=part[:, 1:2], in1=aggr[:, 1:2],
                        op=ALU.add)
                    nc.vector.tensor_scalar_mul(part[:, 1:2], part[:, 1:2],
                                                float(FD))
                    nc.vector.tensor_scalar_mul(part[:, 0:1], aggr[:, 0:1],
                                                float(FD))
                    pst = pslin.tile([128, 512], f32, tag="lin")
                    nc.tensor.matmul(pst[:1, :F], part[:, 0:1], sel_sb[F][:],
                                     start=True, stop=True)
                    nc.tensor.matmul(pst[:1, F:2 * F], part[:, 1:2],
                                     sel_sb[F][:], start=True, stop=True)
                    stats_l = miscp.tile([1, 2 * F], f32, tag="statl")
                    nc.vector.tensor_copy(stats_l[:], pst[:1, :2 * F])
                    bin_ = dramp.tile([1, 2 * F], f32, tag=f"arin{ar_idx}")
                    bout = dramp.tile([1, 2 * F], f32, tag=f"arout{ar_idx}")
                    nc.gpsimd.dma_start(bin_[:], stats_l[:])
                    nc.gpsimd.collective_compute(
                        "AllReduce", ALU.add,
                        replica_groups=[list(range(NCORES))],
                        ins=[bin_.opt()], outs=[bout.opt()])
                    stats_g = miscp.tile([1, 2 * F], f32, tag="statg")
                    nc.sync.dma_start(stats_g[:], bout[:])
                    n_g = float(B * V)
                    # tmp cols [0:F]=mu, [F:2F]=var->rstd ; st cols [0:F]=s, [F:2F]=t
                    st = miscp.tile([1, 2 * F], f32, tag="st")
                    tmp = miscp.tile([1, 2 * F], f32, tag="sttmp")
                    mu2 = miscp.tile([1, F], f32, tag="mu2")
                    nc.vector.tensor_scalar_mul(tmp[:, :2 * F], stats_g[:],
                                                1.0 / n_g)
                    nc.vector.tensor_tensor(out=mu2[:], in0=tmp[:, 0:F],
                                            in1=tmp[:, 0:F], op=ALU.mult)
                    nc.vector.tensor_tensor(out=tmp[:, F:2 * F],
                                            in0=tmp[:, F:2 * F],
                                            in1=mu2[:], op=ALU.subtract)
                    nc.scalar.activation(tmp[:, F:2 * F], tmp[:, F:2 * F],
                                         AF.Sqrt, bias=eps_t[:])
                    nc.vector.reciprocal(tmp[:, F:2 * F], tmp[:, F:2 * F])
                    nc.vector.tensor_tensor(out=st[:, 0:F],
                                            in0=tmp[:, F:2 * F],
                                            in1=gb_sb[li][:, 0:F], op=ALU.mult)
                    nc.vector.tensor_tensor(out=mu2[:], in0=tmp[:, 0:F],
                                            in1=st[:, 0:F], op=ALU.mult)
                    nc.vector.tensor_tensor(out=st[:, F:2 * F],
                                            in0=gb_sb[li][:, F:2 * F],
                                            in1=mu2[:], op=ALU.subtract)
                    pss = pslin.tile([128, 512], f32, tag="lin", name="pss")
                    nc.tensor.transpose(pss[:2 * F, 0:1], st[:],
                                        ident_f[:1, :1])
                    stc = miscp.tile([128, 2], f32, tag=f"stc{ar_idx}")
                    for j in range(Gp):
                        nc.vector.tensor_copy(stc[j * F:(j + 1) * F, 0:1],
                                              pss[:F, 0:1])
                        nc.vector.tensor_copy(stc[j * F:(j + 1) * F, 1:2],
                                              pss[F:2 * F, 0:1])
                    ar_idx += 1
                    nc.scalar.activation(
                        XFn[:], XFn[:], AF.Relu,
                        scale=stc[:, 0:1], bias=stc[:, 1:2])
                    XF_cur = XFn
                else:
                    # --- stage output: ytile [v, fo*32+b] -> [b, v*3+fo] ---
                    for t in range(cfg.nVt):
                        pt = pstr.tile([128, 512], bf16, tag="tr")
                        nc.tensor.transpose(
                            pt[:96, :128],
                            ytile[:128, t * BF:(t + 1) * BF],
                            ident_b[:128, :128])
                        och = outp.tile([BL, 384], f32, tag="out")
                        for fo in range(3):
                            nc.vector.tensor_scalar_add(
                                och[:].rearrange("b (v f) -> b v f", f=3)
                                [:, :, fo],
                                pt[fo * 32:(fo + 1) * 32, :128],
                                float(b3_imm[fo]))
                        nc.sync.dma_start(
                            ydram[:, t * 384:(t + 1) * 384], och[:])

    nc.compile()
    return nc


def kernel(**inputs):
    import sys
    for p in ("/opt/trn_rl_repo", "/opt/trn_rl_repo/concourse"):
        if p not in sys.path:
            sys.path.insert(0, p)
    from concourse.bass_utils import run_bass_kernel_spmd

    host = _build_host(inputs)
    b3 = [float(v) for v in host.pop("b3")]

    key = ("nc",) + tuple(b3)
    if key not in _CACHE:
        _CACHE[key] = _build_nc(b3)
    nc = _CACHE[key]

    in_maps = []
    for c in range(NCORES):
        m = {k: v for k, v in host.items() if k != "xT"}
        m["xT"] = np.ascontiguousarray(host["xT"][:, c * BL:(c + 1) * BL])
        in_maps.append(m)
    res = run_bass_kernel_spmd(nc, in_maps, core_ids=list(range(NCORES)))
    out = np.concatenate(
        [r["y"].reshape(BL, 1280, 3) for r in res.results], axis=0)
    return out.astype(np.float32)


if __name__ == "__main__":
    import reference as R
    inp = R.setup_inputs()
    inp = {k: np.asarray(v) for k, v in inp.items()}
    act = kernel(**inp)
    exp = np.asarray(R.reference(**inp))
    err = np.linalg.norm(act - exp) / np.linalg.norm(exp)
    print("Relative error:", err)



# revision 7
# speedup vs baseline: 1.2013x; 1.2013x over previous
"""Trainium2 Bass kernel for nn_Graph_CNN_Feat_Mesh (Chebyshev GNN decoder).

Strategy (per-core, data-parallel over batch B=256 -> 32/core):
  - All spmms are dense matmuls on the tensor engine (PE) in bf16:
      y = A + L @ (B + L @ (2C)),  A/B/C = feature-space linears of the input.
    L is densified on host; for up4-preceded layers the replication is folded
    into LU = L @ U (contracting the small pre-upsample vertex space).
  - B and A linear terms accumulate directly into the spmm PSUM.
  - Activations live in packed F-layout [(j,Fin) partitions, (b//G)*Vsp + v]
    between layers; the per-layer linear emits V-layout directly; one PE
    transpose per layer returns to F-layout.
  - BatchNorm (training mode, global batch stats) is exact: per-core partial
    sums are AllGather'd across the 8 cores in-kernel (cheaper than
    AllReduce in wall-clock) and summed locally with a K=8 ones-matmul;
    scale/shift+relu fused into ScalarE activations, chunked so the next
    layer can start on early chunks.
  - FC head (2048->512->5120) runs in bf16 with fp32 PSUM; weight DMAs are
    issued in consumption order so fc1 starts ~7us in.
"""

import numpy as np

B = 256
NCORES = 8
BL = B // NCORES  # 32
EPS = 1e-5

_CACHE = {}


def _split_W(W):
    W = np.asarray(W, np.float32)
    return W[:, 0::3], W[:, 1::3], W[:, 2::3]


def _dense_L(rows, cols, vals, V):
    L = np.zeros((V, V), np.float32)
    np.add.at(L, (np.asarray(rows), np.asarray(cols)), np.asarray(vals, np.float32))
    return L


def _pad_rows(a, m):
    if a.shape[0] % m == 0:
        return a
    p = m - a.shape[0] % m
    return np.concatenate([a, np.zeros((p,) + a.shape[1:], a.dtype)], 0)


class _LCfg:
    def __init__(self, name, Vsp, V, Fin, Fout, up4, bn):
        self.name = name
        self.Vsp = Vsp      # source vertex space of C-linear (pre-up4)
        self.V = V          # output vertex count
        self.Fin = Fin
        self.Fout = Fout
        self.G = 128 // Fin          # batches packed on partitions at input
        self.nG = BL // self.G
        self.GF = self.G * Fout      # N of one B/C/A-linear matmul
        self.Gp = 128 // Fout if Fout in (32, 64) else None
        self.nGp = BL // self.Gp if self.Gp else None
        self.up4 = up4
        self.bn = bn
        self.nVt = (V + 127) // 128
        self.nVsp = (Vsp + 127) // 128
        self.BF = BL * Fout          # free width of V-layout per vtile

    def vts(self, t):
        return min(128, self.V - t * 128)

    def sps(self, s):
        return min(128, self.Vsp - s * 128)


CFGS = [
    _LCfg("c0", 80, 320, 64, 64, True, True),
    _LCfg("c1", 320, 320, 64, 32, False, True),
    _LCfg("c2", 320, 1280, 32, 32, True, True),
    _LCfg("c3", 1280, 1280, 32, 3, False, False),
]


def _wbd(W, G, Fin, Fout, which):
    """Block-diagonal rhs weight [128, G*Fout] for the fused linear.
    which: 'A' -> W0 - W2, 'B' -> W1, 'C' -> 2*W2.  col = j*Fout + c."""
    W0, W1, W2 = _split_W(W)
    M = {"A": W0 - W2, "B": W1, "C": 2.0 * W2}[which]  # [Fout, Fin]
    out = np.zeros((128, G * Fout), np.float32)
    for j in range(G):
        out[j * Fin:(j + 1) * Fin, j * Fout:(j + 1) * Fout] = M.T
    return out


def _build_host(inputs):
    import ml_dtypes
    bf = ml_dtypes.bfloat16
    f32 = np.float32
    d = {}
    d["xT"] = np.ascontiguousarray(np.asarray(inputs["x"], f32).T).astype(bf)
    d["fc1wT"] = np.ascontiguousarray(np.asarray(inputs["fc1_w"], f32).T).astype(bf)
    d["fc1b"] = np.ascontiguousarray(
        np.asarray(inputs["fc1_b"], f32).reshape(4, 128).T)  # [128,4]
    d["fc2wT"] = np.ascontiguousarray(np.asarray(inputs["fc2_w"], f32).T).astype(bf)

    L1 = _dense_L(inputs["L1_rows"], inputs["L1_cols"], inputs["L1_vals"], 320)
    L2 = _dense_L(inputs["L2_rows"], inputs["L2_cols"], inputs["L2_vals"], 1280)
    U1 = np.repeat(np.eye(80, dtype=f32), 4, axis=0)    # [320, 80]
    U2 = np.repeat(np.eye(320, dtype=f32), 4, axis=0)   # [1280, 320]
    d["LU0"] = _pad_rows(np.ascontiguousarray((L1 @ U1).T), 128).astype(bf)  # [128,320]
    d["LT1"] = _pad_rows(np.ascontiguousarray(L1.T), 128).astype(bf)         # [384,320]
    d["LU2"] = _pad_rows(np.ascontiguousarray((L2 @ U2).T), 128).astype(bf)  # [384,1280]
    d["LT2"] = np.ascontiguousarray(L2.T).astype(bf)                         # [1280,1280]

    Wn = {"c0": "cl0_w", "c1": "cl1_w", "c2": "cl2_w", "c3": "cl3_w"}
    for cfg in CFGS:
        W = np.asarray(inputs[Wn[cfg.name]], f32)
        for which in "ABC":
            d[f"W{which}_{cfg.name}"] = _wbd(
                W, cfg.G, cfg.Fin, cfg.Fout, which).astype(bf)
    # b3 tiled over the (b, c) column layout of the last-layer PSUM: col = b*3+c
    d["b3row"] = np.ascontiguousarray(
        np.tile(np.asarray(inputs["cl3_b"], f32), BL)[None, :]).astype(bf)  # [1, 96]

    for i, (g, b) in enumerate([("bn0_g", "bn0_b"), ("bn1_g", "bn1_b"),
                                ("bn2_g", "bn2_b")]):
        gb = np.concatenate([np.asarray(inputs[g], f32),
                             np.asarray(inputs[b], f32)])
        d[f"gb{i}"] = np.ascontiguousarray(gb[None, :])  # [1, 2F]

    for F, nm in [(64, "sel64"), (32, "sel32")]:
        Gp = 128 // F
        sel = np.zeros((128, F), f32)
        for j in range(Gp):
            sel[j * F:(j + 1) * F] += np.eye(F, dtype=f32)
        d[nm] = sel
    return d


def _build_nc():
    import sys
    for p in ("/opt/trn_rl_repo", "/opt/trn_rl_repo/concourse"):
        if p not in sys.path:
            sys.path.insert(0, p)
    import concourse.bass as bass  # noqa
    import concourse.mybir as mybir
    import concourse.tile as tile
    from concourse import bacc
    from concourse.masks import make_identity

    f32 = mybir.dt.float32
    bf16 = mybir.dt.bfloat16
    AF = mybir.ActivationFunctionType
    ALU = mybir.AluOpType

    nc = bacc.Bacc(None, target_bir_lowering=False)

    xT = nc.dram_tensor("xT", [2048, BL], bf16, kind="ExternalInput")
    fc1wT = nc.dram_tensor("fc1wT", [2048, 512], bf16, kind="ExternalInput")
    fc1b = nc.dram_tensor("fc1b", [128, 4], f32, kind="ExternalInput")
    fc2wT = nc.dram_tensor("fc2wT", [512, 5120], bf16, kind="ExternalInput")
    LU0 = nc.dram_tensor("LU0", [128, 320], bf16, kind="ExternalInput")
    LT1 = nc.dram_tensor("LT1", [384, 320], bf16, kind="ExternalInput")
    LU2 = nc.dram_tensor("LU2", [384, 1280], bf16, kind="ExternalInput")
    LT2 = nc.dram_tensor("LT2", [1280, 1280], bf16, kind="ExternalInput")
    Wt = {}
    for cfg in CFGS:
        for w in "ABC":
            Wt[f"{w}{cfg.name}"] = nc.dram_tensor(
                f"W{w}_{cfg.name}", [128, cfg.GF], bf16, kind="ExternalInput")
    gbs = [nc.dram_tensor(f"gb{i}", [1, 2 * F], f32, kind="ExternalInput")
           for i, F in enumerate([64, 32, 32])]
    sel64 = nc.dram_tensor("sel64", [128, 64], f32, kind="ExternalInput")
    sel32 = nc.dram_tensor("sel32", [128, 32], f32, kind="ExternalInput")
    b3row = nc.dram_tensor("b3row", [1, 96], bf16, kind="ExternalInput")
    ydram = nc.dram_tensor("y", [BL, 1280 * 3], f32, kind="ExternalOutput")

    with tile.TileContext(nc) as tc:
        with (
            tc.tile_pool(name="const", bufs=1) as constp,
            tc.tile_pool(name="wpool", bufs=1) as wpool,
            tc.tile_pool(name="poolA", bufs=2) as poolA,
            tc.tile_pool(name="poolB", bufs=2) as poolB,
            tc.tile_pool(name="poolC", bufs=1) as poolC,
            tc.tile_pool(name="misc", bufs=1) as miscp,
            tc.tile_pool(name="outp", bufs=3) as outp,
            tc.tile_pool(name="pslin", bufs=2, space="PSUM") as pslin,
            tc.tile_pool(name="psbig", bufs=2, space="PSUM") as psbig,
            tc.tile_pool(name="pstr", bufs=2, space="PSUM") as pstr,
            tc.tile_pool(name="dram", bufs=1, space="DRAM") as dramp,
        ):
            # ---- fc1 inputs first: these DMAs gate the first matmul ----
            xT_sb = miscp.tile([128, 16 * BL], bf16, tag="xT")
            for kt in range(16):
                nc.sync.dma_start(xT_sb[:, kt * BL:(kt + 1) * BL],
                                  xT[kt * 128:(kt + 1) * 128, :])
            fc1b_sb = constp.tile([128, 4], f32, tag="fc1b")
            nc.sync.dma_start(fc1b_sb[:], fc1b[:])
            fc1w_sb = poolA.tile([128, 16 * 512], bf16, tag="A")
            for kt in range(16):
                nc.sync.dma_start(fc1w_sb[:, kt * 512:(kt + 1) * 512],
                                  fc1wT[kt * 128:(kt + 1) * 128, :])

            # ---- small constants (no DMA or tiny) ----
            ident_b = constp.tile([128, 128], bf16, tag="identb")
            make_identity(nc, ident_b[:])
            ident_f = constp.tile([1, 1], f32, tag="identf")
            nc.gpsimd.memset(ident_f[:], 1.0)
            eps_t = constp.tile([1, 1], f32, tag="eps")
            nc.gpsimd.memset(eps_t[:], EPS)
            ones8 = constp.tile([8, 1], f32, tag="ones8")
            nc.gpsimd.memset(ones8[:], 1.0)

            # ================= FC head (bf16, fp32 psum) =================
            h1T = miscp.tile([128, 4 * BL], bf16, tag="h1T")
            ps1 = pslin.tile([128, 4 * BL], f32, tag="lin")
            for mt in range(4):
                for kt in range(16):
                    nc.tensor.matmul(
                        ps1[:, mt * BL:(mt + 1) * BL],
                        fc1w_sb[:, kt * 512 + mt * 128: kt * 512 + (mt + 1) * 128],
                        xT_sb[:, kt * BL:(kt + 1) * BL],
                        start=(kt == 0), stop=(kt == 15))
                nc.scalar.activation(
                    h1T[:, mt * BL:(mt + 1) * BL], ps1[:, mt * BL:(mt + 1) * BL],
                    AF.Relu, bias=fc1b_sb[:, mt:mt + 1])

            # ---- mid-priority loads (small; needed for c0/c1 + BN) ----
            sel_sb = {64: constp.tile([128, 64], f32, tag="sel64", name="sel64sb"),
                      32: constp.tile([128, 32], f32, tag="sel32", name="sel32sb")}
            nc.sync.dma_start(sel_sb[64][:], sel64[:])
            nc.sync.dma_start(sel_sb[32][:], sel32[:])
            gb_sb = []
            for i, F in enumerate([64, 32, 32]):
                t = constp.tile([1, 2 * F], f32, tag=f"gb{i}")
                nc.sync.dma_start(t[:], gbs[i][:])
                gb_sb.append(t)
            b3_sb = constp.tile([1, 96], bf16, tag="b3row")
            nc.sync.dma_start(b3_sb[:], b3row[:])
            onesv = constp.tile([1, 128], bf16, tag="onesv")
            nc.gpsimd.memset(onesv[:], 1.0)

            LUT, LT = {}, {}
            t = wpool.tile([128, 320], bf16, tag="LU0")
            nc.sync.dma_start(t[:], LU0[0:128, :])
            LUT["c0"] = t
            t = wpool.tile([128, 3 * 320], bf16, tag="LT1")
            for s in range(3):
                nc.sync.dma_start(t[:, s * 320:(s + 1) * 320],
                                  LT1[s * 128:(s + 1) * 128, :])
            LT["c0"] = LT["c1"] = LUT["c1"] = t
            W_sb = {}
            for cfg in CFGS:
                for w in "ABC":
                    ti = wpool.tile([128, cfg.GF], bf16, tag=f"W{w}{cfg.name}")
                    nc.sync.dma_start(ti[:], Wt[f"{w}{cfg.name}"][:])
                    W_sb[f"{w}{cfg.name}"] = ti

            # ================= fc2 (streamed in 4 column-chunks) =========
            # psum partition = (v0%2)*64+f, col = mi*BL+b ; channels c = v0*64+f.
            # dest: XF0[(b%2)*64+f, (b//2)*80 + v0],  v0 = 2*(mc*10+mi)+p0
            XF0 = poolC.tile([128, 16 * 80], bf16, tag="XF0")
            for mc in range(4):
                wch = poolB.tile([128, 4 * 1280], bf16, tag="B")
                for kt in range(4):
                    nc.sync.dma_start(
                        wch[:, kt * 1280:(kt + 1) * 1280],
                        fc2wT[kt * 128:(kt + 1) * 128,
                              mc * 1280:(mc + 1) * 1280])
                ps2 = psbig.tile([128, 10 * BL], f32, tag="big")
                for mi in range(10):
                    for kt in range(4):
                        nc.tensor.matmul(
                            ps2[:, mi * BL:(mi + 1) * BL],
                            wch[:, kt * 1280 + mi * 128: kt * 1280 + (mi + 1) * 128],
                            h1T[:, kt * BL:(kt + 1) * BL],
                            start=(kt == 0), stop=(kt == 3))
                src4 = ps2[:].rearrange("p (i g j) -> p i g j", g=16, j=2)
                dst4 = XF0[:].rearrange("p (g u q) -> p g u q", u=40, q=2)
                for p0 in range(2):
                    for j in range(2):
                        nc.scalar.activation(
                            dst4[j * 64:(j + 1) * 64, :,
                                 mc * 10:(mc + 1) * 10, p0]
                            .rearrange("p g i -> p i g"),
                            src4[p0 * 64:(p0 + 1) * 64, :, :, j],
                            AF.Copy)

            # ---- big late loads (needed at c2; stream during c0/c1) ----
            t = wpool.tile([128, 3 * 1280], bf16, tag="LU2")
            for s in range(3):
                nc.sync.dma_start(t[:, s * 1280:(s + 1) * 1280],
                                  LU2[s * 128:(s + 1) * 128, :])
            LUT["c2"] = t
            t = wpool.tile([128, 10 * 1280], bf16, tag="LT2")
            for s in range(10):
                nc.sync.dma_start(t[:, s * 1280:(s + 1) * 1280],
                                  LT2[s * 128:(s + 1) * 128, :])
            LT["c2"] = LT["c3"] = LUT["c3"] = t

            # ================= cheby layers =================
            XF_cur = XF0
            ar_idx = 0

            for li, cfg in enumerate(CFGS):
                V, Vsp, F = cfg.V, cfg.Vsp, cfg.Fout
                BF = cfg.BF
                last = cfg.name == "c3"

                # --- replicate input for B/A linears if up4 ---
                if cfg.up4:
                    XFrep = poolA.tile([128, cfg.nG * V], bf16, tag="A")
                    s_r = XF_cur[:].rearrange("p (g w) -> p g w", w=Vsp)
                    d_r = XFrep[:].rearrange("p (g w r) -> p g w r", w=Vsp, r=4)
                    for r in range(4):
                        nc.vector.tensor_copy(d_r[:, :, :, r], s_r)
                else:
                    XFrep = XF_cur

                # --- C linear (in Vsp space) ---
                XC = poolC.tile([128, cfg.nVsp * BL * F], bf16, tag="XC")
                gpack = max(1, 512 // cfg.GF)
                for s in range(cfg.nVsp):
                    ssz = cfg.sps(s)
                    for g0 in range(0, cfg.nG, gpack):
                        gn = min(gpack, cfg.nG - g0)
                        pc = pslin.tile([128, 512], f32, tag="lin")
                        for gi in range(gn):
                            g = g0 + gi
                            nc.tensor.matmul(
                                pc[:ssz, gi * cfg.GF:(gi + 1) * cfg.GF],
                                XF_cur[:, g * Vsp + s * 128:
                                       g * Vsp + s * 128 + ssz],
                                W_sb[f"C{cfg.name}"][:],
                                start=True, stop=True)
                        nc.scalar.activation(
                            XC[:ssz, s * BL * F + g0 * cfg.GF:
                               s * BL * F + (g0 + gn) * cfg.GF],
                            pc[:ssz, :gn * cfg.GF], AF.Copy)

                # --- inner = LU @ (2C) + B ;  y = L @ inner + A ---
                Xin = poolB.tile([128, cfg.nVt * BF], bf16, tag="B")
                ytile = poolC.tile([128, cfg.nVt * BF], bf16, tag="YT")
                for phase in range(2):
                    srcL = LUT[cfg.name] if phase == 0 else LT[cfg.name]
                    nS = cfg.nVsp if phase == 0 else cfg.nVt
                    ssizes = ([cfg.sps(s) for s in range(nS)] if phase == 0
                              else [cfg.vts(s) for s in range(nS)])
                    rhs = XC if phase == 0 else Xin
                    rhs_w = BL * F if phase == 0 else BF
                    Wacc = W_sb[f"B{cfg.name}" if phase == 0 else f"A{cfg.name}"]
                    dst = Xin if phase == 0 else ytile
                    for t in range(cfg.nVt):
                        vsz = cfg.vts(t)
                        for pc0 in range(0, BF, 1024):
                            pw = min(1024, BF - pc0)
                            pi = psbig.tile([128, max(pw, 512)], f32, tag="big")
                            for nk in range(0, pw, 512):
                                n0 = pc0 + nk
                                n1 = min(n0 + 512, pc0 + pw)
                                for s in range(nS):
                                    ssz = ssizes[s]
                                    nc.tensor.matmul(
                                        pi[:vsz, n0 - pc0:n1 - pc0],
                                        srcL[:ssz, s * V + t * 128:
                                             s * V + t * 128 + vsz],
                                        rhs[:ssz, s * rhs_w + n0:
                                            s * rhs_w + n1],
                                        start=(s == 0), stop=False,
                                        skip_group_check=True)
                                for g in range(n0 // cfg.GF,
                                               (n1 + cfg.GF - 1) // cfg.GF):
                                    nc.tensor.matmul(
                                        pi[:vsz, g * cfg.GF - pc0:
                                           (g + 1) * cfg.GF - pc0],
                                        XFrep[:, g * V + t * 128:
                                              g * V + t * 128 + vsz],
                                        Wacc[:],
                                        start=False,
                                        stop=(not (last and phase == 1)),
                                        skip_group_check=True)
                                if last and phase == 1:
                                    # fold cl3 bias: += ones^T @ b3row
                                    nc.tensor.matmul(
                                        pi[:vsz, n0 - pc0:n1 - pc0],
                                        onesv[:1, :vsz],
                                        b3_sb[:1, n0:n1],
                                        start=False, stop=True,
                                        skip_group_check=True)
                            if last and phase == 1:
                                # reorder (b,fo) -> (fo,b) for output staging
                                nc.vector.tensor_copy(
                                    dst[:vsz, t * BF + pc0: t * BF + pc0 + pw]
                                    .rearrange("p (c b) -> p c b", b=BL),
                                    pi[:vsz, :pw]
                                    .rearrange("p (b c) -> p c b", c=3))
                            elif phase == 0:
                                nc.scalar.activation(
                                    dst[:vsz, t * BF + pc0: t * BF + pc0 + pw],
                                    pi[:vsz, :pw], AF.Copy)
                            else:
                                nc.vector.tensor_copy(
                                    dst[:vsz, t * BF + pc0: t * BF + pc0 + pw],
                                    pi[:vsz, :pw])

                if not last:
                    # --- back-transpose to packed F-layout of next level ---
                    Gp, nGp = cfg.Gp, cfg.nGp
                    XFn = poolA.tile([128, nGp * V], bf16, tag="A")
                    for t in range(cfg.nVt):
                        vsz = cfg.vts(t)
                        for q0 in range(0, nGp, 4):
                            qn = min(4, nGp - q0)
                            pt = pstr.tile([128, 512], bf16, tag="tr")
                            for qi in range(qn):
                                gp = q0 + qi
                                nc.tensor.transpose(
                                    pt[:, qi * 128: qi * 128 + vsz],
                                    ytile[:vsz, t * BF + gp * 128:
                                          t * BF + (gp + 1) * 128],
                                    ident_b[:vsz, :vsz])
                            dstv = XFn[:].rearrange("p (g v) -> p g v", v=V)
                            nc.scalar.activation(
                                dstv[:, q0:q0 + qn, t * 128:t * 128 + vsz],
                                pt[:].rearrange("p (q v) -> p q v", v=128)
                                [:, :qn, :vsz],
                                AF.Copy)

                    # --- BN stats (bf16 pre-BN values) -> AllGather -> s,t ---
                    FD = nGp * V
                    nch = (FD + 511) // 512
                    bnst = miscp.tile([128, nch * 6], f32, tag="bnst")
                    for ch in range(nch):
                        c0_, c1_ = ch * 512, min((ch + 1) * 512, FD)
                        nc.vector.bn_stats(
                            bnst[:, ch * 6:(ch + 1) * 6], XFn[:, c0_:c1_])
                    aggr = miscp.tile([128, 2], f32, tag="aggr")
                    nc.vector.bn_aggr(
                        aggr[:], bnst[:].rearrange("p (c s) -> p c s", s=6))
                    part = miscp.tile([128, 2], f32, tag="part")
                    nc.vector.tensor_tensor(
                        out=part[:, 1:2], in0=aggr[:, 0:1], in1=aggr[:, 0:1],
                        op=ALU.mult)
                    nc.vector.tensor_tensor(
                        out=part[:, 1:2], in0=part[:, 1:2], in1=aggr[:, 1:2],
                        op=ALU.add)
                    nc.vector.tensor_scalar_mul(part[:, 1:2], part[:, 1:2],
                                                float(FD))
                    nc.vector.tensor_scalar_mul(part[:, 0:1], aggr[:, 0:1],
                                                float(FD))
                    pst = pslin.tile([128, 512], f32, tag="lin")
                    nc.tensor.matmul(pst[:1, :F], part[:, 0:1], sel_sb[F][:],
                                     start=True, stop=True)
                    nc.tensor.matmul(pst[:1, F:2 * F], part[:, 1:2],
                                     sel_sb[F][:], start=True, stop=True)
                    stats_l = miscp.tile([1, 2 * F], f32, tag="statl")
                    nc.vector.tensor_copy(stats_l[:], pst[:1, :2 * F])
                    bin_ = dramp.tile([1, 2 * F], f32, tag=f"arin{ar_idx}")
                    bout = dramp.tile([8, 2 * F], f32, tag=f"arout{ar_idx}")
                    nc.gpsimd.dma_start(bin_[:], stats_l[:])
                    nc.gpsimd.collective_compute(
                        "AllGather", ALU.bypass,
                        replica_groups=[list(range(NCORES))],
                        ins=[bin_.opt()], outs=[bout.opt()])
                    sg8 = miscp.tile([8, 2 * F], f32, tag="sg8")
                    nc.sync.dma_start(sg8[:], bout[:])
                    psg = pslin.tile([128, 512], f32, tag="lin", name="psg")
                    nc.tensor.matmul(psg[:1, :2 * F], ones8[:, 0:1], sg8[:],
                                     start=True, stop=True)
                    n_g = float(B * V)
                    # tmp cols [0:F]=mu, [F:2F]=var->rstd ; st cols [0:F]=s, [F:2F]=t
                    st = miscp.tile([1, 2 * F], f32, tag="st")
                    tmp = miscp.tile([1, 2 * F], f32, tag="sttmp")
                    mu2 = miscp.tile([1, F], f32, tag="mu2")
                    nc.vector.tensor_scalar_mul(tmp[:, :2 * F], psg[:1, :2 * F],
                                                1.0 / n_g)
                    nc.vector.tensor_tensor(out=mu2[:], in0=tmp[:, 0:F],
                                            in1=tmp[:, 0:F], op=ALU.mult)
                    nc.vector.tensor_tensor(out=tmp[:, F:2 * F],
                                            in0=tmp[:, F:2 * F],
                                            in1=mu2[:], op=ALU.subtract)
                    nc.scalar.activation(tmp[:, F:2 * F], tmp[:, F:2 * F],
                                         AF.Sqrt, bias=eps_t[:])
                    nc.vector.reciprocal(tmp[:, F:2 * F], tmp[:, F:2 * F])
                    nc.vector.tensor_tensor(out=st[:, 0:F],
                                            in0=tmp[:, F:2 * F],
                                            in1=gb_sb[li][:, 0:F], op=ALU.mult)
                    nc.vector.tensor_tensor(out=mu2[:], in0=tmp[:, 0:F],
                                            in1=st[:, 0:F], op=ALU.mult)
                    nc.vector.tensor_tensor(out=st[:, F:2 * F],
                                            in0=gb_sb[li][:, F:2 * F],
                                            in1=mu2[:], op=ALU.subtract)
                    pss = pslin.tile([128, 512], f32, tag="lin", name="pss")
                    nc.tensor.transpose(pss[:2 * F, 0:1], st[:],
                                        ident_f[:1, :1])
                    stc = miscp.tile([128, 2], f32, tag=f"stc{ar_idx}")
                    for j in range(Gp):
                        nc.vector.tensor_copy(stc[j * F:(j + 1) * F, 0:1],
                                              pss[:F, 0:1])
                        nc.vector.tensor_copy(stc[j * F:(j + 1) * F, 1:2],
                                              pss[F:2 * F, 0:1])
                    ar_idx += 1
                    # chunked scale+relu so the next layer starts early
                    nap = max(1, nGp // 2)
                    for q0 in range(0, nGp, nap):
                        q1 = min(q0 + nap, nGp)
                        nc.scalar.activation(
                            XFn[:, q0 * V:q1 * V], XFn[:, q0 * V:q1 * V],
                            AF.Relu, scale=stc[:, 0:1], bias=stc[:, 1:2])
                    XF_cur = XFn
                else:
                    # --- stage output: ytile [v, fo*32+b] -> [b, v*3+fo] ---
                    for t in range(cfg.nVt):
                        pt = pstr.tile([128, 512], bf16, tag="tr")
                        nc.tensor.transpose(
                            pt[:96, :128],
                            ytile[:128, t * BF:(t + 1) * BF],
                            ident_b[:128, :128])
                        och = outp.tile([BL, 384], f32, tag="out")
                        for fo in range(3):
                            nc.scalar.activation(
                                och[:].rearrange("b (v f) -> b v f", f=3)
                                [:, :, fo],
                                pt[fo * 32:(fo + 1) * 32, :128],
                                AF.Copy)
                        nc.sync.dma_start(
                            ydram[:, t * 384:(t + 1) * 384], och[:])

    nc.compile()
    return nc


def kernel(**inputs):
    import sys
    for p in ("/opt/trn_rl_repo", "/opt/trn_rl_repo/concourse"):
        if p not in sys.path:
            sys.path.insert(0, p)
    from concourse.bass_utils import run_bass_kernel_spmd

    host = _build_host(inputs)

    if "nc" not in _CACHE:
        _CACHE["nc"] = _build_nc()
    nc = _CACHE["nc"]

    in_maps = []
    for c in range(NCORES):
        m = {k: v for k, v in host.items() if k != "xT"}
        m["xT"] = np.ascontiguousarray(host["xT"][:, c * BL:(c + 1) * BL])
        in_maps.append(m)
    res = run_bass_kernel_spmd(nc, in_maps, core_ids=list(range(NCORES)))
    out = np.concatenate(
        [r["y"].reshape(BL, 1280, 3) for r in res.results], axis=0)
    return out.astype(np.float32)


if __name__ == "__main__":
    import reference as R
    inp = R.setup_inputs()
    inp = {k: np.asarray(v) for k, v in inp.items()}
    act = kernel(**inp)
    exp = np.asarray(R.reference(**inp))
    err = np.linalg.norm(act - exp) / np.linalg.norm(exp)
    print("Relative error:", err)


# revision 13
# speedup vs baseline: 1.2403x; 1.0324x over previous
"""Trainium2 Bass kernel for nn_Graph_CNN_Feat_Mesh (Chebyshev GNN decoder).

Strategy (per-core, data-parallel over batch B=256 -> 32/core):
  - All spmms are dense matmuls on the tensor engine (PE) in bf16:
      y = A + L @ (B + L @ (2C)),  A/B/C = feature-space linears of the input.
    L is densified on host; for up4-preceded layers the replication is folded
    into LU = L @ U (contracting the small pre-upsample vertex space).
  - B and A linear terms accumulate directly into the spmm PSUM.
  - Activations live in packed F-layout [(j,Fin) partitions, (b//G)*Vsp + v]
    between layers; the per-layer linear emits V-layout directly; one PE
    transpose per layer returns to F-layout.
  - BatchNorm (training mode, global batch stats) is exact: per-core partial
    sums are AllGather'd across the 8 cores in-kernel (cheaper than
    AllReduce) and summed locally with a K=8 ones-matmul; stats are taken
    per transpose-group so they finish with the last transpose; scale+relu
    is chunked so the next layer starts on early chunks.
  - Weights are host-pre-tiled into [128, *] monoliths and streamed with a
    handful of large DMAs on the gpsimd queue (25ns issue) in consumption
    order; the FC head runs in bf16 with fp32 PSUM.
"""

import numpy as np

B = 256
NCORES = 8
BL = B // NCORES  # 32
EPS = 1e-5

_CACHE = {}


def _split_W(W):
    W = np.asarray(W, np.float32)
    return W[:, 0::3], W[:, 1::3], W[:, 2::3]


def _dense_L(rows, cols, vals, V):
    L = np.zeros((V, V), np.float32)
    np.add.at(L, (np.asarray(rows), np.asarray(cols)), np.asarray(vals, np.float32))
    return L


def _tile128(a):
    """[S*128, N] -> [128, S*N] block-column layout (pad rows to mult of 128)."""
    a = np.asarray(a)
    S = (a.shape[0] + 127) // 128
    if a.shape[0] != S * 128:
        a = np.concatenate(
            [a, np.zeros((S * 128 - a.shape[0], a.shape[1]), a.dtype)], 0)
    return np.ascontiguousarray(
        a.reshape(S, 128, a.shape[1]).transpose(1, 0, 2).reshape(128, -1))


class _LCfg:
    def __init__(self, name, Vsp, V, Fin, Fout, up4, bn):
        self.name = name
        self.Vsp = Vsp      # source vertex space of C-linear (pre-up4)
        self.V = V          # output vertex count
        self.Fin = Fin
        self.Fout = Fout
        self.G = 128 // Fin          # batches packed on partitions at input
        self.nG = BL // self.G
        self.GF = self.G * Fout      # N of one B/C/A-linear matmul
        self.Gp = 128 // Fout if Fout in (32, 64) else None
        self.nGp = BL // self.Gp if self.Gp else None
        self.up4 = up4
        self.bn = bn
        self.nVt = (V + 127) // 128
        self.nVsp = (Vsp + 127) // 128
        self.BF = BL * Fout          # free width of V-layout per vtile

    def vts(self, t):
        return min(128, self.V - t * 128)

    def sps(self, s):
        return min(128, self.Vsp - s * 128)


CFGS = [
    _LCfg("c0", 80, 320, 64, 64, True, True),
    _LCfg("c1", 320, 320, 64, 32, False, True),
    _LCfg("c2", 320, 1280, 32, 32, True, True),
    _LCfg("c3", 1280, 1280, 32, 3, False, False),
]


def _wbd(W, G, Fin, Fout, which):
    """Block-diagonal rhs weight [128, G*Fout] for the fused linear.
    which: 'A' -> W0 - W2, 'B' -> W1, 'C' -> 2*W2.  col = j*Fout + c."""
    W0, W1, W2 = _split_W(W)
    M = {"A": W0 - W2, "B": W1, "C": 2.0 * W2}[which]  # [Fout, Fin]
    out = np.zeros((128, G * Fout), np.float32)
    for j in range(G):
        out[j * Fin:(j + 1) * Fin, j * Fout:(j + 1) * Fout] = M.T
    return out


# column offsets inside the packed weight blobs
_WOFF = {}
_off = 0
for _cfg in CFGS:
    for _w in "ABC":
        _WOFF[f"{_w}{_cfg.name}"] = (_off, _cfg.GF)
        _off += _cfg.GF
WPACK_N = _off  # bf16 pack cols

# f32 pack: sel64 | sel32 | fc1b | selT-s/t rows are separate (see below)
F32_SEL64 = 0
F32_SEL32 = 64
F32_FC1B = 96
F32PACK_N = 100


def _build_host(inputs):
    import ml_dtypes
    bf = ml_dtypes.bfloat16
    f32 = np.float32
    d = {}
    d["xT"] = np.ascontiguousarray(np.asarray(inputs["x"], f32).T).astype(bf)
    d["fc1wt"] = _tile128(
        np.asarray(inputs["fc1_w"], f32).T).astype(bf)           # [128, 16*512]
    d["fc1b"] = np.ascontiguousarray(
        np.asarray(inputs["fc1_b"], f32).reshape(4, 128).T)      # [128,4]
    # fc2: chunk-major (mc), then k-tile: [128, 16*1280]
    w2 = np.asarray(inputs["fc2_w"], f32).T                      # [512, 5120]
    blk = [w2[kt * 128:(kt + 1) * 128, mc * 1280:(mc + 1) * 1280]
           for mc in range(4) for kt in range(4)]
    d["fc2wt"] = np.ascontiguousarray(np.concatenate(blk, 1)).astype(bf)

    L1 = _dense_L(inputs["L1_rows"], inputs["L1_cols"], inputs["L1_vals"], 320)
    L2 = _dense_L(inputs["L2_rows"], inputs["L2_cols"], inputs["L2_vals"], 1280)
    U1 = np.repeat(np.eye(80, dtype=f32), 4, axis=0)    # [320, 80]
    U2 = np.repeat(np.eye(320, dtype=f32), 4, axis=0)   # [1280, 320]
    d["LU0"] = _tile128((L1 @ U1).T).astype(bf)         # [128, 320]
    d["LT1"] = _tile128(L1.T).astype(bf)                # [128, 3*320]
    d["LU2"] = _tile128((L2 @ U2).T).astype(bf)         # [128, 3*1280]
    d["LT2"] = _tile128(L2.T).astype(bf)                # [128, 10*1280]

    Wn = {"c0": "cl0_w", "c1": "cl1_w", "c2": "cl2_w", "c3": "cl3_w"}
    wall = np.zeros((128, WPACK_N), f32)
    for cfg in CFGS:
        W = np.asarray(inputs[Wn[cfg.name]], f32)
        for which in "ABC":
            o, n = _WOFF[f"{which}{cfg.name}"]
            wall[:, o:o + n] = _wbd(W, cfg.G, cfg.Fin, cfg.Fout, which)
    d["wall"] = wall.astype(bf)
    # b3 tiled over the (b, c) column layout of the last-layer PSUM: col = b*3+c
    d["b3row"] = np.ascontiguousarray(
        np.tile(np.asarray(inputs["cl3_b"], f32), BL)[None, :]).astype(bf)

    gbp = np.zeros((1, 256), f32)
    for i, (g, b, o, F) in enumerate([("bn0_g", "bn0_b", 0, 64),
                                      ("bn1_g", "bn1_b", 128, 32),
                                      ("bn2_g", "bn2_b", 192, 32)]):
        gbp[0, o:o + F] = np.asarray(inputs[g], f32)
        gbp[0, o + F:o + 2 * F] = np.asarray(inputs[b], f32)
    d["gbpack"] = gbp

    f32p = np.zeros((128, F32PACK_N), f32)
    for F, o in [(64, F32_SEL64), (32, F32_SEL32)]:
        for j in range(128 // F):
            f32p[j * F:(j + 1) * F, o:o + F] += np.eye(F, dtype=f32)
    f32p[:, F32_FC1B:F32_FC1B + 4] = d.pop("fc1b")
    d["f32pack"] = f32p
    # selT_s/selT_t [2F rows, 128]: stc[p,:] = (s[p%F], t[p%F]) via 2 matmuls
    stp = np.zeros((128, 2 * 128), f32)
    for F, ro in [(64, 0), (32, 0)]:
        pass
    sT = np.zeros((128, 256), f32)   # rows k (2F<=128), cols: [0:128]=s-map, [128:256]=t-map
    # build per-F maps stacked by row-offset: F=64 uses rows 0:128, F=32 uses rows 0:64
    sT64 = np.zeros((128, 256), f32)
    for p in range(128):
        sT64[p % 64, p] = 1.0          # k = c        -> s
        sT64[64 + p % 64, 128 + p] = 1.0  # k = F + c  -> t
    sT32 = np.zeros((64, 256), f32)
    for p in range(128):
        sT32[p % 32, p] = 1.0
        sT32[32 + p % 32, 128 + p] = 1.0
    d["selT64"] = sT64
    d["selT32"] = np.concatenate([sT32, np.zeros((64, 256), f32)], 0)
    return d


def _build_nc():
    import sys
    for p in ("/opt/trn_rl_repo", "/opt/trn_rl_repo/concourse"):
        if p not in sys.path:
            sys.path.insert(0, p)
    import concourse.bass as bass  # noqa
    import concourse.mybir as mybir
    import concourse.tile as tile
    from concourse import bacc
    from concourse.masks import make_identity

    f32 = mybir.dt.float32
    bf16 = mybir.dt.bfloat16
    AF = mybir.ActivationFunctionType
    ALU = mybir.AluOpType

    nc = bacc.Bacc(None, target_bir_lowering=False)

    xT = nc.dram_tensor("xT", [2048, BL], bf16, kind="ExternalInput")
    fc1wt = nc.dram_tensor("fc1wt", [128, 16 * 512], bf16, kind="ExternalInput")
    fc2wt = nc.dram_tensor("fc2wt", [128, 16 * 1280], bf16, kind="ExternalInput")
    LU0 = nc.dram_tensor("LU0", [128, 320], bf16, kind="ExternalInput")
    LT1 = nc.dram_tensor("LT1", [128, 3 * 320], bf16, kind="ExternalInput")
    LU2 = nc.dram_tensor("LU2", [128, 3 * 1280], bf16, kind="ExternalInput")
    LT2 = nc.dram_tensor("LT2", [128, 10 * 1280], bf16, kind="ExternalInput")
    wall = nc.dram_tensor("wall", [128, WPACK_N], bf16, kind="ExternalInput")
    gbpack = nc.dram_tensor("gbpack", [1, 256], f32, kind="ExternalInput")
    f32pack = nc.dram_tensor("f32pack", [128, F32PACK_N], f32, kind="ExternalInput")
    selT64 = nc.dram_tensor("selT64", [128, 256], f32, kind="ExternalInput")
    selT32 = nc.dram_tensor("selT32", [128, 256], f32, kind="ExternalInput")
    b3row = nc.dram_tensor("b3row", [1, 96], bf16, kind="ExternalInput")
    ydram = nc.dram_tensor("y", [BL, 1280 * 3], f32, kind="ExternalOutput")

    with tile.TileContext(nc) as tc:
        with (
            tc.tile_pool(name="const", bufs=1) as constp,
            tc.tile_pool(name="wpool", bufs=1) as wpool,
            tc.tile_pool(name="poolA", bufs=2) as poolA,
            tc.tile_pool(name="poolB", bufs=2) as poolB,
            tc.tile_pool(name="poolC", bufs=1) as poolC,
            tc.tile_pool(name="misc", bufs=1) as miscp,
            tc.tile_pool(name="outp", bufs=3) as outp,
            tc.tile_pool(name="pslin", bufs=2, space="PSUM") as pslin,
            tc.tile_pool(name="psbig", bufs=2, space="PSUM") as psbig,
            tc.tile_pool(name="pstr", bufs=2, space="PSUM") as pstr,
            tc.tile_pool(name="dram", bufs=1, space="DRAM") as dramp,
        ):
            # ---- fc1 inputs first: these DMAs gate the first matmul ----
            xT_sb = miscp.tile([128, 16 * BL], bf16, tag="xT")
            nc.gpsimd.dma_start(
                xT_sb[:].rearrange("p (k b) -> p k b", b=BL),
                xT[:].rearrange("(k p) b -> p k b", p=128))
            f32_sb = constp.tile([128, F32PACK_N], f32, tag="f32pack")
            nc.gpsimd.dma_start(f32_sb[:], f32pack[:])
            fc1w_sb = poolA.tile([128, 16 * 512], bf16, tag="A")
            nc.gpsimd.dma_start(fc1w_sb[:], fc1wt[:])
            sel_sb = {64: f32_sb[:, F32_SEL64:F32_SEL64 + 64],
                      32: f32_sb[:, F32_SEL32:F32_SEL32 + 32]}
            fc1b_sb = f32_sb[:, F32_FC1B:F32_FC1B + 4]

            # ---- small constants (no DMA) ----
            ident_b = constp.tile([128, 128], bf16, tag="identb")
            make_identity(nc, ident_b[:])
            ident_f = constp.tile([1, 1], f32, tag="identf")
            nc.gpsimd.memset(ident_f[:], 1.0)
            eps_t = constp.tile([1, 1], f32, tag="eps")
            nc.gpsimd.memset(eps_t[:], EPS)
            ones8 = constp.tile([8, 1], f32, tag="ones8")
            nc.gpsimd.memset(ones8[:], 1.0)
            onesv = constp.tile([1, 128], bf16, tag="onesv")
            nc.gpsimd.memset(onesv[:], 1.0)
            sq_warm = constp.tile([1, 1], f32, tag="sqwarm")
            nc.scalar.activation(sq_warm[:], eps_t[:], AF.Sqrt, bias=eps_t[:])

            # ================= FC head (bf16, fp32 psum) =================
            h1T = miscp.tile([128, 4 * BL], bf16, tag="h1T")
            ps1 = pslin.tile([128, 4 * BL], f32, tag="lin")
            for mt in range(4):
                for kt in range(16):
                    nc.tensor.matmul(
                        ps1[:, mt * BL:(mt + 1) * BL],
                        fc1w_sb[:, kt * 512 + mt * 128: kt * 512 + (mt + 1) * 128],
                        xT_sb[:, kt * BL:(kt + 1) * BL],
                        start=(kt == 0), stop=(kt == 15))
                nc.scalar.activation(
                    h1T[:, mt * BL:(mt + 1) * BL], ps1[:, mt * BL:(mt + 1) * BL],
                    AF.Relu, bias=fc1b_sb[:, mt:mt + 1])

            # ---- mid-priority loads (small; needed for c0/c1 + BN) ----
            gb_all = constp.tile([1, 256], f32, tag="gbp")
            nc.gpsimd.dma_start(gb_all[:], gbpack[:])
            gb_sb = [gb_all[:, 0:128], gb_all[:, 128:192], gb_all[:, 192:256]]
            sT_sb = {64: constp.tile([128, 256], f32, tag="sT64", name="sT64sb"),
                     32: constp.tile([128, 256], f32, tag="sT32", name="sT32sb")}
            nc.gpsimd.dma_start(sT_sb[64][:], selT64[:])
            nc.gpsimd.dma_start(sT_sb[32][:], selT32[:])
            b3_sb = constp.tile([1, 96], bf16, tag="b3row")
            nc.gpsimd.dma_start(b3_sb[:], b3row[:])

            LUT, LTd = {}, {}
            t = wpool.tile([128, 320], bf16, tag="LU0")
            nc.gpsimd.dma_start(t[:], LU0[:])
            LUT["c0"] = t
            t = wpool.tile([128, 3 * 320], bf16, tag="LT1")
            nc.gpsimd.dma_start(t[:], LT1[:])
            LTd["c0"] = LTd["c1"] = LUT["c1"] = t
            wall_sb = wpool.tile([128, WPACK_N], bf16, tag="wall")
            nc.gpsimd.dma_start(wall_sb[:], wall[:])
            W_sb = {}
            for cfg in CFGS:
                for w in "ABC":
                    o, n = _WOFF[f"{w}{cfg.name}"]
                    W_sb[f"{w}{cfg.name}"] = wall_sb[:, o:o + n]

            # ================= fc2 (streamed in 4 column-chunks) =========
            # psum partition = (v0%2)*64+f, col = mi*BL+b ; channels c = v0*64+f.
            # dest: XF0[(b%2)*64+f, (b//2)*80 + v0],  v0 = 2*(mc*10+mi)+p0
            XF0 = poolC.tile([128, 16 * 80], bf16, tag="XF0")
            cfg0 = CFGS[0]
            XFrep0 = poolA.tile([128, cfg0.nG * cfg0.V], bf16, tag="A")
            s_r0 = XF0[:].rearrange("p (g w) -> p g w", w=80)
            d_r0 = XFrep0[:].rearrange("p (g w r) -> p g w r", w=80, r=4)
            for mc in range(4):
                wch = poolB.tile([128, 4 * 1280], bf16, tag="B")
                nc.gpsimd.dma_start(
                    wch[:], fc2wt[:, mc * 4 * 1280:(mc + 1) * 4 * 1280])
                ps2 = psbig.tile([128, 10 * BL], f32, tag="big")
                for mi in range(10):
                    for kt in range(4):
                        nc.tensor.matmul(
                            ps2[:, mi * BL:(mi + 1) * BL],
                            wch[:, kt * 1280 + mi * 128: kt * 1280 + (mi + 1) * 128],
                            h1T[:, kt * BL:(kt + 1) * BL],
                            start=(kt == 0), stop=(kt == 3))
                src4 = ps2[:].rearrange("p (i g j) -> p i g j", g=16, j=2)
                dst4 = XF0[:].rearrange("p (g u q) -> p g u q", u=40, q=2)
                for p0 in range(2):
                    for j in range(2):
                        nc.scalar.activation(
                            dst4[j * 64:(j + 1) * 64, :,
                                 mc * 10:(mc + 1) * 10, p0]
                            .rearrange("p g i -> p i g"),
                            src4[p0 * 64:(p0 + 1) * 64, :, :, j],
                            AF.Copy)
                # up4-replicate this chunk's w-range (w = v0 in [20mc, 20mc+20))
                for r in range(4):
                    nc.vector.tensor_copy(
                        d_r0[:, :, 20 * mc:20 * (mc + 1), r],
                        s_r0[:, :, 20 * mc:20 * (mc + 1)])

            # ---- big late loads (needed at c2; stream during c0/c1) ----
            t = wpool.tile([128, 3 * 1280], bf16, tag="LU2")
            nc.sync.dma_start(t[:], LU2[:])
            LUT["c2"] = t
            t = wpool.tile([128, 10 * 1280], bf16, tag="LT2")
            nc.sync.dma_start(t[:, :5 * 1280], LT2[:, :5 * 1280])
            nc.sync.dma_start(t[:, 5 * 1280:], LT2[:, 5 * 1280:])
            LTd["c2"] = LTd["c3"] = LUT["c3"] = t

            # ================= cheby layers =================
            XF_cur = XF0
            XFrep_cur = XFrep0
            ar_idx = 0

            for li, cfg in enumerate(CFGS):
                V, Vsp, F = cfg.V, cfg.Vsp, cfg.Fout
                BF = cfg.BF
                last = cfg.name == "c3"
                XFrep = XFrep_cur if cfg.up4 else XF_cur

                # --- C linear (in Vsp space) ---
                XC = poolC.tile([128, cfg.nVsp * BL * F], bf16, tag="XC")
                gpack = max(1, 512 // cfg.GF)
                for s in range(cfg.nVsp):
                    ssz = cfg.sps(s)
                    for g0 in range(0, cfg.nG, gpack):
                        gn = min(gpack, cfg.nG - g0)
                        pc = pslin.tile([128, 512], f32, tag="lin")
                        for gi in range(gn):
                            g = g0 + gi
                            nc.tensor.matmul(
                                pc[:ssz, gi * cfg.GF:(gi + 1) * cfg.GF],
                                XF_cur[:, g * Vsp + s * 128:
                                       g * Vsp + s * 128 + ssz],
                                W_sb[f"C{cfg.name}"][:],
                                start=True, stop=True)
                        nc.scalar.activation(
                            XC[:ssz, s * BL * F + g0 * cfg.GF:
                               s * BL * F + (g0 + gn) * cfg.GF],
                            pc[:ssz, :gn * cfg.GF], AF.Copy)

                # --- inner = LU @ (2C) + B ;  y = L @ inner + A ---
                Xin = poolB.tile([128, cfg.nVt * BF], bf16, tag="B")
                ytile = poolC.tile([128, cfg.nVt * BF], bf16, tag="YT")
                for phase in range(2):
                    srcL = LUT[cfg.name] if phase == 0 else LTd[cfg.name]
                    nS = cfg.nVsp if phase == 0 else cfg.nVt
                    ssizes = ([cfg.sps(s) for s in range(nS)] if phase == 0
                              else [cfg.vts(s) for s in range(nS)])
                    rhs = XC if phase == 0 else Xin
                    rhs_w = BL * F if phase == 0 else BF
                    Wacc = W_sb[f"B{cfg.name}" if phase == 0 else f"A{cfg.name}"]
                    dst = Xin if phase == 0 else ytile
                    for t in range(cfg.nVt):
                        vsz = cfg.vts(t)
                        for pc0 in range(0, BF, 1024):
                            pw = min(1024, BF - pc0)
                            pi = psbig.tile([128, max(pw, 512)], f32, tag="big")
                            for nk in range(0, pw, 512):
                                n0 = pc0 + nk
                                n1 = min(n0 + 512, pc0 + pw)
                                for s in range(nS):
                                    ssz = ssizes[s]
                                    nc.tensor.matmul(
                                        pi[:vsz, n0 - pc0:n1 - pc0],
                                        srcL[:ssz, s * V + t * 128:
                                             s * V + t * 128 + vsz],
                                        rhs[:ssz, s * rhs_w + n0:
                                            s * rhs_w + n1],
                                        start=(s == 0), stop=False,
                                        skip_group_check=True)
                                for g in range(n0 // cfg.GF,
                                               (n1 + cfg.GF - 1) // cfg.GF):
                                    nc.tensor.matmul(
                                        pi[:vsz, g * cfg.GF - pc0:
                                           (g + 1) * cfg.GF - pc0],
                                        XFrep[:, g * V + t * 128:
                                              g * V + t * 128 + vsz],
                                        Wacc[:],
                                        start=False,
                                        stop=(not (last and phase == 1)),
                                        skip_group_check=True)
                                if last and phase == 1:
                                    # fold cl3 bias: += ones^T @ b3row
                                    nc.tensor.matmul(
                                        pi[:vsz, n0 - pc0:n1 - pc0],
                                        onesv[:1, :vsz],
                                        b3_sb[:1, n0:n1],
                                        start=False, stop=True,
                                        skip_group_check=True)
                            if last and phase == 1:
                                # reorder (b,fo) -> (fo,b) for output staging
                                nc.vector.tensor_copy(
                                    dst[:vsz, t * BF + pc0: t * BF + pc0 + pw]
                                    .rearrange("p (c b) -> p c b", b=BL),
                                    pi[:vsz, :pw]
                                    .rearrange("p (b c) -> p c b", c=3))
                            elif phase == 0:
                                nc.scalar.activation(
                                    dst[:vsz, t * BF + pc0: t * BF + pc0 + pw],
                                    pi[:vsz, :pw], AF.Copy)
                            else:
                                nc.vector.tensor_copy(
                                    dst[:vsz, t * BF + pc0: t * BF + pc0 + pw],
                                    pi[:vsz, :pw])

                if not last:
                    # --- back-transpose to packed F-layout; stats per group ---
                    Gp, nGp = cfg.Gp, cfg.nGp
                    nq = (nGp + 3) // 4
                    XFn = poolA.tile([128, nGp * V], bf16, tag="A")
                    dstv = XFn[:].rearrange("p (g v) -> p g v", v=V)
                    nch = cfg.nVt * nq + nGp  # worst case incl. partial tiles
                    bnst = miscp.tile([128, nch * 6], f32, tag="bnst")
                    chn = 0
                    for t in range(cfg.nVt):
                        vsz = cfg.vts(t)
                        for qi0 in range(nq):
                            q0 = qi0 * 4
                            qn = min(4, nGp - q0)
                            pt = pstr.tile([128, 512], bf16, tag="tr")
                            for qi in range(qn):
                                gp = q0 + qi
                                nc.tensor.transpose(
                                    pt[:, qi * 128: qi * 128 + vsz],
                                    ytile[:vsz, t * BF + gp * 128:
                                          t * BF + (gp + 1) * 128],
                                    ident_b[:vsz, :vsz])
                            reg = dstv[:, q0:q0 + qn, t * 128:t * 128 + vsz]
                            nc.scalar.activation(
                                reg,
                                pt[:].rearrange("p (q v) -> p q v", v=128)
                                [:, :qn, :vsz],
                                AF.Copy)
                            if vsz == 128:
                                # stats straight off the transpose PSUM tile
                                nc.vector.bn_stats(
                                    bnst[:, chn * 6:(chn + 1) * 6],
                                    pt[:, :qn * 128])
                                chn += 1
                            else:
                                for qi in range(qn):
                                    gp = q0 + qi
                                    nc.vector.bn_stats(
                                        bnst[:, chn * 6:(chn + 1) * 6],
                                        XFn[:, gp * V + t * 128:
                                            gp * V + t * 128 + vsz])
                                    chn += 1
                    aggr = miscp.tile([128, 2], f32, tag="aggr")
                    nc.vector.bn_aggr(
                        aggr[:], bnst[:, :chn * 6]
                        .rearrange("p (c s) -> p c s", s=6))
                    FD = nGp * V
                    part = miscp.tile([128, 2], f32, tag="part")
                    nc.vector.tensor_tensor(
                        out=part[:, 1:2], in0=aggr[:, 0:1], in1=aggr[:, 0:1],
                        op=ALU.mult)
                    nc.vector.tensor_tensor(
                        out=part[:, 1:2], in0=part[:, 1:2], in1=aggr[:, 1:2],
                        op=ALU.add)
                    nc.vector.tensor_scalar_mul(part[:, 1:2], part[:, 1:2],
                                                float(FD))
                    nc.vector.tensor_scalar_mul(part[:, 0:1], aggr[:, 0:1],
                                                float(FD))
                    pst = pslin.tile([128, 512], f32, tag="lin")
                    nc.tensor.matmul(pst[:1, :F], part[:, 0:1], sel_sb[F],
                                     start=True, stop=True)
                    nc.tensor.matmul(pst[:1, F:2 * F], part[:, 1:2],
                                     sel_sb[F], start=True, stop=True)
                    stats_l = miscp.tile([1, 2 * F], f32, tag="statl")
                    nc.vector.tensor_copy(stats_l[:], pst[:1, :2 * F])
                    bin_ = dramp.tile([1, 2 * F], f32, tag=f"arin{ar_idx}")
                    bout = dramp.tile([8, 2 * F], f32, tag=f"arout{ar_idx}")
                    nc.gpsimd.dma_start(bin_[:], stats_l[:])
                    nc.gpsimd.collective_compute(
                        "AllGather", ALU.bypass,
                        replica_groups=[list(range(NCORES))],
                        ins=[bin_.opt()], outs=[bout.opt()])
                    sg8 = miscp.tile([8, 2 * F], f32, tag="sg8")
                    nc.gpsimd.dma_start(sg8[:], bout[:])
                    psg = pslin.tile([128, 512], f32, tag="lin", name="psg")
                    nc.tensor.matmul(psg[:1, :2 * F], ones8[:, 0:1], sg8[:],
                                     start=True, stop=True)
                    n_g = float(B * V)
                    # tmp cols [0:F]=mu, [F:2F]=var->rstd ; st cols [0:F]=s, [F:2F]=t
                    st = miscp.tile([1, 2 * F], f32, tag="st")
                    tmp = miscp.tile([1, 2 * F], f32, tag="sttmp")
                    mu2 = miscp.tile([1, F], f32, tag="mu2")
                    nc.vector.tensor_scalar_mul(tmp[:, :2 * F], psg[:1, :2 * F],
                                                1.0 / n_g)
                    nc.vector.tensor_tensor(out=mu2[:], in0=tmp[:, 0:F],
                                            in1=tmp[:, 0:F], op=ALU.mult)
                    nc.vector.tensor_tensor(out=tmp[:, F:2 * F],
                                            in0=tmp[:, F:2 * F],
                                            in1=mu2[:], op=ALU.subtract)
                    nc.scalar.activation(tmp[:, F:2 * F], tmp[:, F:2 * F],
                                         AF.Sqrt, bias=eps_t[:])
                    nc.vector.reciprocal(tmp[:, F:2 * F], tmp[:, F:2 * F])
                    nc.vector.tensor_tensor(out=st[:, 0:F],
                                            in0=tmp[:, F:2 * F],
                                            in1=gb_sb[li][:, 0:F], op=ALU.mult)
                    nc.vector.tensor_tensor(out=mu2[:], in0=tmp[:, 0:F],
                                            in1=st[:, 0:F], op=ALU.mult)
                    nc.vector.tensor_tensor(out=st[:, F:2 * F],
                                            in0=gb_sb[li][:, F:2 * F],
                                            in1=mu2[:], op=ALU.subtract)
                    pss = pslin.tile([128, 512], f32, tag="lin", name="pss")
                    nc.tensor.transpose(pss[:2 * F, 0:1], st[:],
                                        ident_f[:1, :1])
                    stv = miscp.tile([128, 1], f32, tag="stv")
                    nc.vector.tensor_copy(stv[:2 * F, :], pss[:2 * F, 0:1])
                    # broadcast (s,t) to all 128 partitions via selT matmuls
                    psc = pslin.tile([128, 512], f32, tag="lin", name="psc")
                    nc.tensor.matmul(psc[:, 0:1], sT_sb[F][:2 * F, 0:128],
                                     stv[:2 * F, :], start=True, stop=True)
                    nc.tensor.matmul(psc[:, 1:2], sT_sb[F][:2 * F, 128:256],
                                     stv[:2 * F, :], start=True, stop=True)
                    stc = miscp.tile([128, 2], f32, tag=f"stc{ar_idx}")
                    nc.vector.tensor_copy(stc[:], psc[:, 0:2])
                    ar_idx += 1
                    # chunked scale+relu (+ up4 replication for next layer)
                    ncfg = CFGS[li + 1]
                    if ncfg.up4:
                        XFrep_cur = poolA.tile(
                            [128, ncfg.nG * ncfg.V], bf16, tag="A")
                        s_r = XFn[:].rearrange("p (g w) -> p g w", w=V)
                        d_r = XFrep_cur[:].rearrange(
                            "p (g w r) -> p g w r", w=V, r=4)
                    nap = max(1, nGp // 4)
                    for q0 in range(0, nGp, nap):
                        q1 = min(q0 + nap, nGp)
                        nc.scalar.activation(
                            XFn[:, q0 * V:q1 * V], XFn[:, q0 * V:q1 * V],
                            AF.Relu, scale=stc[:, 0:1], bias=stc[:, 1:2])
                        if ncfg.up4:
                            for r in range(4):
                                nc.vector.tensor_copy(
                                    d_r[:, q0:q1, :, r], s_r[:, q0:q1, :])
                    XF_cur = XFn
                else:
                    # --- stage output: ytile [v, fo*32+b] -> [b, v*3+fo] ---
                    for t in range(cfg.nVt):
                        pt = pstr.tile([128, 512], bf16, tag="tr")
                        nc.tensor.transpose(
                            pt[:96, :128],
                            ytile[:128, t * BF:(t + 1) * BF],
                            ident_b[:128, :128])
                        och = outp.tile([BL, 384], f32, tag="out")
                        for fo in range(3):
                            nc.scalar.activation(
                                och[:].rearrange("b (v f) -> b v f", f=3)
                                [:, :, fo],
                                pt[fo * 32:(fo + 1) * 32, :128],
                                AF.Copy)
                        nc.gpsimd.dma_start(
                            ydram[:, t * 384:(t + 1) * 384], och[:])

    nc.compile()
    return nc


def kernel(**inputs):
    import sys
    for p in ("/opt/trn_rl_repo", "/opt/trn_rl_repo/concourse"):
        if p not in sys.path:
            sys.path.insert(0, p)
    from concourse.bass_utils import run_bass_kernel_spmd

    host = _build_host(inputs)

    if "nc" not in _CACHE:
        _CACHE["nc"] = _build_nc()
    nc = _CACHE["nc"]

    in_maps = []
    for c in range(NCORES):
        m = {k: v for k, v in host.items() if k != "xT"}
        m["xT"] = np.ascontiguousarray(host["xT"][:, c * BL:(c + 1) * BL])
        in_maps.append(m)
    res = run_bass_kernel_spmd(nc, in_maps, core_ids=list(range(NCORES)))
    out = np.concatenate(
        [r["y"].reshape(BL, 1280, 3) for r in res.results], axis=0)
    return out.astype(np.float32)


if __name__ == "__main__":
    import reference as R
    inp = R.setup_inputs()
    inp = {k: np.asarray(v) for k, v in inp.items()}
    act = kernel(**inp)
    exp = np.asarray(R.reference(**inp))
    err = np.linalg.norm(act - exp) / np.linalg.norm(exp)
    print("Relative error:", err)


# revision 26
# speedup vs baseline: 1.2693x; 1.0234x over previous
"""Trainium2 Bass kernel for nn_Graph_CNN_Feat_Mesh (Chebyshev GNN decoder).

Strategy (per-core, data-parallel over batch B=256 -> 32/core):
  - All spmms are dense matmuls on the tensor engine (PE) in bf16:
      y = A + L @ (B + L @ (2C)),  A/B/C = feature-space linears of the input.
    L is densified on host; for up4-preceded layers the replication is folded
    into LU = L @ U (contracting the small pre-upsample vertex space).
  - B and A linear terms accumulate directly into the spmm PSUM.
  - Activations live in packed F-layout [(j,Fin) partitions, (b//G)*Vsp + v]
    between layers; the per-layer linear emits V-layout directly; one PE
    transpose per layer returns to F-layout.
  - BatchNorm (training mode, global batch stats) is exact: per-core partial
    sums are AllGather'd across the 8 cores in-kernel (cheaper than
    AllReduce) and summed locally with a K=8 ones-matmul; stats are taken
    per transpose-group so they finish with the last transpose; scale+relu
    is chunked so the next layer starts on early chunks.
  - Weights are host-pre-tiled into [128, *] monoliths and streamed with a
    handful of large DMAs on the gpsimd queue (25ns issue) in consumption
    order; the FC head runs in bf16 with fp32 PSUM.
"""

import numpy as np

B = 256
NCORES = 8
BL = B // NCORES  # 32
EPS = 1e-5

_CACHE = {}


def _split_W(W):
    W = np.asarray(W, np.float32)
    return W[:, 0::3], W[:, 1::3], W[:, 2::3]


def _dense_L(rows, cols, vals, V):
    L = np.zeros((V, V), np.float32)
    np.add.at(L, (np.asarray(rows), np.asarray(cols)), np.asarray(vals, np.float32))
    return L


def _tile128(a):
    """[S*128, N] -> [128, S*N] block-column layout (pad rows to mult of 128)."""
    a = np.asarray(a)
    S = (a.shape[0] + 127) // 128
    if a.shape[0] != S * 128:
        a = np.concatenate(
            [a, np.zeros((S * 128 - a.shape[0], a.shape[1]), a.dtype)], 0)
    return np.ascontiguousarray(
        a.reshape(S, 128, a.shape[1]).transpose(1, 0, 2).reshape(128, -1))


class _LCfg:
    def __init__(self, name, Vsp, V, Fin, Fout, up4, bn):
        self.name = name
        self.Vsp = Vsp      # source vertex space of C-linear (pre-up4)
        self.V = V          # output vertex count
        self.Fin = Fin
        self.Fout = Fout
        self.G = 128 // Fin          # batches packed on partitions at input
        self.nG = BL // self.G
        self.GF = self.G * Fout      # N of one B/C/A-linear matmul
        self.Gp = 128 // Fout if Fout in (32, 64) else None
        self.nGp = BL // self.Gp if self.Gp else None
        self.up4 = up4
        self.bn = bn
        self.nVt = (V + 127) // 128
        self.nVsp = (Vsp + 127) // 128
        self.BF = BL * Fout          # free width of V-layout per vtile

    def vts(self, t):
        return min(128, self.V - t * 128)

    def sps(self, s):
        return min(128, self.Vsp - s * 128)


CFGS = [
    _LCfg("c0", 80, 320, 64, 64, True, True),
    _LCfg("c1", 320, 320, 64, 32, False, True),
    _LCfg("c2", 320, 1280, 32, 32, True, True),
    _LCfg("c3", 1280, 1280, 32, 3, False, False),
]


def _wbd(W, G, Fin, Fout, which):
    """Block-diagonal rhs weight [128, G*Fout] for the fused linear.
    which: 'A' -> W0 - W2, 'B' -> W1, 'C' -> 2*W2.  col = j*Fout + c."""
    W0, W1, W2 = _split_W(W)
    M = {"A": W0 - W2, "B": W1, "C": 2.0 * W2}[which]  # [Fout, Fin]
    out = np.zeros((128, G * Fout), np.float32)
    for j in range(G):
        out[j * Fin:(j + 1) * Fin, j * Fout:(j + 1) * Fout] = M.T
    return out


# column offsets inside the packed weight blobs
_WOFF = {}
_off = 0
for _cfg in CFGS:
    for _w in "ABC":
        _WOFF[f"{_w}{_cfg.name}"] = (_off, _cfg.GF)
        _off += _cfg.GF
WPACK_N = _off  # bf16 pack cols

# f32 pack: fc1b | per-layer FD-scaled sel blocks (BN partial-sum reduce)
F32_FC1B = 0
F32_SEL = [4, 68, 100]   # selFD for bn layers 0,1,2 (widths 64,32,32)
F32PACK_N = 132
_BN_F = [64, 32, 32]
_BN_FD = [16 * 320, 8 * 320, 8 * 1280]
_BN_NG = [256 * 320, 256 * 320, 256 * 1280]


def _build_host(inputs):
    import ml_dtypes
    bf = ml_dtypes.bfloat16
    f32 = np.float32
    d = {}
    d["xT"] = np.ascontiguousarray(np.asarray(inputs["x"], f32).T).astype(bf)
    d["fc1wt"] = _tile128(
        np.asarray(inputs["fc1_w"], f32).T).astype(bf)           # [128, 16*512]
    d["fc1b"] = np.ascontiguousarray(
        np.asarray(inputs["fc1_b"], f32).reshape(4, 128).T)      # [128,4]
    # fc2: chunk-major (mc), then k-tile: [128, 16*1280]
    w2 = np.asarray(inputs["fc2_w"], f32).T                      # [512, 5120]
    blk = [w2[kt * 128:(kt + 1) * 128, mc * 1280:(mc + 1) * 1280]
           for mc in range(4) for kt in range(4)]
    d["fc2wt"] = np.ascontiguousarray(np.concatenate(blk, 1)).astype(bf)

    L1 = _dense_L(inputs["L1_rows"], inputs["L1_cols"], inputs["L1_vals"], 320)
    L2 = _dense_L(inputs["L2_rows"], inputs["L2_cols"], inputs["L2_vals"], 1280)
    U1 = np.repeat(np.eye(80, dtype=f32), 4, axis=0)    # [320, 80]
    U2 = np.repeat(np.eye(320, dtype=f32), 4, axis=0)   # [1280, 320]
    d["LU0"] = _tile128((L1 @ U1).T).astype(bf)         # [128, 320]
    d["LT1"] = _tile128(L1.T).astype(bf)                # [128, 3*320]
    d["LU2"] = _tile128((L2 @ U2).T).astype(bf)         # [128, 3*1280]
    d["LT2"] = _tile128(L2.T).astype(bf)                # [128, 10*1280]

    Wn = {"c0": "cl0_w", "c1": "cl1_w", "c2": "cl2_w", "c3": "cl3_w"}
    wall = np.zeros((128, WPACK_N), f32)
    for cfg in CFGS:
        W = np.asarray(inputs[Wn[cfg.name]], f32)
        for which in "ABC":
            o, n = _WOFF[f"{which}{cfg.name}"]
            wall[:, o:o + n] = _wbd(W, cfg.G, cfg.Fin, cfg.Fout, which)
    d["wall"] = wall.astype(bf)
    # b3 tiled over the (b, c) column layout of the last-layer PSUM: col = b*3+c
    d["b3row"] = np.ascontiguousarray(
        np.tile(np.asarray(inputs["cl3_b"], f32), BL)[None, :]).astype(bf)

    gbp = np.zeros((1, 256), f32)
    for i, (g, b, o, F) in enumerate([("bn0_g", "bn0_b", 0, 64),
                                      ("bn1_g", "bn1_b", 128, 32),
                                      ("bn2_g", "bn2_b", 192, 32)]):
        gbp[0, o:o + F] = np.asarray(inputs[g], f32)
        gbp[0, o + F:o + 2 * F] = np.asarray(inputs[b], f32)
    d["gbpack"] = gbp

    f32p = np.zeros((128, F32PACK_N), f32)
    for li in range(3):
        F, o = _BN_F[li], F32_SEL[li]
        for j in range(128 // F):
            f32p[j * F:(j + 1) * F, o:o + F] += _BN_FD[li] * np.eye(F, dtype=f32)
    f32p[:, F32_FC1B:F32_FC1B + 4] = d.pop("fc1b")
    d["f32pack"] = f32p
    # selT_s/selT_t [2F rows, 128]: stc[p,:] = (s[p%F], t[p%F]) via 2 matmuls
    stp = np.zeros((128, 2 * 128), f32)
    for F, ro in [(64, 0), (32, 0)]:
        pass
    sT = np.zeros((128, 256), f32)   # rows k (2F<=128), cols: [0:128]=s-map, [128:256]=t-map
    # build per-F maps stacked by row-offset: F=64 uses rows 0:128, F=32 uses rows 0:64
    sT64 = np.zeros((128, 256), f32)
    for p in range(128):
        sT64[p % 64, p] = 1.0          # k = c        -> s
        sT64[64 + p % 64, 128 + p] = 1.0  # k = F + c  -> t
    sT32 = np.zeros((64, 256), f32)
    for p in range(128):
        sT32[p % 32, p] = 1.0
        sT32[32 + p % 32, 128 + p] = 1.0
    d["selT64"] = sT64
    d["selT32"] = np.concatenate([sT32, np.zeros((64, 256), f32)], 0)
    return d


def _build_nc():
    import sys
    for p in ("/opt/trn_rl_repo", "/opt/trn_rl_repo/concourse"):
        if p not in sys.path:
            sys.path.insert(0, p)
    import concourse.bass as bass  # noqa
    import concourse.mybir as mybir
    import concourse.tile as tile
    from concourse import bacc
    from concourse.masks import make_identity

    f32 = mybir.dt.float32
    bf16 = mybir.dt.bfloat16
    AF = mybir.ActivationFunctionType
    ALU = mybir.AluOpType

    nc = bacc.Bacc(None, target_bir_lowering=False)

    xT = nc.dram_tensor("xT", [2048, BL], bf16, kind="ExternalInput")
    fc1wt = nc.dram_tensor("fc1wt", [128, 16 * 512], bf16, kind="ExternalInput")
    fc2wt = nc.dram_tensor("fc2wt", [128, 16 * 1280], bf16, kind="ExternalInput")
    LU0 = nc.dram_tensor("LU0", [128, 320], bf16, kind="ExternalInput")
    LT1 = nc.dram_tensor("LT1", [128, 3 * 320], bf16, kind="ExternalInput")
    LU2 = nc.dram_tensor("LU2", [128, 3 * 1280], bf16, kind="ExternalInput")
    LT2 = nc.dram_tensor("LT2", [128, 10 * 1280], bf16, kind="ExternalInput")
    wall = nc.dram_tensor("wall", [128, WPACK_N], bf16, kind="ExternalInput")
    gbpack = nc.dram_tensor("gbpack", [1, 256], f32, kind="ExternalInput")
    f32pack = nc.dram_tensor("f32pack", [128, F32PACK_N], f32, kind="ExternalInput")
    selT64 = nc.dram_tensor("selT64", [128, 256], f32, kind="ExternalInput")
    selT32 = nc.dram_tensor("selT32", [128, 256], f32, kind="ExternalInput")
    b3row = nc.dram_tensor("b3row", [1, 96], bf16, kind="ExternalInput")
    ydram = nc.dram_tensor("y", [BL, 1280 * 3], f32, kind="ExternalOutput")

    with tile.TileContext(nc) as tc:
        with (
            tc.tile_pool(name="const", bufs=1) as constp,
            tc.tile_pool(name="wpool", bufs=1) as wpool,
            tc.tile_pool(name="poolA", bufs=2) as poolA,
            tc.tile_pool(name="poolB", bufs=2) as poolB,
            tc.tile_pool(name="poolC", bufs=1) as poolC,
            tc.tile_pool(name="misc", bufs=1) as miscp,
            tc.tile_pool(name="outp", bufs=3) as outp,
            tc.tile_pool(name="pslin", bufs=2, space="PSUM") as pslin,
            tc.tile_pool(name="psbig", bufs=2, space="PSUM") as psbig,
            tc.tile_pool(name="pstr", bufs=2, space="PSUM") as pstr,
            tc.tile_pool(name="dram", bufs=1, space="DRAM") as dramp,
        ):
            # ---- fc1 inputs first: these DMAs gate the first matmul ----
            xT_sb = miscp.tile([128, 16 * BL], bf16, tag="xT")
            nc.gpsimd.dma_start(
                xT_sb[:].rearrange("p (k b) -> p k b", b=BL),
                xT[:].rearrange("(k p) b -> p k b", p=128))
            f32_sb = constp.tile([128, F32PACK_N], f32, tag="f32pack")
            nc.gpsimd.dma_start(f32_sb[:], f32pack[:])
            fc1w_sb = poolA.tile([128, 16 * 512], bf16, tag="A")
            for kc in range(4):
                nc.gpsimd.dma_start(
                    fc1w_sb[:, kc * 4 * 512:(kc + 1) * 4 * 512],
                    fc1wt[:, kc * 4 * 512:(kc + 1) * 4 * 512])
            selfd_sb = [f32_sb[:, F32_SEL[li]:F32_SEL[li] + _BN_F[li]]
                        for li in range(3)]
            fc1b_sb = f32_sb[:, F32_FC1B:F32_FC1B + 4]

            # ---- small constants (no DMA) ----
            ident_b = constp.tile([128, 128], bf16, tag="identb")
            make_identity(nc, ident_b[:])
            ident_f = constp.tile([1, 1], f32, tag="identf")
            nc.gpsimd.memset(ident_f[:], 1.0)
            eps_t = constp.tile([1, 1], f32, tag="eps")
            nc.gpsimd.memset(eps_t[:], EPS)
            onesn = constp.tile([8, 3], f32, tag="onesn")
            for li in range(3):
                nc.gpsimd.memset(onesn[:, li:li + 1], 1.0 / _BN_NG[li])
            onesv = constp.tile([1, 128], bf16, tag="onesv")
            nc.gpsimd.memset(onesv[:], 1.0)
            sq_warm = constp.tile([1, 1], f32, tag="sqwarm")
            nc.scalar.activation(sq_warm[:], eps_t[:], AF.Sqrt, bias=eps_t[:])

            # ================= FC head (bf16, fp32 psum) =================
            # kt-outer so matmuls start as soon as the first fc1w chunk lands
            h1T = miscp.tile([128, 4 * BL], bf16, tag="h1T")
            ps1 = pslin.tile([128, 4 * BL], f32, tag="lin")
            for mt in range(4):
                for kt in range(16):
                    nc.tensor.matmul(
                        ps1[:, mt * BL:(mt + 1) * BL],
                        fc1w_sb[:, kt * 512 + mt * 128: kt * 512 + (mt + 1) * 128],
                        xT_sb[:, kt * BL:(kt + 1) * BL],
                        start=(kt == 0), stop=(kt == 15))
                nc.scalar.activation(
                    h1T[:, mt * BL:(mt + 1) * BL], ps1[:, mt * BL:(mt + 1) * BL],
                    AF.Relu, bias=fc1b_sb[:, mt:mt + 1])

            # ---- mid-priority loads (small; needed for c0/c1 + BN) ----
            gb_all = constp.tile([1, 256], f32, tag="gbp")
            nc.gpsimd.dma_start(gb_all[:], gbpack[:])
            gb_sb = [gb_all[:, 0:128], gb_all[:, 128:192], gb_all[:, 192:256]]
            sT_sb = {64: constp.tile([128, 256], f32, tag="sT64", name="sT64sb"),
                     32: constp.tile([128, 256], f32, tag="sT32", name="sT32sb")}
            nc.gpsimd.dma_start(sT_sb[64][:], selT64[:])
            nc.gpsimd.dma_start(sT_sb[32][:], selT32[:])
            b3_sb = constp.tile([1, 96], bf16, tag="b3row")
            nc.gpsimd.dma_start(b3_sb[:], b3row[:])

            LUT, LTd = {}, {}
            t = wpool.tile([128, 320], bf16, tag="LU0")
            nc.gpsimd.dma_start(t[:], LU0[:])
            LUT["c0"] = t
            t = wpool.tile([128, 3 * 320], bf16, tag="LT1")
            nc.gpsimd.dma_start(t[:], LT1[:])
            LTd["c0"] = LTd["c1"] = LUT["c1"] = t
            wall_sb = wpool.tile([128, WPACK_N], bf16, tag="wall")
            nc.gpsimd.dma_start(wall_sb[:], wall[:])
            W_sb = {}
            for cfg in CFGS:
                for w in "ABC":
                    o, n = _WOFF[f"{w}{cfg.name}"]
                    W_sb[f"{w}{cfg.name}"] = wall_sb[:, o:o + n]

            # ================= fc2 (streamed in 4 column-chunks) =========
            # psum partition = (v0%2)*64+f, col = mi*BL+b ; channels c = v0*64+f.
            # dest: XF0[(b%2)*64+f, (b//2)*80 + v0],  v0 = 2*(mc*10+mi)+p0
            XF0 = poolC.tile([128, 16 * 80], bf16, tag="XF0")
            cfg0 = CFGS[0]
            XFrep0 = poolA.tile([128, cfg0.nG * cfg0.V], bf16, tag="A")
            s_r0 = XF0[:].rearrange("p (g w) -> p g w", w=80)
            d_r0 = XFrep0[:].rearrange("p (g w r) -> p g w r", w=80, r=4)
            for mc in range(4):
                wch = poolB.tile([128, 4 * 1280], bf16, tag="B")
                nc.gpsimd.dma_start(
                    wch[:], fc2wt[:, mc * 4 * 1280:(mc + 1) * 4 * 1280])
                ps2 = psbig.tile([128, 10 * BL], f32, tag="big")
                for mi in range(10):
                    for kt in range(4):
                        nc.tensor.matmul(
                            ps2[:, mi * BL:(mi + 1) * BL],
                            wch[:, kt * 1280 + mi * 128: kt * 1280 + (mi + 1) * 128],
                            h1T[:, kt * BL:(kt + 1) * BL],
                            start=(kt == 0), stop=(kt == 3))
                src4 = ps2[:].rearrange("p (i g j) -> p i g j", g=16, j=2)
                dst4 = XF0[:].rearrange("p (g u q) -> p g u q", u=40, q=2)
                for p0 in range(2):
                    for j in range(2):
                        nc.scalar.activation(
                            dst4[j * 64:(j + 1) * 64, :,
                                 mc * 10:(mc + 1) * 10, p0]
                            .rearrange("p g i -> p i g"),
                            src4[p0 * 64:(p0 + 1) * 64, :, :, j],
                            AF.Copy)
                # up4-replicate this chunk's w-range (w = v0 in [20mc, 20mc+20))
                for r in range(4):
                    nc.vector.tensor_copy(
                        d_r0[:, :, 20 * mc:20 * (mc + 1), r],
                        s_r0[:, :, 20 * mc:20 * (mc + 1)])

            # ---- big late loads (needed at c2; stream during c0/c1) ----
            t = wpool.tile([128, 3 * 1280], bf16, tag="LU2")
            nc.gpsimd.dma_start(t[:], LU2[:])
            LUT["c2"] = t
            t = wpool.tile([128, 10 * 1280], bf16, tag="LT2")
            nc.gpsimd.dma_start(t[:, :5 * 1280], LT2[:, :5 * 1280])
            nc.gpsimd.dma_start(t[:, 5 * 1280:], LT2[:, 5 * 1280:])
            LTd["c2"] = LTd["c3"] = LUT["c3"] = t

            # ================= cheby layers =================
            XF_cur = XF0
            XFrep_cur = XFrep0
            ar_idx = 0

            for li, cfg in enumerate(CFGS):
                V, Vsp, F = cfg.V, cfg.Vsp, cfg.Fout
                BF = cfg.BF
                last = cfg.name == "c3"
                XFrep = XFrep_cur if cfg.up4 else XF_cur

                # --- C linear (in Vsp space) ---
                XC = poolC.tile([128, cfg.nVsp * BL * F], bf16, tag="XC")
                gpack = max(1, 512 // cfg.GF)
                for s in range(cfg.nVsp):
                    ssz = cfg.sps(s)
                    for g0 in range(0, cfg.nG, gpack):
                        gn = min(gpack, cfg.nG - g0)
                        pc = pslin.tile([128, 512], f32, tag="lin")
                        for gi in range(gn):
                            g = g0 + gi
                            nc.tensor.matmul(
                                pc[:ssz, gi * cfg.GF:(gi + 1) * cfg.GF],
                                XF_cur[:, g * Vsp + s * 128:
                                       g * Vsp + s * 128 + ssz],
                                W_sb[f"C{cfg.name}"][:],
                                start=True, stop=True)
                        nc.scalar.activation(
                            XC[:ssz, s * BL * F + g0 * cfg.GF:
                               s * BL * F + (g0 + gn) * cfg.GF],
                            pc[:ssz, :gn * cfg.GF], AF.Copy)

                # --- inner = LU @ (2C) + B ;  y = L @ inner + A ---
                Xin = poolB.tile([128, cfg.nVt * BF], bf16, tag="B")
                ytile = poolC.tile([128, cfg.nVt * BF], bf16, tag="YT")
                for phase in range(2):
                    srcL = LUT[cfg.name] if phase == 0 else LTd[cfg.name]
                    nS = cfg.nVsp if phase == 0 else cfg.nVt
                    ssizes = ([cfg.sps(s) for s in range(nS)] if phase == 0
                              else [cfg.vts(s) for s in range(nS)])
                    rhs = XC if phase == 0 else Xin
                    rhs_w = BL * F if phase == 0 else BF
                    Wacc = W_sb[f"B{cfg.name}" if phase == 0 else f"A{cfg.name}"]
                    dst = Xin if phase == 0 else ytile
                    for t in range(cfg.nVt):
                        vsz = cfg.vts(t)
                        for pc0 in range(0, BF, 1024):
                            pw = min(1024, BF - pc0)
                            pi = psbig.tile([128, max(pw, 512)], f32, tag="big")
                            for nk in range(0, pw, 512):
                                n0 = pc0 + nk
                                n1 = min(n0 + 512, pc0 + pw)
                                for s in range(nS):
                                    ssz = ssizes[s]
                                    nc.tensor.matmul(
                                        pi[:vsz, n0 - pc0:n1 - pc0],
                                        srcL[:ssz, s * V + t * 128:
                                             s * V + t * 128 + vsz],
                                        rhs[:ssz, s * rhs_w + n0:
                                            s * rhs_w + n1],
                                        start=(s == 0), stop=False,
                                        skip_group_check=True)
                                for g in range(n0 // cfg.GF,
                                               (n1 + cfg.GF - 1) // cfg.GF):
                                    nc.tensor.matmul(
                                        pi[:vsz, g * cfg.GF - pc0:
                                           (g + 1) * cfg.GF - pc0],
                                        XFrep[:, g * V + t * 128:
                                              g * V + t * 128 + vsz],
                                        Wacc[:],
                                        start=False,
                                        stop=(not (last and phase == 1)),
                                        skip_group_check=True)
                                if last and phase == 1:
                                    # fold cl3 bias: += ones^T @ b3row
                                    nc.tensor.matmul(
                                        pi[:vsz, n0 - pc0:n1 - pc0],
                                        onesv[:1, :vsz],
                                        b3_sb[:1, n0:n1],
                                        start=False, stop=True,
                                        skip_group_check=True)
                            if last and phase == 1:
                                # reorder (b,fo) -> (fo,b) for output staging
                                nc.vector.tensor_copy(
                                    dst[:vsz, t * BF + pc0: t * BF + pc0 + pw]
                                    .rearrange("p (c b) -> p c b", b=BL),
                                    pi[:vsz, :pw]
                                    .rearrange("p (b c) -> p c b", c=3))
                            elif phase == 0:
                                nc.scalar.activation(
                                    dst[:vsz, t * BF + pc0: t * BF + pc0 + pw],
                                    pi[:vsz, :pw], AF.Copy)
                            else:
                                nc.vector.tensor_copy(
                                    dst[:vsz, t * BF + pc0: t * BF + pc0 + pw],
                                    pi[:vsz, :pw])

                if not last:
                    # --- back-transpose to packed F-layout; stats per group ---
                    Gp, nGp = cfg.Gp, cfg.nGp
                    nq = (nGp + 3) // 4
                    XFn = poolA.tile([128, nGp * V], bf16, tag="A")
                    dstv = XFn[:].rearrange("p (g v) -> p g v", v=V)
                    nch = cfg.nVt * nq + nGp  # worst case incl. partial tiles
                    bnst = miscp.tile([128, nch * 6], f32, tag="bnst")
                    chn = 0
                    for t in range(cfg.nVt):
                        vsz = cfg.vts(t)
                        for qi0 in range(nq):
                            q0 = qi0 * 4
                            qn = min(4, nGp - q0)
                            pt = pstr.tile([128, 512], bf16, tag="tr")
                            for qi in range(qn):
                                gp = q0 + qi
                                nc.tensor.transpose(
                                    pt[:, qi * 128: qi * 128 + vsz],
                                    ytile[:vsz, t * BF + gp * 128:
                                          t * BF + (gp + 1) * 128],
                                    ident_b[:vsz, :vsz])
                            reg = dstv[:, q0:q0 + qn, t * 128:t * 128 + vsz]
                            nc.scalar.activation(
                                reg,
                                pt[:].rearrange("p (q v) -> p q v", v=128)
                                [:, :qn, :vsz],
                                AF.Copy)
                            if vsz == 128:
                                # stats straight off the transpose PSUM tile
                                nc.vector.bn_stats(
                                    bnst[:, chn * 6:(chn + 1) * 6],
                                    pt[:, :qn * 128])
                                chn += 1
                            else:
                                for qi in range(qn):
                                    gp = q0 + qi
                                    nc.vector.bn_stats(
                                        bnst[:, chn * 6:(chn + 1) * 6],
                                        XFn[:, gp * V + t * 128:
                                            gp * V + t * 128 + vsz])
                                    chn += 1
                    aggr = miscp.tile([128, 2], f32, tag="aggr")
                    nc.vector.bn_aggr(
                        aggr[:], bnst[:, :chn * 6]
                        .rearrange("p (c s) -> p c s", s=6))
                    part = miscp.tile([128, 2], f32, tag="part")
                    nc.vector.tensor_tensor(
                        out=part[:, 1:2], in0=aggr[:, 0:1], in1=aggr[:, 0:1],
                        op=ALU.mult)
                    nc.vector.tensor_tensor(
                        out=part[:, 1:2], in0=part[:, 1:2], in1=aggr[:, 1:2],
                        op=ALU.add)
                    pst = pslin.tile([128, 512], f32, tag="lin")
                    nc.tensor.matmul(pst[:1, :F], aggr[:, 0:1], selfd_sb[li],
                                     start=True, stop=True)
                    nc.tensor.matmul(pst[:1, F:2 * F], part[:, 1:2],
                                     selfd_sb[li], start=True, stop=True)
                    stats_l = miscp.tile([1, 2 * F], f32, tag="statl")
                    nc.vector.tensor_copy(stats_l[:], pst[:1, :2 * F])
                    bin_ = dramp.tile([1, 2 * F], f32, tag=f"arin{ar_idx}")
                    bout = dramp.tile([8, 2 * F], f32, tag=f"arout{ar_idx}")
                    nc.gpsimd.dma_start(bin_[:], stats_l[:])
                    nc.gpsimd.collective_compute(
                        "AllGather", ALU.bypass,
                        replica_groups=[list(range(NCORES))],
                        ins=[bin_.opt()], outs=[bout.opt()])
                    sg8 = miscp.tile([8, 2 * F], f32, tag="sg8")
                    nc.gpsimd.dma_start(sg8[:], bout[:])
                    psg = pslin.tile([128, 512], f32, tag="lin", name="psg")
                    nc.tensor.matmul(psg[:1, :2 * F], onesn[:, li:li + 1],
                                     sg8[:], start=True, stop=True)
                    # psg = (mu, E[y^2]) ; st cols [0:F]=s, [F:2F]=t
                    stats_g = miscp.tile([1, 2 * F], f32, tag="statg")
                    nc.vector.tensor_copy(stats_g[:], psg[:1, :2 * F])
                    st = miscp.tile([1, 2 * F], f32, tag="st")
                    tmp = miscp.tile([1, 2 * F], f32, tag="sttmp")
                    mu2 = miscp.tile([1, F], f32, tag="mu2")
                    nc.vector.tensor_tensor(out=mu2[:], in0=stats_g[:, 0:F],
                                            in1=stats_g[:, 0:F], op=ALU.mult)
                    nc.vector.tensor_tensor(out=tmp[:, F:2 * F],
                                            in0=stats_g[:, F:2 * F],
                                            in1=mu2[:], op=ALU.subtract)
                    nc.scalar.activation(tmp[:, F:2 * F], tmp[:, F:2 * F],
                                         AF.Sqrt, bias=eps_t[:])
                    nc.vector.reciprocal(tmp[:, F:2 * F], tmp[:, F:2 * F])
                    nc.vector.tensor_tensor(out=st[:, 0:F],
                                            in0=tmp[:, F:2 * F],
                                            in1=gb_sb[li][:, 0:F], op=ALU.mult)
                    nc.vector.tensor_tensor(out=mu2[:], in0=stats_g[:, 0:F],
                                            in1=st[:, 0:F], op=ALU.mult)
                    nc.vector.tensor_tensor(out=st[:, F:2 * F],
                                            in0=gb_sb[li][:, F:2 * F],
                                            in1=mu2[:], op=ALU.subtract)
                    pss = pslin.tile([128, 512], f32, tag="lin", name="pss")
                    nc.tensor.transpose(pss[:2 * F, 0:1], st[:],
                                        ident_f[:1, :1])
                    stv = miscp.tile([128, 1], f32, tag="stv")
                    nc.vector.tensor_copy(stv[:2 * F, :], pss[:2 * F, 0:1])
                    # broadcast (s,t) to all 128 partitions via selT matmuls
                    psc = pslin.tile([128, 512], f32, tag="lin", name="psc")
                    nc.tensor.matmul(psc[:, 0:1], sT_sb[F][:2 * F, 0:128],
                                     stv[:2 * F, :], start=True, stop=True)
                    nc.tensor.matmul(psc[:, 1:2], sT_sb[F][:2 * F, 128:256],
                                     stv[:2 * F, :], start=True, stop=True)
                    stc = miscp.tile([128, 2], f32, tag=f"stc{ar_idx}")
                    nc.vector.tensor_copy(stc[:], psc[:, 0:2])
                    ar_idx += 1
                    # chunked scale+relu (+ up4 replication for next layer)
                    ncfg = CFGS[li + 1]
                    if ncfg.up4:
                        XFrep_cur = poolA.tile(
                            [128, ncfg.nG * ncfg.V], bf16, tag="A")
                        s_r = XFn[:].rearrange("p (g w) -> p g w", w=V)
                        d_r = XFrep_cur[:].rearrange(
                            "p (g w r) -> p g w r", w=V, r=4)
                    nap = max(1, nGp // 4)
                    for q0 in range(0, nGp, nap):
                        q1 = min(q0 + nap, nGp)
                        nc.scalar.activation(
                            XFn[:, q0 * V:q1 * V], XFn[:, q0 * V:q1 * V],
                            AF.Relu, scale=stc[:, 0:1], bias=stc[:, 1:2])
                        if ncfg.up4:
                            for r in range(4):
                                nc.vector.tensor_copy(
                                    d_r[:, q0:q1, :, r], s_r[:, q0:q1, :])
                    XF_cur = XFn
                else:
                    # --- stage output: ytile [v, fo*32+b] -> [b, v*3+fo] ---
                    for t in range(cfg.nVt):
                        pt = pstr.tile([128, 512], bf16, tag="tr")
                        nc.tensor.transpose(
                            pt[:96, :128],
                            ytile[:128, t * BF:(t + 1) * BF],
                            ident_b[:128, :128])
                        och = outp.tile([BL, 384], f32, tag="out")
                        for fo in range(3):
                            nc.scalar.activation(
                                och[:].rearrange("b (v f) -> b v f", f=3)
                                [:, :, fo],
                                pt[fo * 32:(fo + 1) * 32, :128],
                                AF.Copy)
                        nc.sync.dma_start(
                            ydram[:, t * 384:(t + 1) * 384], och[:])

    nc.compile()
    return nc


def kernel(**inputs):
    import sys
    for p in ("/opt/trn_rl_repo", "/opt/trn_rl_repo/concourse"):
        if p not in sys.path:
            sys.path.insert(0, p)
    from concourse.bass_utils import run_bass_kernel_spmd

    host = _build_host(inputs)

    if "nc" not in _CACHE:
        _CACHE["nc"] = _build_nc()
    nc = _CACHE["nc"]

    in_maps = []
    for c in range(NCORES):
        m = {k: v for k, v in host.items() if k != "xT"}
        m["xT"] = np.ascontiguousarray(host["xT"][:, c * BL:(c + 1) * BL])
        in_maps.append(m)
    res = run_bass_kernel_spmd(nc, in_maps, core_ids=list(range(NCORES)))
    out = np.concatenate(
        [r["y"].reshape(BL, 1280, 3) for r in res.results], axis=0)
    return out.astype(np.float32)


if __name__ == "__main__":
    import reference as R
    inp = R.setup_inputs()
    inp = {k: np.asarray(v) for k, v in inp.items()}
    act = kernel(**inp)
    exp = np.asarray(R.reference(**inp))
    err = np.linalg.norm(act - exp) / np.linalg.norm(exp)
    print("Relative error:", err)


# revision 39
# speedup vs baseline: 1.4658x; 1.1548x over previous
"""Trainium2 Bass kernel for nn_Graph_CNN_Feat_Mesh (Chebyshev GNN decoder).

Strategy (per-core, data-parallel over batch B=256 -> 32/core):
  - All spmms are dense matmuls on the tensor engine (PE) in bf16:
      y = A + L @ (B + L @ (2C)),  A/B/C = feature-space linears of the input.
    L is densified on host; for up4-preceded layers the replication is folded
    into LU = L @ U (contracting the small pre-upsample vertex space).
  - B and A linear terms accumulate directly into the spmm PSUM.
  - Activations live in packed F-layout [(j,Fin) partitions, (b//G)*Vsp + v]
    between layers; the per-layer linear emits V-layout directly; one PE
    transpose per layer returns to F-layout.
  - BatchNorm (training mode, global batch stats) is exact: per-core partial
    sums are AllGather'd across the 8 cores in-kernel (cheaper than
    AllReduce) and summed locally with a K=8 ones-matmul; stats are taken
    per transpose-group so they finish with the last transpose; scale+relu
    is chunked so the next layer starts on early chunks.
  - Weights are host-pre-tiled into [128, *] monoliths and streamed with a
    handful of large DMAs on the gpsimd queue (25ns issue) in consumption
    order; the FC head runs in bf16 with fp32 PSUM.
"""

import numpy as np

B = 256
NCORES = 8
BL = B // NCORES  # 32
EPS = 1e-5
USE_RDMA = False  # remote-DMA BN exchange: unsupported by the timing sim

_CACHE = {}


def _split_W(W):
    W = np.asarray(W, np.float32)
    return W[:, 0::3], W[:, 1::3], W[:, 2::3]


def _dense_L(rows, cols, vals, V):
    L = np.zeros((V, V), np.float32)
    np.add.at(L, (np.asarray(rows), np.asarray(cols)), np.asarray(vals, np.float32))
    return L


def _tile128(a):
    """[S*128, N] -> [128, S*N] block-column layout (pad rows to mult of 128)."""
    a = np.asarray(a)
    S = (a.shape[0] + 127) // 128
    if a.shape[0] != S * 128:
        a = np.concatenate(
            [a, np.zeros((S * 128 - a.shape[0], a.shape[1]), a.dtype)], 0)
    return np.ascontiguousarray(
        a.reshape(S, 128, a.shape[1]).transpose(1, 0, 2).reshape(128, -1))


class _LCfg:
    def __init__(self, name, Vsp, V, Fin, Fout, up4, bn):
        self.name = name
        self.Vsp = Vsp      # source vertex space of C-linear (pre-up4)
        self.V = V          # output vertex count
        self.Fin = Fin
        self.Fout = Fout
        self.G = 128 // Fin          # batches packed on partitions at input
        self.nG = BL // self.G
        self.GF = self.G * Fout      # N of one B/C/A-linear matmul
        self.Gp = 128 // Fout if Fout in (32, 64) else None
        self.nGp = BL // self.Gp if self.Gp else None
        self.up4 = up4
        self.bn = bn
        self.nVt = (V + 127) // 128
        self.nVsp = (Vsp + 127) // 128
        self.BF = BL * Fout          # free width of V-layout per vtile

    def vts(self, t):
        return min(128, self.V - t * 128)

    def sps(self, s):
        return min(128, self.Vsp - s * 128)


CFGS = [
    _LCfg("c0", 80, 320, 64, 64, True, True),
    _LCfg("c1", 320, 320, 64, 32, False, True),
    _LCfg("c2", 320, 1280, 32, 32, True, True),
    _LCfg("c3", 1280, 1280, 32, 3, False, False),
]


def _wbd(W, G, Fin, Fout, which):
    """Block-diagonal rhs weight [128, G*Fout] for the fused linear.
    which: 'A' -> W0 - W2, 'B' -> W1, 'C' -> 2*W2.  col = j*Fout + c."""
    W0, W1, W2 = _split_W(W)
    M = {"A": W0 - W2, "B": W1, "C": 2.0 * W2}[which]  # [Fout, Fin]
    out = np.zeros((128, G * Fout), np.float32)
    for j in range(G):
        out[j * Fin:(j + 1) * Fin, j * Fout:(j + 1) * Fout] = M.T
    return out


# column offsets inside the packed weight blobs
_WOFF = {}
_off = 0
for _cfg in CFGS:
    for _w in "ABC":
        _WOFF[f"{_w}{_cfg.name}"] = (_off, _cfg.GF)
        _off += _cfg.GF
WPACK_N = _off  # bf16 pack cols

# f32 pack: fc1b | per-layer FD-scaled sel blocks (BN partial-sum reduce)
F32_FC1B = 0
F32_SEL = [4, 68, 100]   # selFD for bn layers 0,1,2 (widths 64,32,32)
F32PACK_N = 132
_BN_F = [64, 32, 32]
_BN_FD = [16 * 320, 8 * 320, 8 * 1280]
_BN_NG = [256 * 320, 256 * 320, 256 * 1280]
# with equal per-core/per-group counts, global mu = sum of partition means
# scaled by FD/NG; same factor turns summed (mean^2+var) into E[y^2]
_BN_SCL = [fd / ng for fd, ng in zip(_BN_FD, _BN_NG)]


def _build_host(inputs):
    import ml_dtypes
    bf = ml_dtypes.bfloat16
    f32 = np.float32
    d = {}
    d["xT"] = np.ascontiguousarray(np.asarray(inputs["x"], f32).T).astype(bf)
    d["fc1wt"] = _tile128(
        np.asarray(inputs["fc1_w"], f32).T).astype(bf)           # [128, 16*512]
    d["fc1b"] = np.ascontiguousarray(
        np.asarray(inputs["fc1_b"], f32).reshape(4, 128).T)      # [128,4]
    # fc2: chunk-major (mc), then k-tile: [128, 16*1280]
    w2 = np.asarray(inputs["fc2_w"], f32).T                      # [512, 5120]
    blk = [w2[kt * 128:(kt + 1) * 128, mc * 1280:(mc + 1) * 1280]
           for mc in range(4) for kt in range(4)]
    d["fc2wt"] = np.ascontiguousarray(np.concatenate(blk, 1)).astype(bf)

    L1 = _dense_L(inputs["L1_rows"], inputs["L1_cols"], inputs["L1_vals"], 320)
    L2 = _dense_L(inputs["L2_rows"], inputs["L2_cols"], inputs["L2_vals"], 1280)
    U1 = np.repeat(np.eye(80, dtype=f32), 4, axis=0)    # [320, 80]
    U2 = np.repeat(np.eye(320, dtype=f32), 4, axis=0)   # [1280, 320]
    f8 = ml_dtypes.float8_e4m3
    d["LU0"] = _tile128((L1 @ U1).T).astype(bf)         # [128, 320]
    d["LT1"] = _tile128(L1.T).astype(bf)                # [128, 3*320]
    d["LU2"] = _tile128((L2 @ U2).T).astype(f8)         # [128, 3*1280] fp8
    d["LT2"] = _tile128(L2.T).astype(f8)                # [128, 10*1280] fp8

    Wn = {"c0": "cl0_w", "c1": "cl1_w", "c2": "cl2_w", "c3": "cl3_w"}
    wall = np.zeros((128, WPACK_N), f32)
    for cfg in CFGS:
        W = np.asarray(inputs[Wn[cfg.name]], f32)
        for which in "ABC":
            o, n = _WOFF[f"{which}{cfg.name}"]
            wall[:, o:o + n] = _wbd(W, cfg.G, cfg.Fin, cfg.Fout, which)
    d["wall"] = wall.astype(bf)
    # b3 tiled over the (b, c) column layout of the last-layer PSUM: col = b*3+c
    d["b3row"] = np.ascontiguousarray(
        np.tile(np.asarray(inputs["cl3_b"], f32), BL)[None, :]).astype(bf)

    gbp = np.zeros((1, 256), f32)
    for i, (g, b, o, F) in enumerate([("bn0_g", "bn0_b", 0, 64),
                                      ("bn1_g", "bn1_b", 128, 32),
                                      ("bn2_g", "bn2_b", 192, 32)]):
        gbp[0, o:o + F] = np.asarray(inputs[g], f32)
        gbp[0, o + F:o + 2 * F] = np.asarray(inputs[b], f32)
    d["gbpack"] = gbp

    f32p = np.zeros((128, F32PACK_N), f32)
    for li in range(3):
        F, o = _BN_F[li], F32_SEL[li]
        v = _BN_SCL[li] if USE_RDMA else _BN_FD[li]
        for j in range(128 // F):
            f32p[j * F:(j + 1) * F, o:o + F] += v * np.eye(F, dtype=f32)
    f32p[:, F32_FC1B:F32_FC1B + 4] = d.pop("fc1b")
    d["f32pack"] = f32p
    # selT_s/selT_t [2F rows, 128]: stc[p,:] = (s[p%F], t[p%F]) via 2 matmuls
    stp = np.zeros((128, 2 * 128), f32)
    for F, ro in [(64, 0), (32, 0)]:
        pass
    sT = np.zeros((128, 256), f32)   # rows k (2F<=128), cols: [0:128]=s-map, [128:256]=t-map
    # build per-F maps stacked by row-offset: F=64 uses rows 0:128, F=32 uses rows 0:64
    sT64 = np.zeros((128, 256), f32)
    for p in range(128):
        sT64[p % 64, p] = 1.0          # k = c        -> s
        sT64[64 + p % 64, 128 + p] = 1.0  # k = F + c  -> t
    sT32 = np.zeros((64, 256), f32)
    for p in range(128):
        sT32[p % 32, p] = 1.0
        sT32[32 + p % 32, 128 + p] = 1.0
    d["selT64"] = sT64
    d["selT32"] = np.concatenate([sT32, np.zeros((64, 256), f32)], 0)
    return d


def _build_nc():
    import sys
    for p in ("/opt/trn_rl_repo", "/opt/trn_rl_repo/concourse"):
        if p not in sys.path:
            sys.path.insert(0, p)
    import concourse.bass as bass  # noqa
    import concourse.mybir as mybir
    import concourse.tile as tile
    from concourse import bacc
    from concourse.masks import make_identity

    f32 = mybir.dt.float32
    bf16 = mybir.dt.bfloat16
    fp8 = mybir.dt.float8e4
    DR = mybir.MatmulPerfMode.DoubleRow
    AF = mybir.ActivationFunctionType
    ALU = mybir.AluOpType

    nc = bacc.Bacc(None, target_bir_lowering=False)

    xT = nc.dram_tensor("xT", [2048, BL], bf16, kind="ExternalInput")
    fc1wt = nc.dram_tensor("fc1wt", [128, 16 * 512], bf16, kind="ExternalInput")
    fc2wt = nc.dram_tensor("fc2wt", [128, 16 * 1280], bf16, kind="ExternalInput")
    LU0 = nc.dram_tensor("LU0", [128, 320], bf16, kind="ExternalInput")
    LT1 = nc.dram_tensor("LT1", [128, 3 * 320], bf16, kind="ExternalInput")
    LU2 = nc.dram_tensor("LU2", [128, 3 * 1280], fp8, kind="ExternalInput")
    LT2 = nc.dram_tensor("LT2", [128, 10 * 1280], fp8, kind="ExternalInput")
    wall = nc.dram_tensor("wall", [128, WPACK_N], bf16, kind="ExternalInput")
    gbpack = nc.dram_tensor("gbpack", [1, 256], f32, kind="ExternalInput")
    f32pack = nc.dram_tensor("f32pack", [128, F32PACK_N], f32, kind="ExternalInput")
    selT64 = nc.dram_tensor("selT64", [128, 256], f32, kind="ExternalInput")
    selT32 = nc.dram_tensor("selT32", [128, 256], f32, kind="ExternalInput")
    b3row = nc.dram_tensor("b3row", [1, 96], bf16, kind="ExternalInput")
    ydram = nc.dram_tensor("y", [BL, 1280 * 3], f32, kind="ExternalOutput")

    with tile.TileContext(nc) as tc:
        with (
            tc.tile_pool(name="const", bufs=1) as constp,
            tc.tile_pool(name="wpool", bufs=1) as wpool,
            tc.tile_pool(name="poolA", bufs=2) as poolA,
            tc.tile_pool(name="poolB", bufs=2) as poolB,
            tc.tile_pool(name="poolC", bufs=1) as poolC,
            tc.tile_pool(name="misc", bufs=1) as miscp,
            tc.tile_pool(name="outp", bufs=3) as outp,
            tc.tile_pool(name="pslin", bufs=2, space="PSUM") as pslin,
            tc.tile_pool(name="psbig", bufs=2, space="PSUM") as psbig,
            tc.tile_pool(name="pstr", bufs=2, space="PSUM") as pstr,
            tc.tile_pool(name="dram", bufs=1, space="DRAM") as dramp,
        ):
            # ---- fc1 inputs first: these DMAs gate the first matmul ----
            xT_sb = miscp.tile([128, 16 * BL], bf16, tag="xT")
            nc.gpsimd.dma_start(
                xT_sb[:].rearrange("p (k b) -> p k b", b=BL),
                xT[:].rearrange("(k p) b -> p k b", p=128))
            f32_sb = constp.tile([128, F32PACK_N], f32, tag="f32pack")
            nc.gpsimd.dma_start(f32_sb[:], f32pack[:])
            fc1w_sb = poolA.tile([128, 16 * 512], bf16, tag="A")
            for kc in range(4):
                nc.gpsimd.dma_start(
                    fc1w_sb[:, kc * 4 * 512:(kc + 1) * 4 * 512],
                    fc1wt[:, kc * 4 * 512:(kc + 1) * 4 * 512])
            selfd_sb = [f32_sb[:, F32_SEL[li]:F32_SEL[li] + _BN_F[li]]
                        for li in range(3)]
            fc1b_sb = f32_sb[:, F32_FC1B:F32_FC1B + 4]

            # ---- small constants (no DMA) ----
            ident_b = constp.tile([128, 128], bf16, tag="identb")
            make_identity(nc, ident_b[:])
            ident_f = constp.tile([1, 1], f32, tag="identf")
            nc.gpsimd.memset(ident_f[:], 1.0)
            eps_t = constp.tile([1, 1], f32, tag="eps")
            nc.gpsimd.memset(eps_t[:], EPS)
            onesn = constp.tile([8, 3], f32, tag="onesn")
            for li in range(3):
                nc.gpsimd.memset(onesn[:, li:li + 1], 1.0 / _BN_NG[li])
            onesv = constp.tile([1, 128], bf16, tag="onesv")
            nc.gpsimd.memset(onesv[:], 1.0)
            sq_warm = constp.tile([1, 1], f32, tag="sqwarm")
            nc.scalar.activation(sq_warm[:], eps_t[:], AF.Sqrt, bias=eps_t[:])

            # ================= FC head (bf16, fp32 psum) =================
            # kt-outer so matmuls start as soon as the first fc1w chunk lands
            h1T = miscp.tile([128, 4 * BL], bf16, tag="h1T")
            ps1 = pslin.tile([128, 4 * BL], f32, tag="lin")
            for mt in range(4):
                for kt in range(16):
                    nc.tensor.matmul(
                        ps1[:, mt * BL:(mt + 1) * BL],
                        fc1w_sb[:, kt * 512 + mt * 128: kt * 512 + (mt + 1) * 128],
                        xT_sb[:, kt * BL:(kt + 1) * BL],
                        start=(kt == 0), stop=(kt == 15))
                nc.scalar.activation(
                    h1T[:, mt * BL:(mt + 1) * BL], ps1[:, mt * BL:(mt + 1) * BL],
                    AF.Relu, bias=fc1b_sb[:, mt:mt + 1])

            # ---- mid-priority loads (small; needed for c0/c1 + BN) ----
            gb_all = constp.tile([1, 256], f32, tag="gbp")
            nc.gpsimd.dma_start(gb_all[:], gbpack[:])
            gb_sb = [gb_all[:, 0:128], gb_all[:, 128:192], gb_all[:, 192:256]]
            sT_sb = {64: constp.tile([128, 256], f32, tag="sT64", name="sT64sb"),
                     32: constp.tile([128, 256], f32, tag="sT32", name="sT32sb")}
            nc.gpsimd.dma_start(sT_sb[64][:], selT64[:])
            nc.gpsimd.dma_start(sT_sb[32][:], selT32[:])
            b3_sb = constp.tile([1, 96], bf16, tag="b3row")
            nc.gpsimd.dma_start(b3_sb[:], b3row[:])

            LUT, LTd = {}, {}
            t = wpool.tile([128, 320], bf16, tag="LU0")
            nc.gpsimd.dma_start(t[:], LU0[:])
            LUT["c0"] = t
            t = wpool.tile([128, 3 * 320], bf16, tag="LT1")
            nc.gpsimd.dma_start(t[:], LT1[:])
            LTd["c0"] = LTd["c1"] = LUT["c1"] = t
            wall_sb = wpool.tile([128, WPACK_N], bf16, tag="wall")
            nc.gpsimd.dma_start(wall_sb[:], wall[:])
            W_sb = {}
            for cfg in CFGS:
                for w in "ABC":
                    o, n = _WOFF[f"{w}{cfg.name}"]
                    W_sb[f"{w}{cfg.name}"] = wall_sb[:, o:o + n]

            # ================= fc2 (streamed in 4 column-chunks) =========
            # psum partition = (v0%2)*64+f, col = mi*BL+b ; channels c = v0*64+f.
            # dest: XF0[(b%2)*64+f, (b//2)*80 + v0],  v0 = 2*(mc*10+mi)+p0
            XF0 = poolC.tile([128, 16 * 80], bf16, tag="XF0")
            cfg0 = CFGS[0]
            XFrep0 = poolA.tile([128, cfg0.nG * cfg0.V], bf16, tag="A")
            s_r0 = XF0[:].rearrange("p (g w) -> p g w", w=80)
            d_r0 = XFrep0[:].rearrange("p (g w r) -> p g w r", w=80, r=4)
            for mc in range(4):
                wch = poolB.tile([128, 4 * 1280], bf16, tag="B")
                nc.gpsimd.dma_start(
                    wch[:], fc2wt[:, mc * 4 * 1280:(mc + 1) * 4 * 1280])
                ps2 = psbig.tile([128, 10 * BL], f32, tag="big")
                for mi in range(10):
                    for kt in range(4):
                        nc.tensor.matmul(
                            ps2[:, mi * BL:(mi + 1) * BL],
                            wch[:, kt * 1280 + mi * 128: kt * 1280 + (mi + 1) * 128],
                            h1T[:, kt * BL:(kt + 1) * BL],
                            start=(kt == 0), stop=(kt == 3))
                src4 = ps2[:].rearrange("p (i g j) -> p i g j", g=16, j=2)
                dst4 = XF0[:].rearrange("p (g u q) -> p g u q", u=40, q=2)
                for p0 in range(2):
                    for j in range(2):
                        nc.scalar.activation(
                            dst4[j * 64:(j + 1) * 64, :,
                                 mc * 10:(mc + 1) * 10, p0]
                            .rearrange("p g i -> p i g"),
                            src4[p0 * 64:(p0 + 1) * 64, :, :, j],
                            AF.Copy)
                # up4-replicate this chunk's w-range (w = v0 in [20mc, 20mc+20))
                for r in range(4):
                    nc.vector.tensor_copy(
                        d_r0[:, :, 20 * mc:20 * (mc + 1), r],
                        s_r0[:, :, 20 * mc:20 * (mc + 1)])

            # ---- big late loads (needed at c2; stream during c0/c1) ----
            t = wpool.tile([128, 3 * 1280], fp8, tag="LU2")
            nc.gpsimd.dma_start(t[:], LU2[:])
            LUT["c2"] = t
            t = wpool.tile([128, 10 * 1280], fp8, tag="LT2")
            nc.gpsimd.dma_start(t[:, :5 * 1280], LT2[:, :5 * 1280])
            nc.gpsimd.dma_start(t[:, 5 * 1280:], LT2[:, 5 * 1280:])
            LTd["c2"] = LTd["c3"] = LUT["c3"] = t

            # ================= cheby layers =================
            if USE_RDMA:
                rsem = nc.alloc_semaphore("bn_rsem")
                lsem = nc.alloc_semaphore("bn_lsem")
                rbufs = [constp.tile([128, 16], f32, tag=f"rbuf{i}",
                                     name=f"rbuf{i}")
                         for i in range(3)]
            XF_cur = XF0
            XFrep_cur = XFrep0
            ar_idx = 0

            for li, cfg in enumerate(CFGS):
                V, Vsp, F = cfg.V, cfg.Vsp, cfg.Fout
                BF = cfg.BF
                last = cfg.name == "c3"
                XFrep = XFrep_cur if cfg.up4 else XF_cur

                fp8sp = cfg.name in ("c2", "c3")
                sp_dt = fp8 if fp8sp else bf16
                # --- C linear (in Vsp space) ---
                XC = poolC.tile([128, cfg.nVsp * BL * F], sp_dt, tag="XC")
                gpack = max(1, 512 // cfg.GF)
                for s in range(cfg.nVsp):
                    ssz = cfg.sps(s)
                    for g0 in range(0, cfg.nG, gpack):
                        gn = min(gpack, cfg.nG - g0)
                        pc = pslin.tile([128, 512], f32, tag="lin")
                        for gi in range(gn):
                            g = g0 + gi
                            nc.tensor.matmul(
                                pc[:ssz, gi * cfg.GF:(gi + 1) * cfg.GF],
                                XF_cur[:, g * Vsp + s * 128:
                                       g * Vsp + s * 128 + ssz],
                                W_sb[f"C{cfg.name}"][:],
                                start=True, stop=True)
                        nc.scalar.activation(
                            XC[:ssz, s * BL * F + g0 * cfg.GF:
                               s * BL * F + (g0 + gn) * cfg.GF],
                            pc[:ssz, :gn * cfg.GF], AF.Copy)

                # --- inner = LU @ (2C) + B ;  y = L @ inner + A ---
                Xin = poolB.tile([128, cfg.nVt * BF], sp_dt, tag="B")
                ytile = poolC.tile([128, cfg.nVt * BF], bf16, tag="YT")
                for phase in range(2):
                    srcL = LUT[cfg.name] if phase == 0 else LTd[cfg.name]
                    nS = cfg.nVsp if phase == 0 else cfg.nVt
                    ssizes = ([cfg.sps(s) for s in range(nS)] if phase == 0
                              else [cfg.vts(s) for s in range(nS)])
                    rhs = XC if phase == 0 else Xin
                    rhs_w = BL * F if phase == 0 else BF
                    Wacc = W_sb[f"B{cfg.name}" if phase == 0 else f"A{cfg.name}"]
                    dst = Xin if phase == 0 else ytile
                    for t in range(cfg.nVt):
                        vsz = cfg.vts(t)
                        for pc0 in range(0, BF, 1024):
                            pw = min(1024, BF - pc0)
                            pi = psbig.tile([128, max(pw, 512)], f32, tag="big")
                            for nk in range(0, pw, 512):
                                n0 = pc0 + nk
                                n1 = min(n0 + 512, pc0 + pw)
                                if fp8sp:
                                    # fp8 DoubleRow: contract 2 s-tiles/pass
                                    srcr = srcL[:].rearrange(
                                        "p (s v) -> p s v", v=V)
                                    rhsr = rhs[:].rearrange(
                                        "p (s n) -> p s n", n=rhs_w)
                                    for s0 in range(0, nS, 2):
                                        if s0 + 1 < nS and \
                                                ssizes[s0 + 1] == 128:
                                            nc.tensor.matmul(
                                                pi[:vsz, n0 - pc0:n1 - pc0],
                                                srcr[:, s0:s0 + 2,
                                                     t * 128:t * 128 + vsz],
                                                rhsr[:, s0:s0 + 2, n0:n1],
                                                start=(s0 == 0), stop=False,
                                                skip_group_check=True,
                                                perf_mode=DR)
                                        else:
                                            for s in range(s0, min(s0 + 2,
                                                                   nS)):
                                                ssz = ssizes[s]
                                                nc.tensor.matmul(
                                                    pi[:vsz,
                                                       n0 - pc0:n1 - pc0],
                                                    srcL[:ssz, s * V + t * 128:
                                                         s * V + t * 128 + vsz],
                                                    rhs[:ssz, s * rhs_w + n0:
                                                        s * rhs_w + n1],
                                                    start=(s == 0), stop=False,
                                                    skip_group_check=True)
                                else:
                                    for s in range(nS):
                                        ssz = ssizes[s]
                                        nc.tensor.matmul(
                                            pi[:vsz, n0 - pc0:n1 - pc0],
                                            srcL[:ssz, s * V + t * 128:
                                                 s * V + t * 128 + vsz],
                                            rhs[:ssz, s * rhs_w + n0:
                                                s * rhs_w + n1],
                                            start=(s == 0), stop=False,
                                            skip_group_check=True)
                                for g in range(n0 // cfg.GF,
                                               (n1 + cfg.GF - 1) // cfg.GF):
                                    nc.tensor.matmul(
                                        pi[:vsz, g * cfg.GF - pc0:
                                           (g + 1) * cfg.GF - pc0],
                                        XFrep[:, g * V + t * 128:
                                              g * V + t * 128 + vsz],
                                        Wacc[:],
                                        start=False,
                                        stop=(not (last and phase == 1)),
                                        skip_group_check=True)
                                if last and phase == 1:
                                    # fold cl3 bias: += ones^T @ b3row
                                    nc.tensor.matmul(
                                        pi[:vsz, n0 - pc0:n1 - pc0],
                                        onesv[:1, :vsz],
                                        b3_sb[:1, n0:n1],
                                        start=False, stop=True,
                                        skip_group_check=True)
                            if last and phase == 1:
                                # reorder (b,fo) -> (fo,b) for output staging
                                nc.vector.tensor_copy(
                                    dst[:vsz, t * BF + pc0: t * BF + pc0 + pw]
                                    .rearrange("p (c b) -> p c b", b=BL),
                                    pi[:vsz, :pw]
                                    .rearrange("p (b c) -> p c b", c=3))
                            elif phase == 0:
                                nc.scalar.activation(
                                    dst[:vsz, t * BF + pc0: t * BF + pc0 + pw],
                                    pi[:vsz, :pw], AF.Copy)
                            else:
                                nc.vector.tensor_copy(
                                    dst[:vsz, t * BF + pc0: t * BF + pc0 + pw],
                                    pi[:vsz, :pw])

                if not last:
                    # --- back-transpose to packed F-layout; stats per group ---
                    Gp, nGp = cfg.Gp, cfg.nGp
                    nq = (nGp + 3) // 4
                    XFn = poolA.tile([128, nGp * V], bf16, tag="A")
                    dstv = XFn[:].rearrange("p (g v) -> p g v", v=V)
                    nch = cfg.nVt * nq + nGp  # worst case incl. partial tiles
                    bnst = miscp.tile([128, nch * 6], f32, tag="bnst")
                    chn = 0
                    for t in range(cfg.nVt):
                        vsz = cfg.vts(t)
                        for qi0 in range(nq):
                            q0 = qi0 * 4
                            qn = min(4, nGp - q0)
                            pt = pstr.tile([128, 512], bf16, tag="tr")
                            for qi in range(qn):
                                gp = q0 + qi
                                nc.tensor.transpose(
                                    pt[:, qi * 128: qi * 128 + vsz],
                                    ytile[:vsz, t * BF + gp * 128:
                                          t * BF + (gp + 1) * 128],
                                    ident_b[:vsz, :vsz])
                            reg = dstv[:, q0:q0 + qn, t * 128:t * 128 + vsz]
                            nc.scalar.activation(
                                reg,
                                pt[:].rearrange("p (q v) -> p q v", v=128)
                                [:, :qn, :vsz],
                                AF.Copy)
                            if vsz == 128:
                                # stats straight off the transpose PSUM tile
                                nc.vector.bn_stats(
                                    bnst[:, chn * 6:(chn + 1) * 6],
                                    pt[:, :qn * 128])
                                chn += 1
                            else:
                                for qi in range(qn):
                                    gp = q0 + qi
                                    nc.vector.bn_stats(
                                        bnst[:, chn * 6:(chn + 1) * 6],
                                        XFn[:, gp * V + t * 128:
                                            gp * V + t * 128 + vsz])
                                    chn += 1
                    aggr = miscp.tile([128, 2], f32, tag="aggr")
                    nc.vector.bn_aggr(
                        aggr[:], bnst[:, :chn * 6]
                        .rearrange("p (c s) -> p c s", s=6))
                    part = miscp.tile([128, 2], f32, tag="part")
                    if USE_RDMA and ar_idx > 0:
                        # prior layer's sends must have drained before reuse
                        nc.vector.wait_ge(lsem, 112 * ar_idx)
                    nc.vector.tensor_tensor(
                        out=part[:, 1:2], in0=aggr[:, 0:1], in1=aggr[:, 0:1],
                        op=ALU.mult)
                    nc.vector.tensor_tensor(
                        out=part[:, 1:2], in0=part[:, 1:2], in1=aggr[:, 1:2],
                        op=ALU.add)
                    if USE_RDMA:
                        nc.vector.tensor_copy(part[:, 0:1], aggr[:, 0:1])
                        rb = rbufs[ar_idx]
                        nc.vector.tensor_copy(rb[:, 0:2], part[:])
                        for k in range(1, 8):
                            nc.gpsimd.remote_dma_broadcast(
                                rb[:, 2 * k:2 * k + 2], part[:],
                                remote_sem=rsem, local_sem=lsem,
                                rdests=[(0, k) if i == k else None
                                        for i in range(8)])
                        nc.gpsimd.trigger_dma(count=None)
                        nc.vector.wait_ge(rsem, 14 * (ar_idx + 1))
                        nc.vector.tensor_tensor(
                            out=rb[:, 0:8], in0=rb[:, 0:8], in1=rb[:, 8:16],
                            op=ALU.add)
                        nc.vector.tensor_tensor(
                            out=rb[:, 0:4], in0=rb[:, 0:4], in1=rb[:, 4:8],
                            op=ALU.add)
                        nc.vector.tensor_tensor(
                            out=rb[:, 0:2], in0=rb[:, 0:2], in1=rb[:, 2:4],
                            op=ALU.add)
                        pst = pslin.tile([128, 512], f32, tag="lin")
                        nc.tensor.matmul(pst[:1, :F], rb[:, 0:1],
                                         selfd_sb[li], start=True, stop=True)
                        nc.tensor.matmul(pst[:1, F:2 * F], rb[:, 1:2],
                                         selfd_sb[li], start=True, stop=True)
                        stats_g = miscp.tile([1, 2 * F], f32, tag="statg")
                        nc.vector.tensor_copy(stats_g[:], pst[:1, :2 * F])
                    else:
                        pst = pslin.tile([128, 512], f32, tag="lin")
                        nc.tensor.matmul(pst[:1, :F], aggr[:, 0:1],
                                         selfd_sb[li], start=True, stop=True)
                        nc.tensor.matmul(pst[:1, F:2 * F], part[:, 1:2],
                                         selfd_sb[li], start=True, stop=True)
                        stats_l = miscp.tile([1, 2 * F], f32, tag="statl")
                        nc.vector.tensor_copy(stats_l[:], pst[:1, :2 * F])
                        bin_ = dramp.tile([1, 2 * F], f32, tag=f"arin{ar_idx}")
                        bout = dramp.tile([8, 2 * F], f32, tag=f"arout{ar_idx}")
                        nc.gpsimd.dma_start(bin_[:], stats_l[:])
                        nc.gpsimd.collective_compute(
                            "AllGather", ALU.bypass,
                            replica_groups=[list(range(NCORES))],
                            ins=[bin_.opt()], outs=[bout.opt()])
                        sg8 = miscp.tile([8, 2 * F], f32, tag="sg8")
                        nc.gpsimd.dma_start(sg8[:], bout[:])
                        psg = pslin.tile([128, 512], f32, tag="lin", name="psg")
                        nc.tensor.matmul(psg[:1, :2 * F], onesn[:, li:li + 1],
                                         sg8[:], start=True, stop=True)
                        stats_g = miscp.tile([1, 2 * F], f32, tag="statg")
                        nc.vector.tensor_copy(stats_g[:], psg[:1, :2 * F])
                    # stats_g = (mu, E[y^2]) ; st cols [0:F]=s, [F:2F]=t
                    st = miscp.tile([1, 2 * F], f32, tag="st")
                    tmp = miscp.tile([1, 2 * F], f32, tag="sttmp")
                    mu2 = miscp.tile([1, F], f32, tag="mu2")
                    nc.vector.tensor_tensor(out=mu2[:], in0=stats_g[:, 0:F],
                                            in1=stats_g[:, 0:F], op=ALU.mult)
                    nc.vector.tensor_tensor(out=tmp[:, F:2 * F],
                                            in0=stats_g[:, F:2 * F],
                                            in1=mu2[:], op=ALU.subtract)
                    nc.scalar.activation(tmp[:, F:2 * F], tmp[:, F:2 * F],
                                         AF.Sqrt, bias=eps_t[:])
                    nc.vector.reciprocal(tmp[:, F:2 * F], tmp[:, F:2 * F])
                    nc.vector.tensor_tensor(out=st[:, 0:F],
                                            in0=tmp[:, F:2 * F],
                                            in1=gb_sb[li][:, 0:F], op=ALU.mult)
                    nc.vector.tensor_tensor(out=mu2[:], in0=stats_g[:, 0:F],
                                            in1=st[:, 0:F], op=ALU.mult)
                    nc.vector.tensor_tensor(out=st[:, F:2 * F],
                                            in0=gb_sb[li][:, F:2 * F],
                                            in1=mu2[:], op=ALU.subtract)
                    pss = pslin.tile([128, 512], f32, tag="lin", name="pss")
                    nc.tensor.transpose(pss[:2 * F, 0:1], st[:],
                                        ident_f[:1, :1])
                    stv = miscp.tile([128, 1], f32, tag="stv")
                    nc.vector.tensor_copy(stv[:2 * F, :], pss[:2 * F, 0:1])
                    # broadcast (s,t) to all 128 partitions via selT matmuls
                    psc = pslin.tile([128, 512], f32, tag="lin", name="psc")
                    nc.tensor.matmul(psc[:, 0:1], sT_sb[F][:2 * F, 0:128],
                                     stv[:2 * F, :], start=True, stop=True)
                    nc.tensor.matmul(psc[:, 1:2], sT_sb[F][:2 * F, 128:256],
                                     stv[:2 * F, :], start=True, stop=True)
                    stc = miscp.tile([128, 2], f32, tag=f"stc{ar_idx}")
                    nc.vector.tensor_copy(stc[:], psc[:, 0:2])
                    ar_idx += 1
                    # chunked scale+relu (+ up4 replication for next layer)
                    ncfg = CFGS[li + 1]
                    if ncfg.up4:
                        XFrep_cur = poolA.tile(
                            [128, ncfg.nG * ncfg.V], bf16, tag="A")
                        s_r = XFn[:].rearrange("p (g w) -> p g w", w=V)
                        d_r = XFrep_cur[:].rearrange(
                            "p (g w r) -> p g w r", w=V, r=4)
                    nap = max(1, nGp // 4)
                    for q0 in range(0, nGp, nap):
                        q1 = min(q0 + nap, nGp)
                        nc.scalar.activation(
                            XFn[:, q0 * V:q1 * V], XFn[:, q0 * V:q1 * V],
                            AF.Relu, scale=stc[:, 0:1], bias=stc[:, 1:2])
                        if ncfg.up4:
                            for r in range(4):
                                nc.vector.tensor_copy(
                                    d_r[:, q0:q1, :, r], s_r[:, q0:q1, :])
                    XF_cur = XFn
                else:
                    # --- stage output: ytile [v, fo*32+b] -> [b, v*3+fo] ---
                    for t in range(cfg.nVt):
                        pt = pstr.tile([128, 512], bf16, tag="tr")
                        nc.tensor.transpose(
                            pt[:96, :128],
                            ytile[:128, t * BF:(t + 1) * BF],
                            ident_b[:128, :128])
                        och = outp.tile([BL, 384], f32, tag="out")
                        for fo in range(3):
                            nc.scalar.activation(
                                och[:].rearrange("b (v f) -> b v f", f=3)
                                [:, :, fo],
                                pt[fo * 32:(fo + 1) * 32, :128],
                                AF.Copy)
                        nc.sync.dma_start(
                            ydram[:, t * 384:(t + 1) * 384], och[:])

    nc.compile()
    return nc


def kernel(**inputs):
    import sys
    for p in ("/opt/trn_rl_repo", "/opt/trn_rl_repo/concourse"):
        if p not in sys.path:
            sys.path.insert(0, p)
    from concourse.bass_utils import run_bass_kernel_spmd

    host = _build_host(inputs)

    if "nc" not in _CACHE:
        _CACHE["nc"] = _build_nc()
    nc = _CACHE["nc"]

    in_maps = []
    for c in range(NCORES):
        m = {k: v for k, v in host.items() if k != "xT"}
        m["xT"] = np.ascontiguousarray(host["xT"][:, c * BL:(c + 1) * BL])
        in_maps.append(m)
    res = run_bass_kernel_spmd(nc, in_maps, core_ids=list(range(NCORES)))
    out = np.concatenate(
        [r["y"].reshape(BL, 1280, 3) for r in res.results], axis=0)
    return out.astype(np.float32)


if __name__ == "__main__":
    import reference as R
    inp = R.setup_inputs()
    inp = {k: np.asarray(v) for k, v in inp.items()}
    act = kernel(**inp)
    exp = np.asarray(R.reference(**inp))
    err = np.linalg.norm(act - exp) / np.linalg.norm(exp)
    print("Relative error:", err)


# revision 47
# speedup vs baseline: 1.5256x; 1.0408x over previous
"""Trainium2 Bass kernel for nn_Graph_CNN_Feat_Mesh (Chebyshev GNN decoder).

Strategy (per-core, data-parallel over batch B=256 -> 32/core):
  - All spmms are dense matmuls on the tensor engine (PE) in bf16:
      y = A + L @ (B + L @ (2C)),  A/B/C = feature-space linears of the input.
    L is densified on host; for up4-preceded layers the replication is folded
    into LU = L @ U (contracting the small pre-upsample vertex space).
  - B and A linear terms accumulate directly into the spmm PSUM.
  - Activations live in packed F-layout [(j,Fin) partitions, (b//G)*Vsp + v]
    between layers; the per-layer linear emits V-layout directly; one PE
    transpose per layer returns to F-layout.
  - BatchNorm (training mode, global batch stats) is exact: per-core partial
    sums are AllGather'd across the 8 cores in-kernel (cheaper than
    AllReduce) and summed locally with a K=8 ones-matmul; stats are taken
    per transpose-group so they finish with the last transpose; scale+relu
    is chunked so the next layer starts on early chunks.
  - Weights are host-pre-tiled into [128, *] monoliths and streamed with a
    handful of large DMAs on the gpsimd queue (25ns issue) in consumption
    order; the FC head runs in bf16 with fp32 PSUM.
"""

import numpy as np

B = 256
NCORES = 8
BL = B // NCORES  # 32
EPS = 1e-5
USE_RDMA = False  # remote-DMA BN exchange: unsupported by the timing sim

_CACHE = {}


def _split_W(W):
    W = np.asarray(W, np.float32)
    return W[:, 0::3], W[:, 1::3], W[:, 2::3]


def _dense_L(rows, cols, vals, V):
    L = np.zeros((V, V), np.float32)
    np.add.at(L, (np.asarray(rows), np.asarray(cols)), np.asarray(vals, np.float32))
    return L


def _tile128(a):
    """[S*128, N] -> [128, S*N] block-column layout (pad rows to mult of 128)."""
    a = np.asarray(a)
    S = (a.shape[0] + 127) // 128
    if a.shape[0] != S * 128:
        a = np.concatenate(
            [a, np.zeros((S * 128 - a.shape[0], a.shape[1]), a.dtype)], 0)
    return np.ascontiguousarray(
        a.reshape(S, 128, a.shape[1]).transpose(1, 0, 2).reshape(128, -1))


class _LCfg:
    def __init__(self, name, Vsp, V, Fin, Fout, up4, bn):
        self.name = name
        self.Vsp = Vsp      # source vertex space of C-linear (pre-up4)
        self.V = V          # output vertex count
        self.Fin = Fin
        self.Fout = Fout
        self.G = 128 // Fin          # batches packed on partitions at input
        self.nG = BL // self.G
        self.GF = self.G * Fout      # N of one B/C/A-linear matmul
        self.Gp = 128 // Fout if Fout in (32, 64) else None
        self.nGp = BL // self.Gp if self.Gp else None
        self.up4 = up4
        self.bn = bn
        self.nVt = (V + 127) // 128
        self.nVsp = (Vsp + 127) // 128
        self.BF = BL * Fout          # free width of V-layout per vtile

    def vts(self, t):
        return min(128, self.V - t * 128)

    def sps(self, s):
        return min(128, self.Vsp - s * 128)


CFGS = [
    _LCfg("c0", 80, 320, 64, 64, True, True),
    _LCfg("c1", 320, 320, 64, 32, False, True),
    _LCfg("c2", 320, 1280, 32, 32, True, True),
    _LCfg("c3", 1280, 1280, 32, 3, False, False),
]


def _wbd(W, G, Fin, Fout, which):
    """Block-diagonal rhs weight [128, G*Fout] for the fused linear.
    which: 'A' -> W0 - W2, 'B' -> W1, 'C' -> 2*W2.  col = j*Fout + c."""
    W0, W1, W2 = _split_W(W)
    M = {"A": W0 - W2, "B": W1, "C": 2.0 * W2}[which]  # [Fout, Fin]
    out = np.zeros((128, G * Fout), np.float32)
    for j in range(G):
        out[j * Fin:(j + 1) * Fin, j * Fout:(j + 1) * Fout] = M.T
    return out


# column offsets inside the packed weight blobs
_WOFF = {}
_off = 0
for _cfg in CFGS:
    for _w in "ABC":
        _WOFF[f"{_w}{_cfg.name}"] = (_off, _cfg.GF)
        _off += _cfg.GF
WPACK_N = _off  # bf16 pack cols

# f32 pack: fc1b | per-layer FD-scaled sel blocks (BN partial-sum reduce)
F32_FC1B = 0
F32_SEL = [4, 68, 100]   # selFD for bn layers 0,1,2 (widths 64,32,32)
F32PACK_N = 132
_BN_F = [64, 32, 32]
_BN_FD = [16 * 320, 8 * 320, 8 * 1280]
_BN_NG = [256 * 320, 256 * 320, 256 * 1280]
# with equal per-core/per-group counts, global mu = sum of partition means
# scaled by FD/NG; same factor turns summed (mean^2+var) into E[y^2]
_BN_SCL = [fd / ng for fd, ng in zip(_BN_FD, _BN_NG)]


def _build_host(inputs):
    import ml_dtypes
    bf = ml_dtypes.bfloat16
    f32 = np.float32
    d = {}
    d["xT"] = np.ascontiguousarray(np.asarray(inputs["x"], f32).T).astype(bf)
    d["fc1wt"] = _tile128(
        np.asarray(inputs["fc1_w"], f32).T).astype(bf)           # [128, 16*512]
    d["fc1b"] = np.ascontiguousarray(
        np.asarray(inputs["fc1_b"], f32).reshape(4, 128).T)      # [128,4]
    # fc2: chunk-major (mc), then k-tile: [128, 16*1280]
    w2 = np.asarray(inputs["fc2_w"], f32).T                      # [512, 5120]
    blk = [w2[kt * 128:(kt + 1) * 128, mc * 1280:(mc + 1) * 1280]
           for mc in range(4) for kt in range(4)]
    d["fc2wt"] = np.ascontiguousarray(np.concatenate(blk, 1)).astype(bf)

    L1 = _dense_L(inputs["L1_rows"], inputs["L1_cols"], inputs["L1_vals"], 320)
    L2 = _dense_L(inputs["L2_rows"], inputs["L2_cols"], inputs["L2_vals"], 1280)
    U1 = np.repeat(np.eye(80, dtype=f32), 4, axis=0)    # [320, 80]
    U2 = np.repeat(np.eye(320, dtype=f32), 4, axis=0)   # [1280, 320]
    f8 = ml_dtypes.float8_e4m3
    d["LU0"] = _tile128((L1 @ U1).T).astype(bf)         # [128, 320]
    d["LT1"] = _tile128(L1.T).astype(bf)                # [128, 3*320]
    d["LU2"] = _tile128((L2 @ U2).T).astype(f8)         # [128, 3*1280] fp8
    d["LT2"] = _tile128(L2.T).astype(f8)                # [128, 10*1280] fp8

    Wn = {"c0": "cl0_w", "c1": "cl1_w", "c2": "cl2_w", "c3": "cl3_w"}
    wall = np.zeros((128, WPACK_N), f32)
    for cfg in CFGS:
        W = np.asarray(inputs[Wn[cfg.name]], f32)
        for which in "ABC":
            o, n = _WOFF[f"{which}{cfg.name}"]
            wall[:, o:o + n] = _wbd(W, cfg.G, cfg.Fin, cfg.Fout, which)
    d["wall"] = wall.astype(bf)
    # b3 tiled over the (b, c) column layout of the last-layer PSUM: col = b*3+c
    d["b3row"] = np.ascontiguousarray(
        np.tile(np.asarray(inputs["cl3_b"], f32), BL)[None, :]).astype(bf)

    gbp = np.zeros((1, 256), f32)
    for i, (g, b, o, F) in enumerate([("bn0_g", "bn0_b", 0, 64),
                                      ("bn1_g", "bn1_b", 128, 32),
                                      ("bn2_g", "bn2_b", 192, 32)]):
        gbp[0, o:o + F] = np.asarray(inputs[g], f32)
        gbp[0, o + F:o + 2 * F] = np.asarray(inputs[b], f32)
    d["gbpack"] = gbp

    f32p = np.zeros((128, F32PACK_N), f32)
    for li in range(3):
        F, o = _BN_F[li], F32_SEL[li]
        v = _BN_SCL[li] if USE_RDMA else _BN_FD[li]
        for j in range(128 // F):
            f32p[j * F:(j + 1) * F, o:o + F] += v * np.eye(F, dtype=f32)
    f32p[:, F32_FC1B:F32_FC1B + 4] = d.pop("fc1b")
    d["f32pack"] = f32p
    # selT_s/selT_t [2F rows, 128]: stc[p,:] = (s[p%F], t[p%F]) via 2 matmuls
    stp = np.zeros((128, 2 * 128), f32)
    for F, ro in [(64, 0), (32, 0)]:
        pass
    sT = np.zeros((128, 256), f32)   # rows k (2F<=128), cols: [0:128]=s-map, [128:256]=t-map
    # build per-F maps stacked by row-offset: F=64 uses rows 0:128, F=32 uses rows 0:64
    sT64 = np.zeros((128, 256), f32)
    for p in range(128):
        sT64[p % 64, p] = 1.0          # k = c        -> s
        sT64[64 + p % 64, 128 + p] = 1.0  # k = F + c  -> t
    sT32 = np.zeros((64, 256), f32)
    for p in range(128):
        sT32[p % 32, p] = 1.0
        sT32[32 + p % 32, 128 + p] = 1.0
    d["selT64"] = sT64
    d["selT32"] = np.concatenate([sT32, np.zeros((64, 256), f32)], 0)
    return d


def _build_nc():
    import sys
    for p in ("/opt/trn_rl_repo", "/opt/trn_rl_repo/concourse"):
        if p not in sys.path:
            sys.path.insert(0, p)
    import concourse.bass as bass  # noqa
    import concourse.mybir as mybir
    import concourse.tile as tile
    from concourse import bacc
    from concourse.masks import make_identity

    f32 = mybir.dt.float32
    bf16 = mybir.dt.bfloat16
    fp8 = mybir.dt.float8e4
    DR = mybir.MatmulPerfMode.DoubleRow
    AF = mybir.ActivationFunctionType
    ALU = mybir.AluOpType

    nc = bacc.Bacc(None, target_bir_lowering=False)

    xT = nc.dram_tensor("xT", [2048, BL], bf16, kind="ExternalInput")
    fc1wt = nc.dram_tensor("fc1wt", [128, 16 * 512], bf16, kind="ExternalInput")
    fc2wt = nc.dram_tensor("fc2wt", [128, 16 * 1280], bf16, kind="ExternalInput")
    LU0 = nc.dram_tensor("LU0", [128, 320], bf16, kind="ExternalInput")
    LT1 = nc.dram_tensor("LT1", [128, 3 * 320], bf16, kind="ExternalInput")
    LU2 = nc.dram_tensor("LU2", [128, 3 * 1280], fp8, kind="ExternalInput")
    LT2 = nc.dram_tensor("LT2", [128, 10 * 1280], fp8, kind="ExternalInput")
    wall = nc.dram_tensor("wall", [128, WPACK_N], bf16, kind="ExternalInput")
    gbpack = nc.dram_tensor("gbpack", [1, 256], f32, kind="ExternalInput")
    f32pack = nc.dram_tensor("f32pack", [128, F32PACK_N], f32, kind="ExternalInput")
    selT64 = nc.dram_tensor("selT64", [128, 256], f32, kind="ExternalInput")
    selT32 = nc.dram_tensor("selT32", [128, 256], f32, kind="ExternalInput")
    b3row = nc.dram_tensor("b3row", [1, 96], bf16, kind="ExternalInput")
    ydram = nc.dram_tensor("y", [BL, 1280 * 3], bf16, kind="ExternalOutput")

    with tile.TileContext(nc) as tc:
        with (
            tc.tile_pool(name="const", bufs=1) as constp,
            tc.tile_pool(name="wpool", bufs=1) as wpool,
            tc.tile_pool(name="poolA", bufs=2) as poolA,
            tc.tile_pool(name="poolB", bufs=2) as poolB,
            tc.tile_pool(name="poolC", bufs=1) as poolC,
            tc.tile_pool(name="misc", bufs=1) as miscp,
            tc.tile_pool(name="outp", bufs=3) as outp,
            tc.tile_pool(name="pslin", bufs=2, space="PSUM") as pslin,
            tc.tile_pool(name="psbig", bufs=2, space="PSUM") as psbig,
            tc.tile_pool(name="pstr", bufs=2, space="PSUM") as pstr,
            tc.tile_pool(name="dram", bufs=1, space="DRAM") as dramp,
        ):
            # ---- fc1 inputs first: these DMAs gate the first matmul ----
            xT_sb = miscp.tile([128, 16 * BL], bf16, tag="xT")
            nc.gpsimd.dma_start(
                xT_sb[:].rearrange("p (k b) -> p k b", b=BL),
                xT[:].rearrange("(k p) b -> p k b", p=128))
            f32_sb = constp.tile([128, F32PACK_N], f32, tag="f32pack")
            nc.sync.dma_start(f32_sb[:], f32pack[:])
            fc1w_sb = poolA.tile([128, 16 * 512], bf16, tag="A")
            for kc in range(4):
                nc.gpsimd.dma_start(
                    fc1w_sb[:, kc * 4 * 512:(kc + 1) * 4 * 512],
                    fc1wt[:, kc * 4 * 512:(kc + 1) * 4 * 512])
            selfd_sb = [f32_sb[:, F32_SEL[li]:F32_SEL[li] + _BN_F[li]]
                        for li in range(3)]
            fc1b_sb = f32_sb[:, F32_FC1B:F32_FC1B + 4]

            # ---- small constants (no DMA) ----
            ident_b = constp.tile([128, 128], bf16, tag="identb")
            make_identity(nc, ident_b[:])
            ident_f = constp.tile([1, 1], f32, tag="identf")
            nc.gpsimd.memset(ident_f[:], 1.0)
            eps_t = constp.tile([1, 1], f32, tag="eps")
            nc.gpsimd.memset(eps_t[:], EPS)
            onesn = constp.tile([8, 3], f32, tag="onesn")
            for li in range(3):
                nc.gpsimd.memset(onesn[:, li:li + 1], 1.0 / _BN_NG[li])
            onesv = constp.tile([1, 128], bf16, tag="onesv")
            nc.gpsimd.memset(onesv[:], 1.0)
            sq_warm = constp.tile([1, 1], f32, tag="sqwarm")
            nc.scalar.activation(sq_warm[:], eps_t[:], AF.Sqrt, bias=eps_t[:])

            # ================= FC head (bf16, fp32 psum) =================
            # kt-outer so matmuls start as soon as the first fc1w chunk lands
            h1T = miscp.tile([128, 4 * BL], bf16, tag="h1T")
            ps1 = pslin.tile([128, 4 * BL], f32, tag="lin")
            for mt in range(4):
                for kt in range(16):
                    nc.tensor.matmul(
                        ps1[:, mt * BL:(mt + 1) * BL],
                        fc1w_sb[:, kt * 512 + mt * 128: kt * 512 + (mt + 1) * 128],
                        xT_sb[:, kt * BL:(kt + 1) * BL],
                        start=(kt == 0), stop=(kt == 15))
                nc.scalar.activation(
                    h1T[:, mt * BL:(mt + 1) * BL], ps1[:, mt * BL:(mt + 1) * BL],
                    AF.Relu, bias=fc1b_sb[:, mt:mt + 1])

            # ---- mid-priority loads (small; needed for c0/c1 + BN) ----
            gb_all = constp.tile([1, 256], f32, tag="gbp")
            nc.sync.dma_start(gb_all[:], gbpack[:])
            gb_sb = [gb_all[:, 0:128], gb_all[:, 128:192], gb_all[:, 192:256]]
            sT_sb = {64: constp.tile([128, 256], f32, tag="sT64", name="sT64sb"),
                     32: constp.tile([128, 256], f32, tag="sT32", name="sT32sb")}
            nc.sync.dma_start(sT_sb[64][:], selT64[:])
            nc.sync.dma_start(sT_sb[32][:], selT32[:])
            b3_sb = constp.tile([1, 96], bf16, tag="b3row")
            nc.sync.dma_start(b3_sb[:], b3row[:])

            LUT, LTd = {}, {}
            t = wpool.tile([128, 320], bf16, tag="LU0")
            nc.sync.dma_start(t[:], LU0[:])
            LUT["c0"] = t
            t = wpool.tile([128, 3 * 320], bf16, tag="LT1")
            nc.sync.dma_start(t[:], LT1[:])
            LTd["c0"] = LTd["c1"] = LUT["c1"] = t
            wall_sb = wpool.tile([128, WPACK_N], bf16, tag="wall")
            nc.sync.dma_start(wall_sb[:], wall[:])
            W_sb = {}
            for cfg in CFGS:
                for w in "ABC":
                    o, n = _WOFF[f"{w}{cfg.name}"]
                    W_sb[f"{w}{cfg.name}"] = wall_sb[:, o:o + n]

            # ================= fc2 (streamed in 4 column-chunks) =========
            # psum partition = (v0%2)*64+f, col = mi*BL+b ; channels c = v0*64+f.
            # dest: XF0[(b%2)*64+f, (b//2)*80 + v0],  v0 = 2*(mc*10+mi)+p0
            XF0 = poolC.tile([128, 16 * 80], bf16, tag="XF0")
            cfg0 = CFGS[0]
            XFrep0 = poolA.tile([128, cfg0.nG * cfg0.V], bf16, tag="A")
            s_r0 = XF0[:].rearrange("p (g w) -> p g w", w=80)
            d_r0 = XFrep0[:].rearrange("p (g w r) -> p g w r", w=80, r=4)
            for mc in range(4):
                wch = poolB.tile([128, 4 * 1280], bf16, tag="B")
                nc.gpsimd.dma_start(
                    wch[:], fc2wt[:, mc * 4 * 1280:(mc + 1) * 4 * 1280])
                ps2 = psbig.tile([128, 10 * BL], f32, tag="big")
                for mi in range(10):
                    for kt in range(4):
                        nc.tensor.matmul(
                            ps2[:, mi * BL:(mi + 1) * BL],
                            wch[:, kt * 1280 + mi * 128: kt * 1280 + (mi + 1) * 128],
                            h1T[:, kt * BL:(kt + 1) * BL],
                            start=(kt == 0), stop=(kt == 3))
                src4 = ps2[:].rearrange("p (i g j) -> p i g j", g=16, j=2)
                dst4 = XF0[:].rearrange("p (g u q) -> p g u q", u=40, q=2)
                for p0 in range(2):
                    for j in range(2):
                        nc.scalar.activation(
                            dst4[j * 64:(j + 1) * 64, :,
                                 mc * 10:(mc + 1) * 10, p0]
                            .rearrange("p g i -> p i g"),
                            src4[p0 * 64:(p0 + 1) * 64, :, :, j],
                            AF.Copy)
                # up4-replicate this chunk's w-range (w = v0 in [20mc, 20mc+20))
                for r in range(4):
                    nc.vector.tensor_copy(
                        d_r0[:, :, 20 * mc:20 * (mc + 1), r],
                        s_r0[:, :, 20 * mc:20 * (mc + 1)])

            # ---- big late loads (needed at c2; stream during c0/c1) ----
            t = wpool.tile([128, 3 * 1280], fp8, tag="LU2")
            nc.gpsimd.dma_start(t[:], LU2[:])
            LUT["c2"] = t
            t = wpool.tile([128, 10 * 1280], fp8, tag="LT2")
            nc.gpsimd.dma_start(t[:, :5 * 1280], LT2[:, :5 * 1280])
            nc.gpsimd.dma_start(t[:, 5 * 1280:], LT2[:, 5 * 1280:])
            LTd["c2"] = LTd["c3"] = LUT["c3"] = t

            # ================= cheby layers =================
            if USE_RDMA:
                rsem = nc.alloc_semaphore("bn_rsem")
                lsem = nc.alloc_semaphore("bn_lsem")
                rbufs = [constp.tile([128, 16], f32, tag=f"rbuf{i}",
                                     name=f"rbuf{i}")
                         for i in range(3)]
            XF_cur = XF0
            XFrep_cur = XFrep0
            ar_idx = 0

            for li, cfg in enumerate(CFGS):
                V, Vsp, F = cfg.V, cfg.Vsp, cfg.Fout
                BF = cfg.BF
                last = cfg.name == "c3"
                XFrep = XFrep_cur if cfg.up4 else XF_cur

                fp8sp = cfg.name in ("c2", "c3")
                sp_dt = fp8 if fp8sp else bf16
                # --- C linear (in Vsp space) ---
                XC = poolC.tile([128, cfg.nVsp * BL * F], sp_dt, tag="XC")
                gpack = max(1, 512 // cfg.GF)
                for s in range(cfg.nVsp):
                    ssz = cfg.sps(s)
                    for g0 in range(0, cfg.nG, gpack):
                        gn = min(gpack, cfg.nG - g0)
                        pc = pslin.tile([128, 512], f32, tag="lin")
                        for gi in range(gn):
                            g = g0 + gi
                            nc.tensor.matmul(
                                pc[:ssz, gi * cfg.GF:(gi + 1) * cfg.GF],
                                XF_cur[:, g * Vsp + s * 128:
                                       g * Vsp + s * 128 + ssz],
                                W_sb[f"C{cfg.name}"][:],
                                start=True, stop=True)
                        nc.scalar.activation(
                            XC[:ssz, s * BL * F + g0 * cfg.GF:
                               s * BL * F + (g0 + gn) * cfg.GF],
                            pc[:ssz, :gn * cfg.GF], AF.Copy)

                # --- inner = LU @ (2C) + B ;  y = L @ inner + A ---
                Xin = poolB.tile([128, cfg.nVt * BF], sp_dt, tag="B")
                ytile = poolC.tile([128, cfg.nVt * BF], bf16, tag="YT")
                for phase in range(2):
                    srcL = LUT[cfg.name] if phase == 0 else LTd[cfg.name]
                    nS = cfg.nVsp if phase == 0 else cfg.nVt
                    ssizes = ([cfg.sps(s) for s in range(nS)] if phase == 0
                              else [cfg.vts(s) for s in range(nS)])
                    rhs = XC if phase == 0 else Xin
                    rhs_w = BL * F if phase == 0 else BF
                    Wacc = W_sb[f"B{cfg.name}" if phase == 0 else f"A{cfg.name}"]
                    dst = Xin if phase == 0 else ytile
                    for t in range(cfg.nVt):
                        vsz = cfg.vts(t)
                        for pc0 in range(0, BF, 1024):
                            pw = min(1024, BF - pc0)
                            pi = psbig.tile([128, max(pw, 512)], f32, tag="big")
                            for nk in range(0, pw, 512):
                                n0 = pc0 + nk
                                n1 = min(n0 + 512, pc0 + pw)
                                if fp8sp:
                                    # fp8 DoubleRow: contract 2 s-tiles/pass
                                    srcr = srcL[:].rearrange(
                                        "p (s v) -> p s v", v=V)
                                    rhsr = rhs[:].rearrange(
                                        "p (s n) -> p s n", n=rhs_w)
                                    for s0 in range(0, nS, 2):
                                        if s0 + 1 < nS and \
                                                ssizes[s0 + 1] == 128:
                                            nc.tensor.matmul(
                                                pi[:vsz, n0 - pc0:n1 - pc0],
                                                srcr[:, s0:s0 + 2,
                                                     t * 128:t * 128 + vsz],
                                                rhsr[:, s0:s0 + 2, n0:n1],
                                                start=(s0 == 0), stop=False,
                                                skip_group_check=True,
                                                perf_mode=DR)
                                        else:
                                            for s in range(s0, min(s0 + 2,
                                                                   nS)):
                                                ssz = ssizes[s]
                                                nc.tensor.matmul(
                                                    pi[:vsz,
                                                       n0 - pc0:n1 - pc0],
                                                    srcL[:ssz, s * V + t * 128:
                                                         s * V + t * 128 + vsz],
                                                    rhs[:ssz, s * rhs_w + n0:
                                                        s * rhs_w + n1],
                                                    start=(s == 0), stop=False,
                                                    skip_group_check=True)
                                else:
                                    for s in range(nS):
                                        ssz = ssizes[s]
                                        nc.tensor.matmul(
                                            pi[:vsz, n0 - pc0:n1 - pc0],
                                            srcL[:ssz, s * V + t * 128:
                                                 s * V + t * 128 + vsz],
                                            rhs[:ssz, s * rhs_w + n0:
                                                s * rhs_w + n1],
                                            start=(s == 0), stop=False,
                                            skip_group_check=True)
                                for g in range(n0 // cfg.GF,
                                               (n1 + cfg.GF - 1) // cfg.GF):
                                    nc.tensor.matmul(
                                        pi[:vsz, g * cfg.GF - pc0:
                                           (g + 1) * cfg.GF - pc0],
                                        XFrep[:, g * V + t * 128:
                                              g * V + t * 128 + vsz],
                                        Wacc[:],
                                        start=False,
                                        stop=(not (last and phase == 1)),
                                        skip_group_check=True)
                                if last and phase == 1:
                                    # fold cl3 bias: += ones^T @ b3row
                                    nc.tensor.matmul(
                                        pi[:vsz, n0 - pc0:n1 - pc0],
                                        onesv[:1, :vsz],
                                        b3_sb[:1, n0:n1],
                                        start=False, stop=True,
                                        skip_group_check=True)
                            if last and phase == 1:
                                # reorder (b,fo) -> (fo,b) for output staging
                                nc.vector.tensor_copy(
                                    dst[:vsz, t * BF + pc0: t * BF + pc0 + pw]
                                    .rearrange("p (c b) -> p c b", b=BL),
                                    pi[:vsz, :pw]
                                    .rearrange("p (b c) -> p c b", c=3))
                            elif phase == 0:
                                nc.scalar.activation(
                                    dst[:vsz, t * BF + pc0: t * BF + pc0 + pw],
                                    pi[:vsz, :pw], AF.Copy)
                            else:
                                nc.vector.tensor_copy(
                                    dst[:vsz, t * BF + pc0: t * BF + pc0 + pw],
                                    pi[:vsz, :pw])

                if not last:
                    # --- back-transpose to packed F-layout; stats per group ---
                    Gp, nGp = cfg.Gp, cfg.nGp
                    nq = (nGp + 3) // 4
                    XFn = poolA.tile([128, nGp * V], bf16, tag="A")
                    dstv = XFn[:].rearrange("p (g v) -> p g v", v=V)
                    nch = cfg.nVt * nq + nGp  # worst case incl. partial tiles
                    bnst = miscp.tile([128, nch * 6], f32, tag="bnst")
                    chn = 0
                    for t in range(cfg.nVt):
                        vsz = cfg.vts(t)
                        for qi0 in range(nq):
                            q0 = qi0 * 4
                            qn = min(4, nGp - q0)
                            pt = pstr.tile([128, 512], bf16, tag="tr")
                            for qi in range(qn):
                                gp = q0 + qi
                                nc.tensor.transpose(
                                    pt[:, qi * 128: qi * 128 + vsz],
                                    ytile[:vsz, t * BF + gp * 128:
                                          t * BF + (gp + 1) * 128],
                                    ident_b[:vsz, :vsz])
                            reg = dstv[:, q0:q0 + qn, t * 128:t * 128 + vsz]
                            nc.scalar.activation(
                                reg,
                                pt[:].rearrange("p (q v) -> p q v", v=128)
                                [:, :qn, :vsz],
                                AF.Copy)
                            if vsz == 128:
                                # stats straight off the transpose PSUM tile
                                nc.vector.bn_stats(
                                    bnst[:, chn * 6:(chn + 1) * 6],
                                    pt[:, :qn * 128])
                                chn += 1
                            else:
                                for qi in range(qn):
                                    gp = q0 + qi
                                    nc.vector.bn_stats(
                                        bnst[:, chn * 6:(chn + 1) * 6],
                                        XFn[:, gp * V + t * 128:
                                            gp * V + t * 128 + vsz])
                                    chn += 1
                    aggr = miscp.tile([128, 2], f32, tag="aggr")
                    nc.vector.bn_aggr(
                        aggr[:], bnst[:, :chn * 6]
                        .rearrange("p (c s) -> p c s", s=6))
                    part = miscp.tile([128, 2], f32, tag="part")
                    if USE_RDMA and ar_idx > 0:
                        # prior layer's sends must have drained before reuse
                        nc.vector.wait_ge(lsem, 112 * ar_idx)
                    nc.vector.tensor_tensor(
                        out=part[:, 1:2], in0=aggr[:, 0:1], in1=aggr[:, 0:1],
                        op=ALU.mult)
                    nc.vector.tensor_tensor(
                        out=part[:, 1:2], in0=part[:, 1:2], in1=aggr[:, 1:2],
                        op=ALU.add)
                    if USE_RDMA:
                        nc.vector.tensor_copy(part[:, 0:1], aggr[:, 0:1])
                        rb = rbufs[ar_idx]
                        nc.vector.tensor_copy(rb[:, 0:2], part[:])
                        for k in range(1, 8):
                            nc.gpsimd.remote_dma_broadcast(
                                rb[:, 2 * k:2 * k + 2], part[:],
                                remote_sem=rsem, local_sem=lsem,
                                rdests=[(0, k) if i == k else None
                                        for i in range(8)])
                        nc.gpsimd.trigger_dma(count=None)
                        nc.vector.wait_ge(rsem, 14 * (ar_idx + 1))
                        nc.vector.tensor_tensor(
                            out=rb[:, 0:8], in0=rb[:, 0:8], in1=rb[:, 8:16],
                            op=ALU.add)
                        nc.vector.tensor_tensor(
                            out=rb[:, 0:4], in0=rb[:, 0:4], in1=rb[:, 4:8],
                            op=ALU.add)
                        nc.vector.tensor_tensor(
                            out=rb[:, 0:2], in0=rb[:, 0:2], in1=rb[:, 2:4],
                            op=ALU.add)
                        pst = pslin.tile([128, 512], f32, tag="lin")
                        nc.tensor.matmul(pst[:1, :F], rb[:, 0:1],
                                         selfd_sb[li], start=True, stop=True)
                        nc.tensor.matmul(pst[:1, F:2 * F], rb[:, 1:2],
                                         selfd_sb[li], start=True, stop=True)
                        stats_g = miscp.tile([1, 2 * F], f32, tag="statg")
                        nc.vector.tensor_copy(stats_g[:], pst[:1, :2 * F])
                    else:
                        pst = pslin.tile([128, 512], f32, tag="lin")
                        nc.tensor.matmul(pst[:1, :F], aggr[:, 0:1],
                                         selfd_sb[li], start=True, stop=True)
                        nc.tensor.matmul(pst[:1, F:2 * F], part[:, 1:2],
                                         selfd_sb[li], start=True, stop=True)
                        stats_l = miscp.tile([1, 2 * F], f32, tag="statl")
                        nc.vector.tensor_copy(stats_l[:], pst[:1, :2 * F])
                        bin_ = dramp.tile([1, 2 * F], f32, tag=f"arin{ar_idx}")
                        bout = dramp.tile([8, 2 * F], f32, tag=f"arout{ar_idx}")
                        nc.gpsimd.dma_start(bin_[:], stats_l[:])
                        nc.gpsimd.collective_compute(
                            "AllGather", ALU.bypass,
                            replica_groups=[list(range(NCORES))],
                            ins=[bin_.opt()], outs=[bout.opt()])
                        sg8 = miscp.tile([8, 2 * F], f32, tag="sg8")
                        nc.gpsimd.dma_start(sg8[:], bout[:])
                        psg = pslin.tile([128, 512], f32, tag="lin", name="psg")
                        nc.tensor.matmul(psg[:1, :2 * F], onesn[:, li:li + 1],
                                         sg8[:], start=True, stop=True)
                        stats_g = miscp.tile([1, 2 * F], f32, tag="statg")
                        nc.vector.tensor_copy(stats_g[:], psg[:1, :2 * F])
                    # stats_g = (mu, E[y^2]) ; st cols [0:F]=s, [F:2F]=t
                    st = miscp.tile([1, 2 * F], f32, tag="st")
                    tmp = miscp.tile([1, 2 * F], f32, tag="sttmp")
                    mu2 = miscp.tile([1, F], f32, tag="mu2")
                    nc.vector.tensor_tensor(out=mu2[:], in0=stats_g[:, 0:F],
                                            in1=stats_g[:, 0:F], op=ALU.mult)
                    nc.vector.tensor_tensor(out=tmp[:, F:2 * F],
                                            in0=stats_g[:, F:2 * F],
                                            in1=mu2[:], op=ALU.subtract)
                    nc.scalar.activation(tmp[:, F:2 * F], tmp[:, F:2 * F],
                                         AF.Sqrt, bias=eps_t[:])
                    nc.vector.reciprocal(tmp[:, F:2 * F], tmp[:, F:2 * F])
                    nc.vector.tensor_tensor(out=st[:, 0:F],
                                            in0=tmp[:, F:2 * F],
                                            in1=gb_sb[li][:, 0:F], op=ALU.mult)
                    nc.vector.tensor_tensor(out=mu2[:], in0=stats_g[:, 0:F],
                                            in1=st[:, 0:F], op=ALU.mult)
                    nc.vector.tensor_tensor(out=st[:, F:2 * F],
                                            in0=gb_sb[li][:, F:2 * F],
                                            in1=mu2[:], op=ALU.subtract)
                    pss = pslin.tile([128, 512], f32, tag="lin", name="pss")
                    nc.tensor.transpose(pss[:2 * F, 0:1], st[:],
                                        ident_f[:1, :1])
                    stv = miscp.tile([128, 1], f32, tag="stv")
                    nc.vector.tensor_copy(stv[:2 * F, :], pss[:2 * F, 0:1])
                    # broadcast (s,t) to all 128 partitions via selT matmuls
                    psc = pslin.tile([128, 512], f32, tag="lin", name="psc")
                    nc.tensor.matmul(psc[:, 0:1], sT_sb[F][:2 * F, 0:128],
                                     stv[:2 * F, :], start=True, stop=True)
                    nc.tensor.matmul(psc[:, 1:2], sT_sb[F][:2 * F, 128:256],
                                     stv[:2 * F, :], start=True, stop=True)
                    stc = miscp.tile([128, 2], f32, tag=f"stc{ar_idx}")
                    nc.vector.tensor_copy(stc[:], psc[:, 0:2])
                    ar_idx += 1
                    # chunked scale+relu (+ up4 replication for next layer)
                    ncfg = CFGS[li + 1]
                    if ncfg.up4:
                        XFrep_cur = poolA.tile(
                            [128, ncfg.nG * ncfg.V], bf16, tag="A")
                        s_r = XFn[:].rearrange("p (g w) -> p g w", w=V)
                        d_r = XFrep_cur[:].rearrange(
                            "p (g w r) -> p g w r", w=V, r=4)
                    nap = max(1, nGp // 4)
                    for q0 in range(0, nGp, nap):
                        q1 = min(q0 + nap, nGp)
                        c0_, c1_ = q0 * V, q1 * V
                        cm = c0_ + (c1_ - c0_) * 5 // 9  # Act a bit slower
                        nc.scalar.activation(
                            XFn[:, c0_:cm], XFn[:, c0_:cm],
                            AF.Relu, scale=stc[:, 0:1], bias=stc[:, 1:2])
                        nc.vector.tensor_scalar(
                            out=XFn[:, cm:c1_], in0=XFn[:, cm:c1_],
                            scalar1=stc[:, 0:1], scalar2=stc[:, 1:2],
                            op0=ALU.mult, op1=ALU.add)
                        nc.vector.tensor_scalar_max(
                            XFn[:, cm:c1_], XFn[:, cm:c1_], 0.0)
                        if ncfg.up4:
                            for r in range(4):
                                nc.vector.tensor_copy(
                                    d_r[:, q0:q1, :, r], s_r[:, q0:q1, :])
                    XF_cur = XFn
                else:
                    # --- stage output: ytile [v, fo*32+b] -> [b, v*3+fo] ---
                    for t in range(cfg.nVt):
                        pt = pstr.tile([128, 512], bf16, tag="tr")
                        nc.tensor.transpose(
                            pt[:96, :128],
                            ytile[:128, t * BF:(t + 1) * BF],
                            ident_b[:128, :128])
                        och = outp.tile([BL, 384], bf16, tag="out")
                        for fo in range(3):
                            nc.scalar.activation(
                                och[:].rearrange("b (v f) -> b v f", f=3)
                                [:, :, fo],
                                pt[fo * 32:(fo + 1) * 32, :128],
                                AF.Copy)
                        nc.sync.dma_start(
                            ydram[:, t * 384:(t + 1) * 384], och[:])

    nc.compile()
    return nc


def kernel(**inputs):
    import sys
    for p in ("/opt/trn_rl_repo", "/opt/trn_rl_repo/concourse"):
        if p not in sys.path:
            sys.path.insert(0, p)
    from concourse.bass_utils import run_bass_kernel_spmd

    host = _build_host(inputs)

    if "nc" not in _CACHE:
        _CACHE["nc"] = _build_nc()
    nc = _CACHE["nc"]

    in_maps = []
    for c in range(NCORES):
        m = {k: v for k, v in host.items() if k != "xT"}
        m["xT"] = np.ascontiguousarray(host["xT"][:, c * BL:(c + 1) * BL])
        in_maps.append(m)
    res = run_bass_kernel_spmd(nc, in_maps, core_ids=list(range(NCORES)))
    out = np.concatenate(
        [np.asarray(r["y"], np.float32).reshape(BL, 1280, 3)
         for r in res.results], axis=0)
    return out.astype(np.float32)


if __name__ == "__main__":
    import reference as R
    inp = R.setup_inputs()
    inp = {k: np.asarray(v) for k, v in inp.items()}
    act = kernel(**inp)
    exp = np.asarray(R.reference(**inp))
    err = np.linalg.norm(act - exp) / np.linalg.norm(exp)
    print("Relative error:", err)


# revision 48
# speedup vs baseline: 1.5466x; 1.0138x over previous
"""Trainium2 Bass kernel for nn_Graph_CNN_Feat_Mesh (Chebyshev GNN decoder).

Strategy (per-core, data-parallel over batch B=256 -> 32/core):
  - All spmms are dense matmuls on the tensor engine (PE) in bf16:
      y = A + L @ (B + L @ (2C)),  A/B/C = feature-space linears of the input.
    L is densified on host; for up4-preceded layers the replication is folded
    into LU = L @ U (contracting the small pre-upsample vertex space).
  - B and A linear terms accumulate directly into the spmm PSUM.
  - Activations live in packed F-layout [(j,Fin) partitions, (b//G)*Vsp + v]
    between layers; the per-layer linear emits V-layout directly; one PE
    transpose per layer returns to F-layout.
  - BatchNorm (training mode, global batch stats) is exact: per-core partial
    sums are AllGather'd across the 8 cores in-kernel (cheaper than
    AllReduce) and summed locally with a K=8 ones-matmul; stats are taken
    per transpose-group so they finish with the last transpose; scale+relu
    is chunked so the next layer starts on early chunks.
  - Weights are host-pre-tiled into [128, *] monoliths and streamed with a
    handful of large DMAs on the gpsimd queue (25ns issue) in consumption
    order; the FC head runs in bf16 with fp32 PSUM.
"""

import numpy as np

B = 256
NCORES = 8
BL = B // NCORES  # 32
EPS = 1e-5
USE_RDMA = False  # remote-DMA BN exchange: unsupported by the timing sim

_CACHE = {}


def _split_W(W):
    W = np.asarray(W, np.float32)
    return W[:, 0::3], W[:, 1::3], W[:, 2::3]


def _dense_L(rows, cols, vals, V):
    L = np.zeros((V, V), np.float32)
    np.add.at(L, (np.asarray(rows), np.asarray(cols)), np.asarray(vals, np.float32))
    return L


def _tile128(a):
    """[S*128, N] -> [128, S*N] block-column layout (pad rows to mult of 128)."""
    a = np.asarray(a)
    S = (a.shape[0] + 127) // 128
    if a.shape[0] != S * 128:
        a = np.concatenate(
            [a, np.zeros((S * 128 - a.shape[0], a.shape[1]), a.dtype)], 0)
    return np.ascontiguousarray(
        a.reshape(S, 128, a.shape[1]).transpose(1, 0, 2).reshape(128, -1))


class _LCfg:
    def __init__(self, name, Vsp, V, Fin, Fout, up4, bn):
        self.name = name
        self.Vsp = Vsp      # source vertex space of C-linear (pre-up4)
        self.V = V          # output vertex count
        self.Fin = Fin
        self.Fout = Fout
        self.G = 128 // Fin          # batches packed on partitions at input
        self.nG = BL // self.G
        self.GF = self.G * Fout      # N of one B/C/A-linear matmul
        self.Gp = 128 // Fout if Fout in (32, 64) else None
        self.nGp = BL // self.Gp if self.Gp else None
        self.up4 = up4
        self.bn = bn
        self.nVt = (V + 127) // 128
        self.nVsp = (Vsp + 127) // 128
        self.BF = BL * Fout          # free width of V-layout per vtile

    def vts(self, t):
        return min(128, self.V - t * 128)

    def sps(self, s):
        return min(128, self.Vsp - s * 128)


CFGS = [
    _LCfg("c0", 80, 320, 64, 64, True, True),
    _LCfg("c1", 320, 320, 64, 32, False, True),
    _LCfg("c2", 320, 1280, 32, 32, True, True),
    _LCfg("c3", 1280, 1280, 32, 3, False, False),
]


def _wbd(W, G, Fin, Fout, which):
    """Block-diagonal rhs weight [128, G*Fout] for the fused linear.
    which: 'A' -> W0 - W2, 'B' -> W1, 'C' -> 2*W2.  col = j*Fout + c."""
    W0, W1, W2 = _split_W(W)
    M = {"A": W0 - W2, "B": W1, "C": 2.0 * W2}[which]  # [Fout, Fin]
    out = np.zeros((128, G * Fout), np.float32)
    for j in range(G):
        out[j * Fin:(j + 1) * Fin, j * Fout:(j + 1) * Fout] = M.T
    return out


# column offsets inside the packed weight blobs
_WOFF = {}
_off = 0
for _cfg in CFGS:
    for _w in "ABC":
        _WOFF[f"{_w}{_cfg.name}"] = (_off, _cfg.GF)
        _off += _cfg.GF
WPACK_N = _off  # bf16 pack cols

# f32 pack: fc1b | per-layer FD-scaled sel blocks (BN partial-sum reduce)
F32_FC1B = 0
F32_SEL = [4, 68, 100]   # selFD for bn layers 0,1,2 (widths 64,32,32)
F32PACK_N = 132
_BN_F = [64, 32, 32]
_BN_FD = [16 * 320, 8 * 320, 8 * 1280]
_BN_NG = [256 * 320, 256 * 320, 256 * 1280]
# with equal per-core/per-group counts, global mu = sum of partition means
# scaled by FD/NG; same factor turns summed (mean^2+var) into E[y^2]
_BN_SCL = [fd / ng for fd, ng in zip(_BN_FD, _BN_NG)]


def _build_host(inputs):
    import ml_dtypes
    bf = ml_dtypes.bfloat16
    f32 = np.float32
    d = {}
    d["xT"] = np.ascontiguousarray(np.asarray(inputs["x"], f32).T).astype(bf)
    d["fc1wt"] = _tile128(
        np.asarray(inputs["fc1_w"], f32).T).astype(bf)           # [128, 16*512]
    d["fc1b"] = np.ascontiguousarray(
        np.asarray(inputs["fc1_b"], f32).reshape(4, 128).T)      # [128,4]
    # fc2: chunk-major (mc), then k-tile: [128, 16*1280]
    w2 = np.asarray(inputs["fc2_w"], f32).T                      # [512, 5120]
    blk = [w2[kt * 128:(kt + 1) * 128, mc * 1280:(mc + 1) * 1280]
           for mc in range(4) for kt in range(4)]
    d["fc2wt"] = np.ascontiguousarray(np.concatenate(blk, 1)).astype(bf)

    L1 = _dense_L(inputs["L1_rows"], inputs["L1_cols"], inputs["L1_vals"], 320)
    L2 = _dense_L(inputs["L2_rows"], inputs["L2_cols"], inputs["L2_vals"], 1280)
    U1 = np.repeat(np.eye(80, dtype=f32), 4, axis=0)    # [320, 80]
    U2 = np.repeat(np.eye(320, dtype=f32), 4, axis=0)   # [1280, 320]
    f8 = ml_dtypes.float8_e4m3
    d["LU0"] = _tile128((L1 @ U1).T).astype(bf)         # [128, 320]
    d["LT1"] = _tile128(L1.T).astype(bf)                # [128, 3*320]
    d["LU2"] = _tile128((L2 @ U2).T).astype(f8)         # [128, 3*1280] fp8
    d["LT2"] = _tile128(L2.T).astype(f8)                # [128, 10*1280] fp8

    Wn = {"c0": "cl0_w", "c1": "cl1_w", "c2": "cl2_w", "c3": "cl3_w"}
    wall = np.zeros((128, WPACK_N), f32)
    for cfg in CFGS:
        W = np.asarray(inputs[Wn[cfg.name]], f32)
        for which in "ABC":
            o, n = _WOFF[f"{which}{cfg.name}"]
            wall[:, o:o + n] = _wbd(W, cfg.G, cfg.Fin, cfg.Fout, which)
    d["wall"] = wall.astype(bf)
    # b3 tiled over the (b, c) column layout of the last-layer PSUM: col = b*3+c
    d["b3row"] = np.ascontiguousarray(
        np.tile(np.asarray(inputs["cl3_b"], f32), BL)[None, :]).astype(bf)

    gbp = np.zeros((1, 256), f32)
    for i, (g, b, o, F) in enumerate([("bn0_g", "bn0_b", 0, 64),
                                      ("bn1_g", "bn1_b", 128, 32),
                                      ("bn2_g", "bn2_b", 192, 32)]):
        gbp[0, o:o + F] = np.asarray(inputs[g], f32)
        gbp[0, o + F:o + 2 * F] = np.asarray(inputs[b], f32)
    d["gbpack"] = gbp

    f32p = np.zeros((128, F32PACK_N), f32)
    for li in range(3):
        F, o = _BN_F[li], F32_SEL[li]
        v = _BN_SCL[li] if USE_RDMA else _BN_FD[li]
        for j in range(128 // F):
            f32p[j * F:(j + 1) * F, o:o + F] += v * np.eye(F, dtype=f32)
    f32p[:, F32_FC1B:F32_FC1B + 4] = d.pop("fc1b")
    d["f32pack"] = f32p
    # selT_s/selT_t [2F rows, 128]: stc[p,:] = (s[p%F], t[p%F]) via 2 matmuls
    stp = np.zeros((128, 2 * 128), f32)
    for F, ro in [(64, 0), (32, 0)]:
        pass
    sT = np.zeros((128, 256), f32)   # rows k (2F<=128), cols: [0:128]=s-map, [128:256]=t-map
    # build per-F maps stacked by row-offset: F=64 uses rows 0:128, F=32 uses rows 0:64
    sT64 = np.zeros((128, 256), f32)
    for p in range(128):
        sT64[p % 64, p] = 1.0          # k = c        -> s
        sT64[64 + p % 64, 128 + p] = 1.0  # k = F + c  -> t
    sT32 = np.zeros((64, 256), f32)
    for p in range(128):
        sT32[p % 32, p] = 1.0
        sT32[32 + p % 32, 128 + p] = 1.0
    d["selT64"] = sT64
    d["selT32"] = np.concatenate([sT32, np.zeros((64, 256), f32)], 0)
    return d


def _build_nc():
    import sys
    for p in ("/opt/trn_rl_repo", "/opt/trn_rl_repo/concourse"):
        if p not in sys.path:
            sys.path.insert(0, p)
    import concourse.bass as bass  # noqa
    import concourse.mybir as mybir
    import concourse.tile as tile
    from concourse import bacc
    from concourse.masks import make_identity

    f32 = mybir.dt.float32
    bf16 = mybir.dt.bfloat16
    fp8 = mybir.dt.float8e4
    DR = mybir.MatmulPerfMode.DoubleRow
    AF = mybir.ActivationFunctionType
    ALU = mybir.AluOpType

    nc = bacc.Bacc(None, target_bir_lowering=False)

    xT = nc.dram_tensor("xT", [2048, BL], bf16, kind="ExternalInput")
    fc1wt = nc.dram_tensor("fc1wt", [128, 16 * 512], bf16, kind="ExternalInput")
    fc2wt = nc.dram_tensor("fc2wt", [128, 16 * 1280], bf16, kind="ExternalInput")
    LU0 = nc.dram_tensor("LU0", [128, 320], bf16, kind="ExternalInput")
    LT1 = nc.dram_tensor("LT1", [128, 3 * 320], bf16, kind="ExternalInput")
    LU2 = nc.dram_tensor("LU2", [128, 3 * 1280], fp8, kind="ExternalInput")
    LT2 = nc.dram_tensor("LT2", [128, 10 * 1280], fp8, kind="ExternalInput")
    wall = nc.dram_tensor("wall", [128, WPACK_N], bf16, kind="ExternalInput")
    gbpack = nc.dram_tensor("gbpack", [1, 256], f32, kind="ExternalInput")
    f32pack = nc.dram_tensor("f32pack", [128, F32PACK_N], f32, kind="ExternalInput")
    selT64 = nc.dram_tensor("selT64", [128, 256], f32, kind="ExternalInput")
    selT32 = nc.dram_tensor("selT32", [128, 256], f32, kind="ExternalInput")
    b3row = nc.dram_tensor("b3row", [1, 96], bf16, kind="ExternalInput")
    ydram = nc.dram_tensor("y", [BL, 1280 * 3], bf16, kind="ExternalOutput")

    with tile.TileContext(nc) as tc:
        with (
            tc.tile_pool(name="const", bufs=1) as constp,
            tc.tile_pool(name="wpool", bufs=1) as wpool,
            tc.tile_pool(name="poolA", bufs=2) as poolA,
            tc.tile_pool(name="poolB", bufs=2) as poolB,
            tc.tile_pool(name="poolC", bufs=1) as poolC,
            tc.tile_pool(name="misc", bufs=1) as miscp,
            tc.tile_pool(name="outp", bufs=3) as outp,
            tc.tile_pool(name="pslin", bufs=2, space="PSUM") as pslin,
            tc.tile_pool(name="psbig", bufs=2, space="PSUM") as psbig,
            tc.tile_pool(name="pstr", bufs=2, space="PSUM") as pstr,
            tc.tile_pool(name="dram", bufs=1, space="DRAM") as dramp,
        ):
            # ---- fc1 inputs first: these DMAs gate the first matmul ----
            fc1w_sb = poolA.tile([128, 16 * 512], bf16, tag="A")
            nc.gpsimd.dma_start(fc1w_sb[:, :4 * 512], fc1wt[:, :4 * 512])
            xT_sb = miscp.tile([128, 16 * BL], bf16, tag="xT")
            nc.gpsimd.dma_start(
                xT_sb[:].rearrange("p (k b) -> p k b", b=BL),
                xT[:].rearrange("(k p) b -> p k b", p=128))
            for kc in range(1, 4):
                nc.gpsimd.dma_start(
                    fc1w_sb[:, kc * 4 * 512:(kc + 1) * 4 * 512],
                    fc1wt[:, kc * 4 * 512:(kc + 1) * 4 * 512])
            f32_sb = constp.tile([128, F32PACK_N], f32, tag="f32pack")
            nc.sync.dma_start(f32_sb[:], f32pack[:])
            selfd_sb = [f32_sb[:, F32_SEL[li]:F32_SEL[li] + _BN_F[li]]
                        for li in range(3)]
            fc1b_sb = f32_sb[:, F32_FC1B:F32_FC1B + 4]

            # ---- small constants (no DMA) ----
            ident_b = constp.tile([128, 128], bf16, tag="identb")
            make_identity(nc, ident_b[:])
            ident_f = constp.tile([1, 1], f32, tag="identf")
            nc.gpsimd.memset(ident_f[:], 1.0)
            eps_t = constp.tile([1, 1], f32, tag="eps")
            nc.gpsimd.memset(eps_t[:], EPS)
            onesn = constp.tile([8, 3], f32, tag="onesn")
            for li in range(3):
                nc.gpsimd.memset(onesn[:, li:li + 1], 1.0 / _BN_NG[li])
            onesv = constp.tile([1, 128], bf16, tag="onesv")
            nc.gpsimd.memset(onesv[:], 1.0)
            sq_warm = constp.tile([1, 1], f32, tag="sqwarm")
            nc.scalar.activation(sq_warm[:], eps_t[:], AF.Sqrt, bias=eps_t[:])

            # ================= FC head (bf16, fp32 psum) =================
            # kt-outer so matmuls start as soon as the first fc1w chunk lands
            h1T = miscp.tile([128, 4 * BL], bf16, tag="h1T")
            ps1 = pslin.tile([128, 4 * BL], f32, tag="lin")
            for mt in range(4):
                for kt in range(16):
                    nc.tensor.matmul(
                        ps1[:, mt * BL:(mt + 1) * BL],
                        fc1w_sb[:, kt * 512 + mt * 128: kt * 512 + (mt + 1) * 128],
                        xT_sb[:, kt * BL:(kt + 1) * BL],
                        start=(kt == 0), stop=(kt == 15))
                nc.scalar.activation(
                    h1T[:, mt * BL:(mt + 1) * BL], ps1[:, mt * BL:(mt + 1) * BL],
                    AF.Relu, bias=fc1b_sb[:, mt:mt + 1])

            # ---- mid-priority loads (small; needed for c0/c1 + BN) ----
            gb_all = constp.tile([1, 256], f32, tag="gbp")
            nc.sync.dma_start(gb_all[:], gbpack[:])
            gb_sb = [gb_all[:, 0:128], gb_all[:, 128:192], gb_all[:, 192:256]]
            sT_sb = {64: constp.tile([128, 256], f32, tag="sT64", name="sT64sb"),
                     32: constp.tile([128, 256], f32, tag="sT32", name="sT32sb")}
            nc.sync.dma_start(sT_sb[64][:], selT64[:])
            nc.sync.dma_start(sT_sb[32][:], selT32[:])
            b3_sb = constp.tile([1, 96], bf16, tag="b3row")
            nc.sync.dma_start(b3_sb[:], b3row[:])

            LUT, LTd = {}, {}
            t = wpool.tile([128, 320], bf16, tag="LU0")
            nc.sync.dma_start(t[:], LU0[:])
            LUT["c0"] = t
            t = wpool.tile([128, 3 * 320], bf16, tag="LT1")
            nc.sync.dma_start(t[:], LT1[:])
            LTd["c0"] = LTd["c1"] = LUT["c1"] = t
            wall_sb = wpool.tile([128, WPACK_N], bf16, tag="wall")
            nc.sync.dma_start(wall_sb[:], wall[:])
            W_sb = {}
            for cfg in CFGS:
                for w in "ABC":
                    o, n = _WOFF[f"{w}{cfg.name}"]
                    W_sb[f"{w}{cfg.name}"] = wall_sb[:, o:o + n]

            # ================= fc2 (streamed in 4 column-chunks) =========
            # psum partition = (v0%2)*64+f, col = mi*BL+b ; channels c = v0*64+f.
            # dest: XF0[(b%2)*64+f, (b//2)*80 + v0],  v0 = 2*(mc*10+mi)+p0
            XF0 = poolC.tile([128, 16 * 80], bf16, tag="XF0")
            cfg0 = CFGS[0]
            XFrep0 = poolA.tile([128, cfg0.nG * cfg0.V], bf16, tag="A")
            s_r0 = XF0[:].rearrange("p (g w) -> p g w", w=80)
            d_r0 = XFrep0[:].rearrange("p (g w r) -> p g w r", w=80, r=4)
            for mc in range(4):
                wch = poolB.tile([128, 4 * 1280], bf16, tag="B")
                nc.gpsimd.dma_start(
                    wch[:], fc2wt[:, mc * 4 * 1280:(mc + 1) * 4 * 1280])
                ps2 = psbig.tile([128, 10 * BL], f32, tag="big")
                for mi in range(10):
                    for kt in range(4):
                        nc.tensor.matmul(
                            ps2[:, mi * BL:(mi + 1) * BL],
                            wch[:, kt * 1280 + mi * 128: kt * 1280 + (mi + 1) * 128],
                            h1T[:, kt * BL:(kt + 1) * BL],
                            start=(kt == 0), stop=(kt == 3))
                src4 = ps2[:].rearrange("p (i g j) -> p i g j", g=16, j=2)
                dst4 = XF0[:].rearrange("p (g u q) -> p g u q", u=40, q=2)
                for p0 in range(2):
                    for j in range(2):
                        nc.scalar.activation(
                            dst4[j * 64:(j + 1) * 64, :,
                                 mc * 10:(mc + 1) * 10, p0]
                            .rearrange("p g i -> p i g"),
                            src4[p0 * 64:(p0 + 1) * 64, :, :, j],
                            AF.Copy)
                # up4-replicate this chunk's w-range (w = v0 in [20mc, 20mc+20))
                for r in range(4):
                    nc.vector.tensor_copy(
                        d_r0[:, :, 20 * mc:20 * (mc + 1), r],
                        s_r0[:, :, 20 * mc:20 * (mc + 1)])

            # ---- big late loads (needed at c2; stream during c0/c1) ----
            t = wpool.tile([128, 3 * 1280], fp8, tag="LU2")
            nc.gpsimd.dma_start(t[:], LU2[:])
            LUT["c2"] = t
            t = wpool.tile([128, 10 * 1280], fp8, tag="LT2")
            nc.gpsimd.dma_start(t[:, :5 * 1280], LT2[:, :5 * 1280])
            nc.gpsimd.dma_start(t[:, 5 * 1280:], LT2[:, 5 * 1280:])
            LTd["c2"] = LTd["c3"] = LUT["c3"] = t

            # ================= cheby layers =================
            if USE_RDMA:
                rsem = nc.alloc_semaphore("bn_rsem")
                lsem = nc.alloc_semaphore("bn_lsem")
                rbufs = [constp.tile([128, 16], f32, tag=f"rbuf{i}",
                                     name=f"rbuf{i}")
                         for i in range(3)]
            XF_cur = XF0
            XFrep_cur = XFrep0
            ar_idx = 0

            for li, cfg in enumerate(CFGS):
                V, Vsp, F = cfg.V, cfg.Vsp, cfg.Fout
                BF = cfg.BF
                last = cfg.name == "c3"
                XFrep = XFrep_cur if cfg.up4 else XF_cur

                fp8sp = cfg.name in ("c2", "c3")
                sp_dt = fp8 if fp8sp else bf16
                # --- C linear (in Vsp space) ---
                XC = poolC.tile([128, cfg.nVsp * BL * F], sp_dt, tag="XC")
                gpack = max(1, 512 // cfg.GF)
                for s in range(cfg.nVsp):
                    ssz = cfg.sps(s)
                    for g0 in range(0, cfg.nG, gpack):
                        gn = min(gpack, cfg.nG - g0)
                        pc = pslin.tile([128, 512], f32, tag="lin")
                        for gi in range(gn):
                            g = g0 + gi
                            nc.tensor.matmul(
                                pc[:ssz, gi * cfg.GF:(gi + 1) * cfg.GF],
                                XF_cur[:, g * Vsp + s * 128:
                                       g * Vsp + s * 128 + ssz],
                                W_sb[f"C{cfg.name}"][:],
                                start=True, stop=True)
                        nc.scalar.activation(
                            XC[:ssz, s * BL * F + g0 * cfg.GF:
                               s * BL * F + (g0 + gn) * cfg.GF],
                            pc[:ssz, :gn * cfg.GF], AF.Copy)

                # --- inner = LU @ (2C) + B ;  y = L @ inner + A ---
                Xin = poolB.tile([128, cfg.nVt * BF], sp_dt, tag="B")
                ytile = poolC.tile([128, cfg.nVt * BF], bf16, tag="YT")
                for phase in range(2):
                    srcL = LUT[cfg.name] if phase == 0 else LTd[cfg.name]
                    nS = cfg.nVsp if phase == 0 else cfg.nVt
                    ssizes = ([cfg.sps(s) for s in range(nS)] if phase == 0
                              else [cfg.vts(s) for s in range(nS)])
                    rhs = XC if phase == 0 else Xin
                    rhs_w = BL * F if phase == 0 else BF
                    Wacc = W_sb[f"B{cfg.name}" if phase == 0 else f"A{cfg.name}"]
                    dst = Xin if phase == 0 else ytile
                    for t in range(cfg.nVt):
                        vsz = cfg.vts(t)
                        for pc0 in range(0, BF, 1024):
                            pw = min(1024, BF - pc0)
                            pi = psbig.tile([128, max(pw, 512)], f32, tag="big")
                            for nk in range(0, pw, 512):
                                n0 = pc0 + nk
                                n1 = min(n0 + 512, pc0 + pw)
                                if fp8sp:
                                    # fp8 DoubleRow: contract 2 s-tiles/pass
                                    srcr = srcL[:].rearrange(
                                        "p (s v) -> p s v", v=V)
                                    rhsr = rhs[:].rearrange(
                                        "p (s n) -> p s n", n=rhs_w)
                                    for s0 in range(0, nS, 2):
                                        if s0 + 1 < nS and \
                                                ssizes[s0 + 1] == 128:
                                            nc.tensor.matmul(
                                                pi[:vsz, n0 - pc0:n1 - pc0],
                                                srcr[:, s0:s0 + 2,
                                                     t * 128:t * 128 + vsz],
                                                rhsr[:, s0:s0 + 2, n0:n1],
                                                start=(s0 == 0), stop=False,
                                                skip_group_check=True,
                                                perf_mode=DR)
                                        else:
                                            for s in range(s0, min(s0 + 2,
                                                                   nS)):
                                                ssz = ssizes[s]
                                                nc.tensor.matmul(
                                                    pi[:vsz,
                                                       n0 - pc0:n1 - pc0],
                                                    srcL[:ssz, s * V + t * 128:
                                                         s * V + t * 128 + vsz],
                                                    rhs[:ssz, s * rhs_w + n0:
                                                        s * rhs_w + n1],
                                                    start=(s == 0), stop=False,
                                                    skip_group_check=True)
                                else:
                                    for s in range(nS):
                                        ssz = ssizes[s]
                                        nc.tensor.matmul(
                                            pi[:vsz, n0 - pc0:n1 - pc0],
                                            srcL[:ssz, s * V + t * 128:
                                                 s * V + t * 128 + vsz],
                                            rhs[:ssz, s * rhs_w + n0:
                                                s * rhs_w + n1],
                                            start=(s == 0), stop=False,
                                            skip_group_check=True)
                                for g in range(n0 // cfg.GF,
                                               (n1 + cfg.GF - 1) // cfg.GF):
                                    nc.tensor.matmul(
                                        pi[:vsz, g * cfg.GF - pc0:
                                           (g + 1) * cfg.GF - pc0],
                                        XFrep[:, g * V + t * 128:
                                              g * V + t * 128 + vsz],
                                        Wacc[:],
                                        start=False,
                                        stop=(not (last and phase == 1)),
                                        skip_group_check=True)
                                if last and phase == 1:
                                    # fold cl3 bias: += ones^T @ b3row
                                    nc.tensor.matmul(
                                        pi[:vsz, n0 - pc0:n1 - pc0],
                                        onesv[:1, :vsz],
                                        b3_sb[:1, n0:n1],
                                        start=False, stop=True,
                                        skip_group_check=True)
                            if last and phase == 1:
                                # reorder (b,fo) -> (fo,b) for output staging
                                nc.vector.tensor_copy(
                                    dst[:vsz, t * BF + pc0: t * BF + pc0 + pw]
                                    .rearrange("p (c b) -> p c b", b=BL),
                                    pi[:vsz, :pw]
                                    .rearrange("p (b c) -> p c b", c=3))
                            elif phase == 0:
                                nc.scalar.activation(
                                    dst[:vsz, t * BF + pc0: t * BF + pc0 + pw],
                                    pi[:vsz, :pw], AF.Copy)
                            else:
                                nc.vector.tensor_copy(
                                    dst[:vsz, t * BF + pc0: t * BF + pc0 + pw],
                                    pi[:vsz, :pw])

                if not last:
                    # --- back-transpose to packed F-layout; stats per group ---
                    Gp, nGp = cfg.Gp, cfg.nGp
                    nq = (nGp + 3) // 4
                    XFn = poolA.tile([128, nGp * V], bf16, tag="A")
                    dstv = XFn[:].rearrange("p (g v) -> p g v", v=V)
                    nch = cfg.nVt * nq + nGp  # worst case incl. partial tiles
                    bnst = miscp.tile([128, nch * 6], f32, tag="bnst")
                    chn = 0
                    for t in range(cfg.nVt):
                        vsz = cfg.vts(t)
                        for qi0 in range(nq):
                            q0 = qi0 * 4
                            qn = min(4, nGp - q0)
                            pt = pstr.tile([128, 512], bf16, tag="tr")
                            for qi in range(qn):
                                gp = q0 + qi
                                nc.tensor.transpose(
                                    pt[:, qi * 128: qi * 128 + vsz],
                                    ytile[:vsz, t * BF + gp * 128:
                                          t * BF + (gp + 1) * 128],
                                    ident_b[:vsz, :vsz])
                            reg = dstv[:, q0:q0 + qn, t * 128:t * 128 + vsz]
                            nc.scalar.activation(
                                reg,
                                pt[:].rearrange("p (q v) -> p q v", v=128)
                                [:, :qn, :vsz],
                                AF.Copy)
                            if vsz == 128:
                                # stats straight off the transpose PSUM tile
                                nc.vector.bn_stats(
                                    bnst[:, chn * 6:(chn + 1) * 6],
                                    pt[:, :qn * 128])
                                chn += 1
                            else:
                                for qi in range(qn):
                                    gp = q0 + qi
                                    nc.vector.bn_stats(
                                        bnst[:, chn * 6:(chn + 1) * 6],
                                        XFn[:, gp * V + t * 128:
                                            gp * V + t * 128 + vsz])
                                    chn += 1
                    aggr = miscp.tile([128, 2], f32, tag="aggr")
                    nc.vector.bn_aggr(
                        aggr[:], bnst[:, :chn * 6]
                        .rearrange("p (c s) -> p c s", s=6))
                    part = miscp.tile([128, 2], f32, tag="part")
                    if USE_RDMA and ar_idx > 0:
                        # prior layer's sends must have drained before reuse
                        nc.vector.wait_ge(lsem, 112 * ar_idx)
                    nc.vector.tensor_tensor(
                        out=part[:, 1:2], in0=aggr[:, 0:1], in1=aggr[:, 0:1],
                        op=ALU.mult)
                    nc.vector.tensor_tensor(
                        out=part[:, 1:2], in0=part[:, 1:2], in1=aggr[:, 1:2],
                        op=ALU.add)
                    if USE_RDMA:
                        nc.vector.tensor_copy(part[:, 0:1], aggr[:, 0:1])
                        rb = rbufs[ar_idx]
                        nc.vector.tensor_copy(rb[:, 0:2], part[:])
                        for k in range(1, 8):
                            nc.gpsimd.remote_dma_broadcast(
                                rb[:, 2 * k:2 * k + 2], part[:],
                                remote_sem=rsem, local_sem=lsem,
                                rdests=[(0, k) if i == k else None
                                        for i in range(8)])
                        nc.gpsimd.trigger_dma(count=None)
                        nc.vector.wait_ge(rsem, 14 * (ar_idx + 1))
                        nc.vector.tensor_tensor(
                            out=rb[:, 0:8], in0=rb[:, 0:8], in1=rb[:, 8:16],
                            op=ALU.add)
                        nc.vector.tensor_tensor(
                            out=rb[:, 0:4], in0=rb[:, 0:4], in1=rb[:, 4:8],
                            op=ALU.add)
                        nc.vector.tensor_tensor(
                            out=rb[:, 0:2], in0=rb[:, 0:2], in1=rb[:, 2:4],
                            op=ALU.add)
                        pst = pslin.tile([128, 512], f32, tag="lin")
                        nc.tensor.matmul(pst[:1, :F], rb[:, 0:1],
                                         selfd_sb[li], start=True, stop=True)
                        nc.tensor.matmul(pst[:1, F:2 * F], rb[:, 1:2],
                                         selfd_sb[li], start=True, stop=True)
                        stats_g = miscp.tile([1, 2 * F], f32, tag="statg")
                        nc.vector.tensor_copy(stats_g[:], pst[:1, :2 * F])
                    else:
                        pst = pslin.tile([128, 512], f32, tag="lin")
                        nc.tensor.matmul(pst[:1, :F], aggr[:, 0:1],
                                         selfd_sb[li], start=True, stop=True)
                        nc.tensor.matmul(pst[:1, F:2 * F], part[:, 1:2],
                                         selfd_sb[li], start=True, stop=True)
                        stats_l = miscp.tile([1, 2 * F], f32, tag="statl")
                        nc.vector.tensor_copy(stats_l[:], pst[:1, :2 * F])
                        bin_ = dramp.tile([1, 2 * F], f32, tag=f"arin{ar_idx}")
                        bout = dramp.tile([8, 2 * F], f32, tag=f"arout{ar_idx}")
                        nc.sync.dma_start(bin_[:], stats_l[:])
                        nc.gpsimd.collective_compute(
                            "AllGather", ALU.bypass,
                            replica_groups=[list(range(NCORES))],
                            ins=[bin_.opt()], outs=[bout.opt()])
                        sg8 = miscp.tile([8, 2 * F], f32, tag="sg8")
                        nc.sync.dma_start(sg8[:], bout[:])
                        psg = pslin.tile([128, 512], f32, tag="lin", name="psg")
                        nc.tensor.matmul(psg[:1, :2 * F], onesn[:, li:li + 1],
                                         sg8[:], start=True, stop=True)
                        stats_g = miscp.tile([1, 2 * F], f32, tag="statg")
                        nc.vector.tensor_copy(stats_g[:], psg[:1, :2 * F])
                    # stats_g = (mu, E[y^2]) ; st cols [0:F]=s, [F:2F]=t
                    st = miscp.tile([1, 2 * F], f32, tag="st")
                    tmp = miscp.tile([1, 2 * F], f32, tag="sttmp")
                    mu2 = miscp.tile([1, F], f32, tag="mu2")
                    nc.vector.tensor_tensor(out=mu2[:], in0=stats_g[:, 0:F],
                                            in1=stats_g[:, 0:F], op=ALU.mult)
                    nc.vector.tensor_tensor(out=tmp[:, F:2 * F],
                                            in0=stats_g[:, F:2 * F],
                                            in1=mu2[:], op=ALU.subtract)
                    nc.scalar.activation(tmp[:, F:2 * F], tmp[:, F:2 * F],
                                         AF.Sqrt, bias=eps_t[:])
                    nc.vector.reciprocal(tmp[:, F:2 * F], tmp[:, F:2 * F])
                    nc.vector.tensor_tensor(out=st[:, 0:F],
                                            in0=tmp[:, F:2 * F],
                                            in1=gb_sb[li][:, 0:F], op=ALU.mult)
                    nc.vector.tensor_tensor(out=mu2[:], in0=stats_g[:, 0:F],
                                            in1=st[:, 0:F], op=ALU.mult)
                    nc.vector.tensor_tensor(out=st[:, F:2 * F],
                                            in0=gb_sb[li][:, F:2 * F],
                                            in1=mu2[:], op=ALU.subtract)
                    pss = pslin.tile([128, 512], f32, tag="lin", name="pss")
                    nc.tensor.transpose(pss[:2 * F, 0:1], st[:],
                                        ident_f[:1, :1])
                    stv = miscp.tile([128, 1], f32, tag="stv")
                    nc.vector.tensor_copy(stv[:2 * F, :], pss[:2 * F, 0:1])
                    # broadcast (s,t) to all 128 partitions via selT matmuls
                    psc = pslin.tile([128, 512], f32, tag="lin", name="psc")
                    nc.tensor.matmul(psc[:, 0:1], sT_sb[F][:2 * F, 0:128],
                                     stv[:2 * F, :], start=True, stop=True)
                    nc.tensor.matmul(psc[:, 1:2], sT_sb[F][:2 * F, 128:256],
                                     stv[:2 * F, :], start=True, stop=True)
                    stc = miscp.tile([128, 2], f32, tag=f"stc{ar_idx}")
                    nc.vector.tensor_copy(stc[:], psc[:, 0:2])
                    ar_idx += 1
                    # chunked scale+relu (+ up4 replication for next layer)
                    ncfg = CFGS[li + 1]
                    if ncfg.up4:
                        XFrep_cur = poolA.tile(
                            [128, ncfg.nG * ncfg.V], bf16, tag="A")
                        s_r = XFn[:].rearrange("p (g w) -> p g w", w=V)
                        d_r = XFrep_cur[:].rearrange(
                            "p (g w r) -> p g w r", w=V, r=4)
                    nap = max(1, nGp // 4)
                    for q0 in range(0, nGp, nap):
                        q1 = min(q0 + nap, nGp)
                        c0_, c1_ = q0 * V, q1 * V
                        cm = c0_ + (c1_ - c0_) * 5 // 9  # Act a bit slower
                        nc.scalar.activation(
                            XFn[:, c0_:cm], XFn[:, c0_:cm],
                            AF.Relu, scale=stc[:, 0:1], bias=stc[:, 1:2])
                        nc.vector.tensor_scalar(
                            out=XFn[:, cm:c1_], in0=XFn[:, cm:c1_],
                            scalar1=stc[:, 0:1], scalar2=stc[:, 1:2],
                            op0=ALU.mult, op1=ALU.add)
                        nc.vector.tensor_scalar_max(
                            XFn[:, cm:c1_], XFn[:, cm:c1_], 0.0)
                        if ncfg.up4:
                            for r in range(4):
                                nc.vector.tensor_copy(
                                    d_r[:, q0:q1, :, r], s_r[:, q0:q1, :])
                    XF_cur = XFn
                else:
                    # --- stage output: ytile [v, fo*32+b] -> [b, v*3+fo] ---
                    for t in range(cfg.nVt):
                        pt = pstr.tile([128, 512], bf16, tag="tr")
                        nc.tensor.transpose(
                            pt[:96, :128],
                            ytile[:128, t * BF:(t + 1) * BF],
                            ident_b[:128, :128])
                        och = outp.tile([BL, 384], bf16, tag="out")
                        for fo in range(3):
                            nc.scalar.activation(
                                och[:].rearrange("b (v f) -> b v f", f=3)
                                [:, :, fo],
                                pt[fo * 32:(fo + 1) * 32, :128],
                                AF.Copy)
                        nc.sync.dma_start(
                            ydram[:, t * 384:(t + 1) * 384], och[:])

    nc.compile()
    return nc


def kernel(**inputs):
    import sys
    for p in ("/opt/trn_rl_repo", "/opt/trn_rl_repo/concourse"):
        if p not in sys.path:
            sys.path.insert(0, p)
    from concourse.bass_utils import run_bass_kernel_spmd

    host = _build_host(inputs)

    if "nc" not in _CACHE:
        _CACHE["nc"] = _build_nc()
    nc = _CACHE["nc"]

    in_maps = []
    for c in range(NCORES):
        m = {k: v for k, v in host.items() if k != "xT"}
        m["xT"] = np.ascontiguousarray(host["xT"][:, c * BL:(c + 1) * BL])
        in_maps.append(m)
    res = run_bass_kernel_spmd(nc, in_maps, core_ids=list(range(NCORES)))
    out = np.concatenate(
        [np.asarray(r["y"], np.float32).reshape(BL, 1280, 3)
         for r in res.results], axis=0)
    return out.astype(np.float32)


if __name__ == "__main__":
    import reference as R
    inp = R.setup_inputs()
    inp = {k: np.asarray(v) for k, v in inp.items()}
    act = kernel(**inp)
    exp = np.asarray(R.reference(**inp))
    err = np.linalg.norm(act - exp) / np.linalg.norm(exp)
    print("Relative error:", err)


# revision 49
# speedup vs baseline: 1.5477x; 1.0007x over previous
"""Trainium2 Bass kernel for nn_Graph_CNN_Feat_Mesh (Chebyshev GNN decoder).

Strategy (per-core, data-parallel over batch B=256 -> 32/core):
  - All spmms are dense matmuls on the tensor engine (PE) in bf16:
      y = A + L @ (B + L @ (2C)),  A/B/C = feature-space linears of the input.
    L is densified on host; for up4-preceded layers the replication is folded
    into LU = L @ U (contracting the small pre-upsample vertex space).
  - B and A linear terms accumulate directly into the spmm PSUM.
  - Activations live in packed F-layout [(j,Fin) partitions, (b//G)*Vsp + v]
    between layers; the per-layer linear emits V-layout directly; one PE
    transpose per layer returns to F-layout.
  - BatchNorm (training mode, global batch stats) is exact: per-core partial
    sums are AllGather'd across the 8 cores in-kernel (cheaper than
    AllReduce) and summed locally with a K=8 ones-matmul; stats are taken
    per transpose-group so they finish with the last transpose; scale+relu
    is chunked so the next layer starts on early chunks.
  - Weights are host-pre-tiled into [128, *] monoliths and streamed with a
    handful of large DMAs on the gpsimd queue (25ns issue) in consumption
    order; the FC head runs in bf16 with fp32 PSUM.
"""

import numpy as np

B = 256
NCORES = 8
BL = B // NCORES  # 32
EPS = 1e-5
USE_RDMA = False  # remote-DMA BN exchange: unsupported by the timing sim

_CACHE = {}


def _split_W(W):
    W = np.asarray(W, np.float32)
    return W[:, 0::3], W[:, 1::3], W[:, 2::3]


def _dense_L(rows, cols, vals, V):
    L = np.zeros((V, V), np.float32)
    np.add.at(L, (np.asarray(rows), np.asarray(cols)), np.asarray(vals, np.float32))
    return L


def _tile128(a):
    """[S*128, N] -> [128, S*N] block-column layout (pad rows to mult of 128)."""
    a = np.asarray(a)
    S = (a.shape[0] + 127) // 128
    if a.shape[0] != S * 128:
        a = np.concatenate(
            [a, np.zeros((S * 128 - a.shape[0], a.shape[1]), a.dtype)], 0)
    return np.ascontiguousarray(
        a.reshape(S, 128, a.shape[1]).transpose(1, 0, 2).reshape(128, -1))


class _LCfg:
    def __init__(self, name, Vsp, V, Fin, Fout, up4, bn):
        self.name = name
        self.Vsp = Vsp      # source vertex space of C-linear (pre-up4)
        self.V = V          # output vertex count
        self.Fin = Fin
        self.Fout = Fout
        self.G = 128 // Fin          # batches packed on partitions at input
        self.nG = BL // self.G
        self.GF = self.G * Fout      # N of one B/C/A-linear matmul
        self.Gp = 128 // Fout if Fout in (32, 64) else None
        self.nGp = BL // self.Gp if self.Gp else None
        self.up4 = up4
        self.bn = bn
        self.nVt = (V + 127) // 128
        self.nVsp = (Vsp + 127) // 128
        self.BF = BL * Fout          # free width of V-layout per vtile

    def vts(self, t):
        return min(128, self.V - t * 128)

    def sps(self, s):
        return min(128, self.Vsp - s * 128)


CFGS = [
    _LCfg("c0", 80, 320, 64, 64, True, True),
    _LCfg("c1", 320, 320, 64, 32, False, True),
    _LCfg("c2", 320, 1280, 32, 32, True, True),
    _LCfg("c3", 1280, 1280, 32, 3, False, False),
]


def _wbd(W, G, Fin, Fout, which):
    """Block-diagonal rhs weight [128, G*Fout] for the fused linear.
    which: 'A' -> W0 - W2, 'B' -> W1, 'C' -> 2*W2.  col = j*Fout + c."""
    W0, W1, W2 = _split_W(W)
    M = {"A": W0 - W2, "B": W1, "C": 2.0 * W2}[which]  # [Fout, Fin]
    out = np.zeros((128, G * Fout), np.float32)
    for j in range(G):
        out[j * Fin:(j + 1) * Fin, j * Fout:(j + 1) * Fout] = M.T
    return out


# column offsets inside the packed weight blobs
_WOFF = {}
_off = 0
for _cfg in CFGS:
    for _w in "ABC":
        _WOFF[f"{_w}{_cfg.name}"] = (_off, _cfg.GF)
        _off += _cfg.GF
WPACK_N = _off  # bf16 pack cols

# f32 pack: fc1b | per-layer FD-scaled sel blocks (BN partial-sum reduce)
F32_FC1B = 0
F32_SEL = [4, 68, 100]   # selFD for bn layers 0,1,2 (widths 64,32,32)
F32PACK_N = 132
_BN_F = [64, 32, 32]
_BN_FD = [16 * 320, 8 * 320, 8 * 1280]
_BN_NG = [256 * 320, 256 * 320, 256 * 1280]
# with equal per-core/per-group counts, global mu = sum of partition means
# scaled by FD/NG; same factor turns summed (mean^2+var) into E[y^2]
_BN_SCL = [fd / ng for fd, ng in zip(_BN_FD, _BN_NG)]


def _build_host(inputs):
    import ml_dtypes
    bf = ml_dtypes.bfloat16
    f32 = np.float32
    d = {}
    d["xT"] = np.ascontiguousarray(np.asarray(inputs["x"], f32).T).astype(bf)
    d["fc1wt"] = _tile128(
        np.asarray(inputs["fc1_w"], f32).T).astype(bf)           # [128, 16*512]
    d["fc1b"] = np.ascontiguousarray(
        np.asarray(inputs["fc1_b"], f32).reshape(4, 128).T)      # [128,4]
    # fc2: chunk-major (mc), then k-tile: [128, 16*1280]
    w2 = np.asarray(inputs["fc2_w"], f32).T                      # [512, 5120]
    blk = [w2[kt * 128:(kt + 1) * 128, mc * 1280:(mc + 1) * 1280]
           for mc in range(4) for kt in range(4)]
    d["fc2wt"] = np.ascontiguousarray(np.concatenate(blk, 1)).astype(bf)

    L1 = _dense_L(inputs["L1_rows"], inputs["L1_cols"], inputs["L1_vals"], 320)
    L2 = _dense_L(inputs["L2_rows"], inputs["L2_cols"], inputs["L2_vals"], 1280)
    U1 = np.repeat(np.eye(80, dtype=f32), 4, axis=0)    # [320, 80]
    U2 = np.repeat(np.eye(320, dtype=f32), 4, axis=0)   # [1280, 320]
    f8 = ml_dtypes.float8_e4m3
    d["LU0"] = _tile128((L1 @ U1).T).astype(bf)         # [128, 320]
    d["LT1"] = _tile128(L1.T).astype(bf)                # [128, 3*320]
    d["LU2"] = _tile128((L2 @ U2).T).astype(f8)         # [128, 3*1280] fp8
    d["LT2"] = _tile128(L2.T).astype(f8)                # [128, 10*1280] fp8

    Wn = {"c0": "cl0_w", "c1": "cl1_w", "c2": "cl2_w", "c3": "cl3_w"}
    wall = np.zeros((128, WPACK_N), f32)
    for cfg in CFGS:
        W = np.asarray(inputs[Wn[cfg.name]], f32)
        for which in "ABC":
            o, n = _WOFF[f"{which}{cfg.name}"]
            wall[:, o:o + n] = _wbd(W, cfg.G, cfg.Fin, cfg.Fout, which)
    d["wall"] = wall.astype(bf)
    # b3 tiled over the (b, c) column layout of the last-layer PSUM: col = b*3+c
    d["b3row"] = np.ascontiguousarray(
        np.tile(np.asarray(inputs["cl3_b"], f32), BL)[None, :]).astype(bf)

    gbp = np.zeros((1, 256), f32)
    for i, (g, b, o, F) in enumerate([("bn0_g", "bn0_b", 0, 64),
                                      ("bn1_g", "bn1_b", 128, 32),
                                      ("bn2_g", "bn2_b", 192, 32)]):
        gbp[0, o:o + F] = np.asarray(inputs[g], f32)
        gbp[0, o + F:o + 2 * F] = np.asarray(inputs[b], f32)
    d["gbpack"] = gbp

    f32p = np.zeros((128, F32PACK_N), f32)
    for li in range(3):
        F, o = _BN_F[li], F32_SEL[li]
        v = _BN_SCL[li] if USE_RDMA else _BN_FD[li]
        for j in range(128 // F):
            f32p[j * F:(j + 1) * F, o:o + F] += v * np.eye(F, dtype=f32)
    f32p[:, F32_FC1B:F32_FC1B + 4] = d.pop("fc1b")
    d["f32pack"] = f32p
    # selT_s/selT_t [2F rows, 128]: stc[p,:] = (s[p%F], t[p%F]) via 2 matmuls
    stp = np.zeros((128, 2 * 128), f32)
    for F, ro in [(64, 0), (32, 0)]:
        pass
    sT = np.zeros((128, 256), f32)   # rows k (2F<=128), cols: [0:128]=s-map, [128:256]=t-map
    # build per-F maps stacked by row-offset: F=64 uses rows 0:128, F=32 uses rows 0:64
    sT64 = np.zeros((128, 256), f32)
    for p in range(128):
        sT64[p % 64, p] = 1.0          # k = c        -> s
        sT64[64 + p % 64, 128 + p] = 1.0  # k = F + c  -> t
    sT32 = np.zeros((64, 256), f32)
    for p in range(128):
        sT32[p % 32, p] = 1.0
        sT32[32 + p % 32, 128 + p] = 1.0
    d["selT64"] = sT64
    d["selT32"] = np.concatenate([sT32, np.zeros((64, 256), f32)], 0)
    return d


def _build_nc():
    import sys
    for p in ("/opt/trn_rl_repo", "/opt/trn_rl_repo/concourse"):
        if p not in sys.path:
            sys.path.insert(0, p)
    import concourse.bass as bass  # noqa
    import concourse.mybir as mybir
    import concourse.tile as tile
    from concourse import bacc
    from concourse.masks import make_identity

    f32 = mybir.dt.float32
    bf16 = mybir.dt.bfloat16
    fp8 = mybir.dt.float8e4
    DR = mybir.MatmulPerfMode.DoubleRow
    AF = mybir.ActivationFunctionType
    ALU = mybir.AluOpType

    nc = bacc.Bacc(None, target_bir_lowering=False)

    xT = nc.dram_tensor("xT", [2048, BL], bf16, kind="ExternalInput")
    fc1wt = nc.dram_tensor("fc1wt", [128, 16 * 512], bf16, kind="ExternalInput")
    fc2wt = nc.dram_tensor("fc2wt", [128, 16 * 1280], bf16, kind="ExternalInput")
    LU0 = nc.dram_tensor("LU0", [128, 320], bf16, kind="ExternalInput")
    LT1 = nc.dram_tensor("LT1", [128, 3 * 320], bf16, kind="ExternalInput")
    LU2 = nc.dram_tensor("LU2", [128, 3 * 1280], fp8, kind="ExternalInput")
    LT2 = nc.dram_tensor("LT2", [128, 10 * 1280], fp8, kind="ExternalInput")
    wall = nc.dram_tensor("wall", [128, WPACK_N], bf16, kind="ExternalInput")
    gbpack = nc.dram_tensor("gbpack", [1, 256], f32, kind="ExternalInput")
    f32pack = nc.dram_tensor("f32pack", [128, F32PACK_N], f32, kind="ExternalInput")
    selT64 = nc.dram_tensor("selT64", [128, 256], f32, kind="ExternalInput")
    selT32 = nc.dram_tensor("selT32", [128, 256], f32, kind="ExternalInput")
    b3row = nc.dram_tensor("b3row", [1, 96], bf16, kind="ExternalInput")
    ydram = nc.dram_tensor("y", [BL, 1280 * 3], bf16, kind="ExternalOutput")

    with tile.TileContext(nc) as tc:
        with (
            tc.tile_pool(name="const", bufs=1) as constp,
            tc.tile_pool(name="wpool", bufs=1) as wpool,
            tc.tile_pool(name="poolA", bufs=2) as poolA,
            tc.tile_pool(name="poolB", bufs=2) as poolB,
            tc.tile_pool(name="poolC", bufs=1) as poolC,
            tc.tile_pool(name="misc", bufs=1) as miscp,
            tc.tile_pool(name="outp", bufs=3) as outp,
            tc.tile_pool(name="pslin", bufs=2, space="PSUM") as pslin,
            tc.tile_pool(name="psbig", bufs=2, space="PSUM") as psbig,
            tc.tile_pool(name="pstr", bufs=2, space="PSUM") as pstr,
            tc.tile_pool(name="dram", bufs=1, space="DRAM") as dramp,
        ):
            # ---- fc1 inputs first: these DMAs gate the first matmul ----
            fc1w_sb = poolA.tile([128, 16 * 512], bf16, tag="A")
            nc.gpsimd.dma_start(fc1w_sb[:, :4 * 512], fc1wt[:, :4 * 512])
            xT_sb = miscp.tile([128, 16 * BL], bf16, tag="xT")
            nc.gpsimd.dma_start(
                xT_sb[:].rearrange("p (k b) -> p k b", b=BL),
                xT[:].rearrange("(k p) b -> p k b", p=128))
            for kc in range(1, 4):
                nc.gpsimd.dma_start(
                    fc1w_sb[:, kc * 4 * 512:(kc + 1) * 4 * 512],
                    fc1wt[:, kc * 4 * 512:(kc + 1) * 4 * 512])
            f32_sb = constp.tile([128, F32PACK_N], f32, tag="f32pack")
            nc.sync.dma_start(f32_sb[:], f32pack[:])
            selfd_sb = [f32_sb[:, F32_SEL[li]:F32_SEL[li] + _BN_F[li]]
                        for li in range(3)]
            fc1b_sb = f32_sb[:, F32_FC1B:F32_FC1B + 4]

            # ---- small constants (no DMA) ----
            ident_b = constp.tile([128, 128], bf16, tag="identb")
            make_identity(nc, ident_b[:])
            ident_f = constp.tile([1, 1], f32, tag="identf")
            nc.gpsimd.memset(ident_f[:], 1.0)
            eps_t = constp.tile([1, 1], f32, tag="eps")
            nc.gpsimd.memset(eps_t[:], EPS)
            onesn = constp.tile([8, 3], f32, tag="onesn")
            for li in range(3):
                nc.gpsimd.memset(onesn[:, li:li + 1], 1.0 / _BN_NG[li])
            onesv = constp.tile([1, 128], bf16, tag="onesv")
            nc.gpsimd.memset(onesv[:], 1.0)
            sq_warm = constp.tile([1, 1], f32, tag="sqwarm")
            nc.scalar.activation(sq_warm[:], eps_t[:], AF.Sqrt, bias=eps_t[:])

            # ================= FC head (bf16, fp32 psum) =================
            # kt-outer so matmuls start as soon as the first fc1w chunk lands
            h1T = miscp.tile([128, 4 * BL], bf16, tag="h1T")
            ps1 = pslin.tile([128, 4 * BL], f32, tag="lin")
            for mt in range(4):
                for kt in range(16):
                    nc.tensor.matmul(
                        ps1[:, mt * BL:(mt + 1) * BL],
                        fc1w_sb[:, kt * 512 + mt * 128: kt * 512 + (mt + 1) * 128],
                        xT_sb[:, kt * BL:(kt + 1) * BL],
                        start=(kt == 0), stop=(kt == 15))
                nc.scalar.activation(
                    h1T[:, mt * BL:(mt + 1) * BL], ps1[:, mt * BL:(mt + 1) * BL],
                    AF.Relu, bias=fc1b_sb[:, mt:mt + 1])

            # ---- mid-priority loads (small; needed for c0/c1 + BN) ----
            gb_all = constp.tile([1, 256], f32, tag="gbp")
            nc.sync.dma_start(gb_all[:], gbpack[:])
            gb_sb = [gb_all[:, 0:128], gb_all[:, 128:192], gb_all[:, 192:256]]
            sT_sb = {64: constp.tile([128, 256], f32, tag="sT64", name="sT64sb"),
                     32: constp.tile([128, 256], f32, tag="sT32", name="sT32sb")}
            nc.sync.dma_start(sT_sb[64][:], selT64[:])
            nc.sync.dma_start(sT_sb[32][:], selT32[:])
            b3_sb = constp.tile([1, 96], bf16, tag="b3row")
            nc.sync.dma_start(b3_sb[:], b3row[:])

            LUT, LTd = {}, {}
            t = wpool.tile([128, 320], bf16, tag="LU0")
            nc.sync.dma_start(t[:], LU0[:])
            LUT["c0"] = t
            t = wpool.tile([128, 3 * 320], bf16, tag="LT1")
            nc.sync.dma_start(t[:], LT1[:])
            LTd["c0"] = LTd["c1"] = LUT["c1"] = t
            wall_sb = wpool.tile([128, WPACK_N], bf16, tag="wall")
            nc.sync.dma_start(wall_sb[:], wall[:])
            W_sb = {}
            for cfg in CFGS:
                for w in "ABC":
                    o, n = _WOFF[f"{w}{cfg.name}"]
                    W_sb[f"{w}{cfg.name}"] = wall_sb[:, o:o + n]

            # ================= fc2 (streamed in 4 column-chunks) =========
            # psum partition = (v0%2)*64+f, col = mi*BL+b ; channels c = v0*64+f.
            # dest: XF0[(b%2)*64+f, (b//2)*80 + v0],  v0 = 2*(mc*10+mi)+p0
            XF0 = poolC.tile([128, 16 * 80], bf16, tag="XF0")
            cfg0 = CFGS[0]
            XFrep0 = poolA.tile([128, cfg0.nG * cfg0.V], bf16, tag="A")
            s_r0 = XF0[:].rearrange("p (g w) -> p g w", w=80)
            d_r0 = XFrep0[:].rearrange("p (g w r) -> p g w r", w=80, r=4)
            for mc in range(4):
                wch = poolB.tile([128, 4 * 1280], bf16, tag="B")
                nc.gpsimd.dma_start(
                    wch[:], fc2wt[:, mc * 4 * 1280:(mc + 1) * 4 * 1280])
                ps2 = psbig.tile([128, 10 * BL], f32, tag="big")
                for mi in range(10):
                    for kt in range(4):
                        nc.tensor.matmul(
                            ps2[:, mi * BL:(mi + 1) * BL],
                            wch[:, kt * 1280 + mi * 128: kt * 1280 + (mi + 1) * 128],
                            h1T[:, kt * BL:(kt + 1) * BL],
                            start=(kt == 0), stop=(kt == 3))
                src4 = ps2[:].rearrange("p (i g j) -> p i g j", g=16, j=2)
                dst4 = XF0[:].rearrange("p (g u q) -> p g u q", u=40, q=2)
                for p0 in range(2):
                    for j in range(2):
                        nc.scalar.activation(
                            dst4[j * 64:(j + 1) * 64, :,
                                 mc * 10:(mc + 1) * 10, p0]
                            .rearrange("p g i -> p i g"),
                            src4[p0 * 64:(p0 + 1) * 64, :, :, j],
                            AF.Copy)
                # up4-replicate this chunk's w-range (w = v0 in [20mc, 20mc+20))
                for r in range(4):
                    nc.vector.tensor_copy(
                        d_r0[:, :, 20 * mc:20 * (mc + 1), r],
                        s_r0[:, :, 20 * mc:20 * (mc + 1)])

            # ---- big late loads (needed at c2; stream during c0/c1) ----
            t = wpool.tile([128, 3 * 1280], fp8, tag="LU2")
            nc.gpsimd.dma_start(t[:], LU2[:])
            LUT["c2"] = t
            t = wpool.tile([128, 10 * 1280], fp8, tag="LT2")
            nc.gpsimd.dma_start(t[:, :5 * 1280], LT2[:, :5 * 1280])
            nc.gpsimd.dma_start(t[:, 5 * 1280:], LT2[:, 5 * 1280:])
            LTd["c2"] = LTd["c3"] = LUT["c3"] = t

            # ================= cheby layers =================
            if USE_RDMA:
                rsem = nc.alloc_semaphore("bn_rsem")
                lsem = nc.alloc_semaphore("bn_lsem")
                rbufs = [constp.tile([128, 16], f32, tag=f"rbuf{i}",
                                     name=f"rbuf{i}")
                         for i in range(3)]
            XF_cur = XF0
            XFrep_cur = XFrep0
            ar_idx = 0

            for li, cfg in enumerate(CFGS):
                V, Vsp, F = cfg.V, cfg.Vsp, cfg.Fout
                BF = cfg.BF
                last = cfg.name == "c3"
                XFrep = XFrep_cur if cfg.up4 else XF_cur

                fp8sp = cfg.name in ("c2", "c3")
                sp_dt = fp8 if fp8sp else bf16
                # --- C linear (in Vsp space) ---
                XC = poolC.tile([128, cfg.nVsp * BL * F], sp_dt, tag="XC")
                gpack = max(1, 512 // cfg.GF)
                for s in range(cfg.nVsp):
                    ssz = cfg.sps(s)
                    for g0 in range(0, cfg.nG, gpack):
                        gn = min(gpack, cfg.nG - g0)
                        pc = pslin.tile([128, 512], f32, tag="lin")
                        for gi in range(gn):
                            g = g0 + gi
                            nc.tensor.matmul(
                                pc[:ssz, gi * cfg.GF:(gi + 1) * cfg.GF],
                                XF_cur[:, g * Vsp + s * 128:
                                       g * Vsp + s * 128 + ssz],
                                W_sb[f"C{cfg.name}"][:],
                                start=True, stop=True)
                        nc.scalar.activation(
                            XC[:ssz, s * BL * F + g0 * cfg.GF:
                               s * BL * F + (g0 + gn) * cfg.GF],
                            pc[:ssz, :gn * cfg.GF], AF.Copy)

                # --- inner = LU @ (2C) + B ;  y = L @ inner + A ---
                Xin = poolB.tile([128, cfg.nVt * BF], sp_dt, tag="B")
                ytile = poolC.tile([128, cfg.nVt * BF], bf16, tag="YT")
                for phase in range(2):
                    srcL = LUT[cfg.name] if phase == 0 else LTd[cfg.name]
                    nS = cfg.nVsp if phase == 0 else cfg.nVt
                    ssizes = ([cfg.sps(s) for s in range(nS)] if phase == 0
                              else [cfg.vts(s) for s in range(nS)])
                    rhs = XC if phase == 0 else Xin
                    rhs_w = BL * F if phase == 0 else BF
                    Wacc = W_sb[f"B{cfg.name}" if phase == 0 else f"A{cfg.name}"]
                    dst = Xin if phase == 0 else ytile
                    for t in range(cfg.nVt):
                        vsz = cfg.vts(t)
                        for pc0 in range(0, BF, 1024):
                            pw = min(1024, BF - pc0)
                            pi = psbig.tile([128, max(pw, 512)], f32, tag="big")
                            for nk in range(0, pw, 512):
                                n0 = pc0 + nk
                                n1 = min(n0 + 512, pc0 + pw)
                                if fp8sp:
                                    # fp8 DoubleRow: contract 2 s-tiles/pass
                                    srcr = srcL[:].rearrange(
                                        "p (s v) -> p s v", v=V)
                                    rhsr = rhs[:].rearrange(
                                        "p (s n) -> p s n", n=rhs_w)
                                    for s0 in range(0, nS, 2):
                                        if s0 + 1 < nS and \
                                                ssizes[s0 + 1] == 128:
                                            nc.tensor.matmul(
                                                pi[:vsz, n0 - pc0:n1 - pc0],
                                                srcr[:, s0:s0 + 2,
                                                     t * 128:t * 128 + vsz],
                                                rhsr[:, s0:s0 + 2, n0:n1],
                                                start=(s0 == 0), stop=False,
                                                skip_group_check=True,
                                                perf_mode=DR)
                                        else:
                                            for s in range(s0, min(s0 + 2,
                                                                   nS)):
                                                ssz = ssizes[s]
                                                nc.tensor.matmul(
                                                    pi[:vsz,
                                                       n0 - pc0:n1 - pc0],
                                                    srcL[:ssz, s * V + t * 128:
                                                         s * V + t * 128 + vsz],
                                                    rhs[:ssz, s * rhs_w + n0:
                                                        s * rhs_w + n1],
                                                    start=(s == 0), stop=False,
                                                    skip_group_check=True)
                                else:
                                    for s in range(nS):
                                        ssz = ssizes[s]
                                        nc.tensor.matmul(
                                            pi[:vsz, n0 - pc0:n1 - pc0],
                                            srcL[:ssz, s * V + t * 128:
                                                 s * V + t * 128 + vsz],
                                            rhs[:ssz, s * rhs_w + n0:
                                                s * rhs_w + n1],
                                            start=(s == 0), stop=False,
                                            skip_group_check=True)
                                for g in range(n0 // cfg.GF,
                                               (n1 + cfg.GF - 1) // cfg.GF):
                                    nc.tensor.matmul(
                                        pi[:vsz, g * cfg.GF - pc0:
                                           (g + 1) * cfg.GF - pc0],
                                        XFrep[:, g * V + t * 128:
                                              g * V + t * 128 + vsz],
                                        Wacc[:],
                                        start=False,
                                        stop=(not (last and phase == 1)),
                                        skip_group_check=True)
                                if last and phase == 1:
                                    # fold cl3 bias: += ones^T @ b3row
                                    nc.tensor.matmul(
                                        pi[:vsz, n0 - pc0:n1 - pc0],
                                        onesv[:1, :vsz],
                                        b3_sb[:1, n0:n1],
                                        start=False, stop=True,
                                        skip_group_check=True)
                            if last and phase == 1:
                                # reorder (b,fo) -> (fo,b) for output staging
                                nc.vector.tensor_copy(
                                    dst[:vsz, t * BF + pc0: t * BF + pc0 + pw]
                                    .rearrange("p (c b) -> p c b", b=BL),
                                    pi[:vsz, :pw]
                                    .rearrange("p (b c) -> p c b", c=3))
                            elif phase == 0:
                                nc.scalar.activation(
                                    dst[:vsz, t * BF + pc0: t * BF + pc0 + pw],
                                    pi[:vsz, :pw], AF.Copy)
                            else:
                                nc.vector.tensor_copy(
                                    dst[:vsz, t * BF + pc0: t * BF + pc0 + pw],
                                    pi[:vsz, :pw])

                if not last:
                    # --- back-transpose to packed F-layout; stats per group ---
                    Gp, nGp = cfg.Gp, cfg.nGp
                    nq = (nGp + 3) // 4
                    XFn = poolA.tile([128, nGp * V], bf16, tag="A")
                    dstv = XFn[:].rearrange("p (g v) -> p g v", v=V)
                    nch = cfg.nVt * nq + nGp  # worst case incl. partial tiles
                    bnst = miscp.tile([128, nch * 6], f32, tag="bnst")
                    chn = 0
                    for t in range(cfg.nVt):
                        vsz = cfg.vts(t)
                        for qi0 in range(nq):
                            q0 = qi0 * 4
                            qn = min(4, nGp - q0)
                            pt = pstr.tile([128, 512], bf16, tag="tr")
                            for qi in range(qn):
                                gp = q0 + qi
                                nc.tensor.transpose(
                                    pt[:, qi * 128: qi * 128 + vsz],
                                    ytile[:vsz, t * BF + gp * 128:
                                          t * BF + (gp + 1) * 128],
                                    ident_b[:vsz, :vsz])
                            reg = dstv[:, q0:q0 + qn, t * 128:t * 128 + vsz]
                            nc.scalar.activation(
                                reg,
                                pt[:].rearrange("p (q v) -> p q v", v=128)
                                [:, :qn, :vsz],
                                AF.Copy)
                            if vsz == 128:
                                # stats straight off the transpose PSUM tile
                                nc.vector.bn_stats(
                                    bnst[:, chn * 6:(chn + 1) * 6],
                                    pt[:, :qn * 128])
                                chn += 1
                            else:
                                for qi in range(qn):
                                    gp = q0 + qi
                                    nc.vector.bn_stats(
                                        bnst[:, chn * 6:(chn + 1) * 6],
                                        XFn[:, gp * V + t * 128:
                                            gp * V + t * 128 + vsz])
                                    chn += 1
                    aggr = miscp.tile([128, 2], f32, tag="aggr")
                    nc.vector.bn_aggr(
                        aggr[:], bnst[:, :chn * 6]
                        .rearrange("p (c s) -> p c s", s=6))
                    part = miscp.tile([128, 2], f32, tag="part")
                    if USE_RDMA and ar_idx > 0:
                        # prior layer's sends must have drained before reuse
                        nc.vector.wait_ge(lsem, 112 * ar_idx)
                    nc.vector.tensor_tensor(
                        out=part[:, 1:2], in0=aggr[:, 0:1], in1=aggr[:, 0:1],
                        op=ALU.mult)
                    nc.vector.tensor_tensor(
                        out=part[:, 1:2], in0=part[:, 1:2], in1=aggr[:, 1:2],
                        op=ALU.add)
                    if USE_RDMA:
                        nc.vector.tensor_copy(part[:, 0:1], aggr[:, 0:1])
                        rb = rbufs[ar_idx]
                        nc.vector.tensor_copy(rb[:, 0:2], part[:])
                        for k in range(1, 8):
                            nc.gpsimd.remote_dma_broadcast(
                                rb[:, 2 * k:2 * k + 2], part[:],
                                remote_sem=rsem, local_sem=lsem,
                                rdests=[(0, k) if i == k else None
                                        for i in range(8)])
                        nc.gpsimd.trigger_dma(count=None)
                        nc.vector.wait_ge(rsem, 14 * (ar_idx + 1))
                        nc.vector.tensor_tensor(
                            out=rb[:, 0:8], in0=rb[:, 0:8], in1=rb[:, 8:16],
                            op=ALU.add)
                        nc.vector.tensor_tensor(
                            out=rb[:, 0:4], in0=rb[:, 0:4], in1=rb[:, 4:8],
                            op=ALU.add)
                        nc.vector.tensor_tensor(
                            out=rb[:, 0:2], in0=rb[:, 0:2], in1=rb[:, 2:4],
                            op=ALU.add)
                        pst = pslin.tile([128, 512], f32, tag="lin")
                        nc.tensor.matmul(pst[:1, :F], rb[:, 0:1],
                                         selfd_sb[li], start=True, stop=True)
                        nc.tensor.matmul(pst[:1, F:2 * F], rb[:, 1:2],
                                         selfd_sb[li], start=True, stop=True)
                        stats_g = miscp.tile([1, 2 * F], f32, tag="statg")
                        nc.vector.tensor_copy(stats_g[:], pst[:1, :2 * F])
                    else:
                        pst = pslin.tile([128, 512], f32, tag="lin")
                        nc.tensor.matmul(pst[:1, :F], aggr[:, 0:1],
                                         selfd_sb[li], start=True, stop=True)
                        nc.tensor.matmul(pst[:1, F:2 * F], part[:, 1:2],
                                         selfd_sb[li], start=True, stop=True)
                        stats_l = miscp.tile([1, 2 * F], f32, tag="statl")
                        nc.vector.tensor_copy(stats_l[:], pst[:1, :2 * F])
                        bin_ = dramp.tile([1, 2 * F], f32, tag=f"arin{ar_idx}")
                        bout = dramp.tile([8, 2 * F], f32, tag=f"arout{ar_idx}")
                        nc.sync.dma_start(bin_[:], stats_l[:])
                        nc.gpsimd.collective_compute(
                            "AllGather", ALU.bypass,
                            replica_groups=[list(range(NCORES))],
                            ins=[bin_.opt()], outs=[bout.opt()])
                        sg8 = miscp.tile([8, 2 * F], f32, tag="sg8")
                        nc.sync.dma_start(sg8[:], bout[:])
                        psg = pslin.tile([128, 512], f32, tag="lin", name="psg")
                        nc.tensor.matmul(psg[:1, :2 * F], onesn[:, li:li + 1],
                                         sg8[:], start=True, stop=True)
                        stats_g = miscp.tile([1, 2 * F], f32, tag="statg")
                        nc.vector.tensor_copy(stats_g[:], psg[:1, :2 * F])
                    # stats_g = (mu, E[y^2]) ; st cols [0:F]=s, [F:2F]=t
                    st = miscp.tile([1, 2 * F], f32, tag="st")
                    tmp = miscp.tile([1, 2 * F], f32, tag="sttmp")
                    mu2 = miscp.tile([1, F], f32, tag="mu2")
                    nc.vector.tensor_tensor(out=mu2[:], in0=stats_g[:, 0:F],
                                            in1=stats_g[:, 0:F], op=ALU.mult)
                    nc.vector.tensor_tensor(out=tmp[:, F:2 * F],
                                            in0=stats_g[:, F:2 * F],
                                            in1=mu2[:], op=ALU.subtract)
                    nc.scalar.activation(tmp[:, F:2 * F], tmp[:, F:2 * F],
                                         AF.Sqrt, bias=eps_t[:])
                    nc.vector.reciprocal(tmp[:, F:2 * F], tmp[:, F:2 * F])
                    nc.vector.tensor_tensor(out=st[:, 0:F],
                                            in0=tmp[:, F:2 * F],
                                            in1=gb_sb[li][:, 0:F], op=ALU.mult)
                    nc.vector.tensor_tensor(out=mu2[:], in0=stats_g[:, 0:F],
                                            in1=st[:, 0:F], op=ALU.mult)
                    nc.vector.tensor_tensor(out=st[:, F:2 * F],
                                            in0=gb_sb[li][:, F:2 * F],
                                            in1=mu2[:], op=ALU.subtract)
                    pss = pslin.tile([128, 512], f32, tag="lin", name="pss")
                    nc.tensor.transpose(pss[:2 * F, 0:1], st[:],
                                        ident_f[:1, :1])
                    stv = miscp.tile([128, 1], f32, tag="stv")
                    nc.vector.tensor_copy(stv[:2 * F, :], pss[:2 * F, 0:1])
                    # broadcast (s,t) to all 128 partitions via selT matmuls
                    psc = pslin.tile([128, 512], f32, tag="lin", name="psc")
                    nc.tensor.matmul(psc[:, 0:1], sT_sb[F][:2 * F, 0:128],
                                     stv[:2 * F, :], start=True, stop=True)
                    nc.tensor.matmul(psc[:, 1:2], sT_sb[F][:2 * F, 128:256],
                                     stv[:2 * F, :], start=True, stop=True)
                    stc = miscp.tile([128, 2], f32, tag=f"stc{ar_idx}")
                    nc.vector.tensor_copy(stc[:], psc[:, 0:2])
                    ar_idx += 1
                    # chunked scale+relu (+ up4 replication for next layer)
                    ncfg = CFGS[li + 1]
                    if ncfg.up4:
                        XFrep_cur = poolA.tile(
                            [128, ncfg.nG * ncfg.V], bf16, tag="A")
                        s_r = XFn[:].rearrange("p (g w) -> p g w", w=V)
                        d_r = XFrep_cur[:].rearrange(
                            "p (g w r) -> p g w r", w=V, r=4)
                    nap = max(1, nGp // 4)
                    bnds = [0, 1] + list(range(1 + nap, nGp, nap)) + [nGp]
                    bnds = sorted(set(b for b in bnds if b <= nGp))
                    for q0, q1 in zip(bnds[:-1], bnds[1:]):
                        c0_, c1_ = q0 * V, q1 * V
                        cm = c0_ + (c1_ - c0_) * 5 // 9  # Act a bit slower
                        nc.scalar.activation(
                            XFn[:, c0_:cm], XFn[:, c0_:cm],
                            AF.Relu, scale=stc[:, 0:1], bias=stc[:, 1:2])
                        nc.vector.tensor_scalar(
                            out=XFn[:, cm:c1_], in0=XFn[:, cm:c1_],
                            scalar1=stc[:, 0:1], scalar2=stc[:, 1:2],
                            op0=ALU.mult, op1=ALU.add)
                        nc.vector.tensor_scalar_max(
                            XFn[:, cm:c1_], XFn[:, cm:c1_], 0.0)
                        if ncfg.up4:
                            for r in range(4):
                                nc.vector.tensor_copy(
                                    d_r[:, q0:q1, :, r], s_r[:, q0:q1, :])
                    XF_cur = XFn
                else:
                    # --- stage output: ytile [v, fo*32+b] -> [b, v*3+fo] ---
                    for t in range(cfg.nVt):
                        pt = pstr.tile([128, 512], bf16, tag="tr")
                        nc.tensor.transpose(
                            pt[:96, :128],
                            ytile[:128, t * BF:(t + 1) * BF],
                            ident_b[:128, :128])
                        och = outp.tile([BL, 384], bf16, tag="out")
                        for fo in range(3):
                            nc.scalar.activation(
                                och[:].rearrange("b (v f) -> b v f", f=3)
                                [:, :, fo],
                                pt[fo * 32:(fo + 1) * 32, :128],
                                AF.Copy)
                        nc.sync.dma_start(
                            ydram[:, t * 384:(t + 1) * 384], och[:])

    nc.compile()
    return nc


def kernel(**inputs):
    import sys
    for p in ("/opt/trn_rl_repo", "/opt/trn_rl_repo/concourse"):
        if p not in sys.path:
            sys.path.insert(0, p)
    from concourse.bass_utils import run_bass_kernel_spmd

    host = _build_host(inputs)

    if "nc" not in _CACHE:
        _CACHE["nc"] = _build_nc()
    nc = _CACHE["nc"]

    in_maps = []
    for c in range(NCORES):
        m = {k: v for k, v in host.items() if k != "xT"}
        m["xT"] = np.ascontiguousarray(host["xT"][:, c * BL:(c + 1) * BL])
        in_maps.append(m)
    res = run_bass_kernel_spmd(nc, in_maps, core_ids=list(range(NCORES)))
    out = np.concatenate(
        [np.asarray(r["y"], np.float32).reshape(BL, 1280, 3)
         for r in res.results], axis=0)
    return out.astype(np.float32)


if __name__ == "__main__":
    import reference as R
    inp = R.setup_inputs()
    inp = {k: np.asarray(v) for k, v in inp.items()}
    act = kernel(**inp)
    exp = np.asarray(R.reference(**inp))
    err = np.linalg.norm(act - exp) / np.linalg.norm(exp)
    print("Relative error:", err)


# revision 50
# speedup vs baseline: 1.5960x; 1.0312x over previous
"""Trainium2 Bass kernel for nn_Graph_CNN_Feat_Mesh (Chebyshev GNN decoder).

Strategy (per-core, data-parallel over batch B=256 -> 32/core):
  - All spmms are dense matmuls on the tensor engine (PE) in bf16:
      y = A + L @ (B + L @ (2C)),  A/B/C = feature-space linears of the input.
    L is densified on host; for up4-preceded layers the replication is folded
    into LU = L @ U (contracting the small pre-upsample vertex space).
  - B and A linear terms accumulate directly into the spmm PSUM.
  - Activations live in packed F-layout [(j,Fin) partitions, (b//G)*Vsp + v]
    between layers; the per-layer linear emits V-layout directly; one PE
    transpose per layer returns to F-layout.
  - BatchNorm (training mode, global batch stats) is exact: per-core partial
    sums are AllGather'd across the 8 cores in-kernel (cheaper than
    AllReduce) and summed locally with a K=8 ones-matmul; stats are taken
    per transpose-group so they finish with the last transpose; scale+relu
    is chunked so the next layer starts on early chunks.
  - Weights are host-pre-tiled into [128, *] monoliths and streamed with a
    handful of large DMAs on the gpsimd queue (25ns issue) in consumption
    order; the FC head runs in bf16 with fp32 PSUM.
"""

import numpy as np

B = 256
NCORES = 8
BL = B // NCORES  # 32
EPS = 1e-5
USE_RDMA = False  # remote-DMA BN exchange: unsupported by the timing sim

_CACHE = {}


def _split_W(W):
    W = np.asarray(W, np.float32)
    return W[:, 0::3], W[:, 1::3], W[:, 2::3]


def _dense_L(rows, cols, vals, V):
    L = np.zeros((V, V), np.float32)
    np.add.at(L, (np.asarray(rows), np.asarray(cols)), np.asarray(vals, np.float32))
    return L


def _tile128(a):
    """[S*128, N] -> [128, S*N] block-column layout (pad rows to mult of 128)."""
    a = np.asarray(a)
    S = (a.shape[0] + 127) // 128
    if a.shape[0] != S * 128:
        a = np.concatenate(
            [a, np.zeros((S * 128 - a.shape[0], a.shape[1]), a.dtype)], 0)
    return np.ascontiguousarray(
        a.reshape(S, 128, a.shape[1]).transpose(1, 0, 2).reshape(128, -1))


class _LCfg:
    def __init__(self, name, Vsp, V, Fin, Fout, up4, bn):
        self.name = name
        self.Vsp = Vsp      # source vertex space of C-linear (pre-up4)
        self.V = V          # output vertex count
        self.Fin = Fin
        self.Fout = Fout
        self.G = 128 // Fin          # batches packed on partitions at input
        self.nG = BL // self.G
        self.GF = self.G * Fout      # N of one B/C/A-linear matmul
        self.Gp = 128 // Fout if Fout in (32, 64) else None
        self.nGp = BL // self.Gp if self.Gp else None
        self.up4 = up4
        self.bn = bn
        self.nVt = (V + 127) // 128
        self.nVsp = (Vsp + 127) // 128
        self.BF = BL * Fout          # free width of V-layout per vtile

    def vts(self, t):
        return min(128, self.V - t * 128)

    def sps(self, s):
        return min(128, self.Vsp - s * 128)


CFGS = [
    _LCfg("c0", 80, 320, 64, 64, True, True),
    _LCfg("c1", 320, 320, 64, 32, False, True),
    _LCfg("c2", 320, 1280, 32, 32, True, True),
    _LCfg("c3", 1280, 1280, 32, 3, False, False),
]


def _wbd(W, G, Fin, Fout, which):
    """Block-diagonal rhs weight [128, G*Fout] for the fused linear.
    which: 'A' -> W0 - W2, 'B' -> W1, 'C' -> 2*W2.  col = j*Fout + c."""
    W0, W1, W2 = _split_W(W)
    M = {"A": W0 - W2, "B": W1, "C": 2.0 * W2}[which]  # [Fout, Fin]
    out = np.zeros((128, G * Fout), np.float32)
    for j in range(G):
        out[j * Fin:(j + 1) * Fin, j * Fout:(j + 1) * Fout] = M.T
    return out


# column offsets inside the packed weight blobs
_WOFF = {}
_off = 0
for _cfg in CFGS:
    for _w in "ABC":
        _WOFF[f"{_w}{_cfg.name}"] = (_off, _cfg.GF)
        _off += _cfg.GF
WPACK_N = _off  # bf16 pack cols

# f32 pack: fc1b | per-layer FD-scaled sel blocks (BN partial-sum reduce)
F32_FC1B = 0
F32_SEL = [4, 68, 100]   # selFD for bn layers 0,1,2 (widths 64,32,32)
F32PACK_N = 132
_BN_F = [64, 32, 32]
_BN_FD = [16 * 320, 8 * 320, 8 * 1280]
_BN_NG = [256 * 320, 256 * 320, 256 * 1280]
# with equal per-core/per-group counts, global mu = sum of partition means
# scaled by FD/NG; same factor turns summed (mean^2+var) into E[y^2]
_BN_SCL = [fd / ng for fd, ng in zip(_BN_FD, _BN_NG)]


def _build_host(inputs):
    import ml_dtypes
    bf = ml_dtypes.bfloat16
    f32 = np.float32
    d = {}
    d["xT"] = np.ascontiguousarray(np.asarray(inputs["x"], f32).T).astype(bf)
    d["fc1wt"] = _tile128(
        np.asarray(inputs["fc1_w"], f32).T).astype(bf)           # [128, 16*512]
    d["fc1b"] = np.ascontiguousarray(
        np.asarray(inputs["fc1_b"], f32).reshape(4, 128).T)      # [128,4]
    # fc2: chunk-major (mc), then k-tile: [128, 16*1280]
    w2 = np.asarray(inputs["fc2_w"], f32).T                      # [512, 5120]
    blk = [w2[kt * 128:(kt + 1) * 128, mc * 1280:(mc + 1) * 1280]
           for mc in range(4) for kt in range(4)]
    d["fc2wt"] = np.ascontiguousarray(np.concatenate(blk, 1)).astype(bf)

    L1 = _dense_L(inputs["L1_rows"], inputs["L1_cols"], inputs["L1_vals"], 320)
    L2 = _dense_L(inputs["L2_rows"], inputs["L2_cols"], inputs["L2_vals"], 1280)
    U1 = np.repeat(np.eye(80, dtype=f32), 4, axis=0)    # [320, 80]
    U2 = np.repeat(np.eye(320, dtype=f32), 4, axis=0)   # [1280, 320]
    f8 = ml_dtypes.float8_e4m3
    d["LU0"] = _tile128((L1 @ U1).T).astype(bf)         # [128, 320]
    d["LT1"] = _tile128(L1.T).astype(bf)                # [128, 3*320]
    d["LU2"] = _tile128((L2 @ U2).T).astype(f8)         # [128, 3*1280] fp8
    d["LT2"] = _tile128(L2.T).astype(f8)                # [128, 10*1280] fp8

    Wn = {"c0": "cl0_w", "c1": "cl1_w", "c2": "cl2_w", "c3": "cl3_w"}
    wall = np.zeros((128, WPACK_N), f32)
    for cfg in CFGS:
        W = np.asarray(inputs[Wn[cfg.name]], f32)
        for which in "ABC":
            o, n = _WOFF[f"{which}{cfg.name}"]
            wall[:, o:o + n] = _wbd(W, cfg.G, cfg.Fin, cfg.Fout, which)
    d["wall"] = wall.astype(bf)
    # b3 tiled over the (b, c) column layout of the last-layer PSUM: col = b*3+c
    d["b3row"] = np.ascontiguousarray(
        np.tile(np.asarray(inputs["cl3_b"], f32), BL)[None, :]).astype(bf)

    gbp = np.zeros((1, 256), f32)
    for i, (g, b, o, F) in enumerate([("bn0_g", "bn0_b", 0, 64),
                                      ("bn1_g", "bn1_b", 128, 32),
                                      ("bn2_g", "bn2_b", 192, 32)]):
        gbp[0, o:o + F] = np.asarray(inputs[g], f32)
        gbp[0, o + F:o + 2 * F] = np.asarray(inputs[b], f32)
    d["gbpack"] = gbp

    f32p = np.zeros((128, F32PACK_N), f32)
    for li in range(3):
        F, o = _BN_F[li], F32_SEL[li]
        v = _BN_SCL[li] if USE_RDMA else _BN_FD[li]
        for j in range(128 // F):
            f32p[j * F:(j + 1) * F, o:o + F] += v * np.eye(F, dtype=f32)
    f32p[:, F32_FC1B:F32_FC1B + 4] = d.pop("fc1b")
    d["f32pack"] = f32p
    # selT_s/selT_t [2F rows, 128]: stc[p,:] = (s[p%F], t[p%F]) via 2 matmuls
    stp = np.zeros((128, 2 * 128), f32)
    for F, ro in [(64, 0), (32, 0)]:
        pass
    sT = np.zeros((128, 256), f32)   # rows k (2F<=128), cols: [0:128]=s-map, [128:256]=t-map
    # build per-F maps stacked by row-offset: F=64 uses rows 0:128, F=32 uses rows 0:64
    sT64 = np.zeros((128, 256), f32)
    for p in range(128):
        sT64[p % 64, p] = 1.0          # k = c        -> s
        sT64[64 + p % 64, 128 + p] = 1.0  # k = F + c  -> t
    sT32 = np.zeros((64, 256), f32)
    for p in range(128):
        sT32[p % 32, p] = 1.0
        sT32[32 + p % 32, 128 + p] = 1.0
    d["selT64"] = sT64
    d["selT32"] = np.concatenate([sT32, np.zeros((64, 256), f32)], 0)
    return d


def _build_nc():
    import sys
    for p in ("/opt/trn_rl_repo", "/opt/trn_rl_repo/concourse"):
        if p not in sys.path:
            sys.path.insert(0, p)
    import concourse.bass as bass  # noqa
    import concourse.mybir as mybir
    import concourse.tile as tile
    from concourse import bacc
    from concourse.masks import make_identity

    f32 = mybir.dt.float32
    bf16 = mybir.dt.bfloat16
    fp8 = mybir.dt.float8e4
    DR = mybir.MatmulPerfMode.DoubleRow
    AF = mybir.ActivationFunctionType
    ALU = mybir.AluOpType

    nc = bacc.Bacc(None, target_bir_lowering=False)

    xT = nc.dram_tensor("xT", [2048, BL], bf16, kind="ExternalInput")
    fc1wt = nc.dram_tensor("fc1wt", [128, 16 * 512], bf16, kind="ExternalInput")
    fc2wt = nc.dram_tensor("fc2wt", [128, 16 * 1280], bf16, kind="ExternalInput")
    LU0 = nc.dram_tensor("LU0", [128, 320], bf16, kind="ExternalInput")
    LT1 = nc.dram_tensor("LT1", [128, 3 * 320], bf16, kind="ExternalInput")
    LU2 = nc.dram_tensor("LU2", [128, 3 * 1280], fp8, kind="ExternalInput")
    LT2 = nc.dram_tensor("LT2", [128, 10 * 1280], fp8, kind="ExternalInput")
    wall = nc.dram_tensor("wall", [128, WPACK_N], bf16, kind="ExternalInput")
    gbpack = nc.dram_tensor("gbpack", [1, 256], f32, kind="ExternalInput")
    f32pack = nc.dram_tensor("f32pack", [128, F32PACK_N], f32, kind="ExternalInput")
    selT64 = nc.dram_tensor("selT64", [128, 256], f32, kind="ExternalInput")
    selT32 = nc.dram_tensor("selT32", [128, 256], f32, kind="ExternalInput")
    b3row = nc.dram_tensor("b3row", [1, 96], bf16, kind="ExternalInput")
    ydram = nc.dram_tensor("y", [128, 960], bf16, kind="ExternalOutput")

    with tile.TileContext(nc) as tc:
        with (
            tc.tile_pool(name="const", bufs=1) as constp,
            tc.tile_pool(name="wpool", bufs=1) as wpool,
            tc.tile_pool(name="poolA", bufs=2) as poolA,
            tc.tile_pool(name="poolB", bufs=2) as poolB,
            tc.tile_pool(name="poolC", bufs=1) as poolC,
            tc.tile_pool(name="misc", bufs=1) as miscp,
            tc.tile_pool(name="outp", bufs=3) as outp,
            tc.tile_pool(name="pslin", bufs=2, space="PSUM") as pslin,
            tc.tile_pool(name="psbig", bufs=2, space="PSUM") as psbig,
            tc.tile_pool(name="pstr", bufs=2, space="PSUM") as pstr,
            tc.tile_pool(name="dram", bufs=1, space="DRAM") as dramp,
        ):
            # ---- fc1 inputs first: these DMAs gate the first matmul ----
            fc1w_sb = poolA.tile([128, 16 * 512], bf16, tag="A")
            nc.gpsimd.dma_start(fc1w_sb[:, :4 * 512], fc1wt[:, :4 * 512])
            xT_sb = miscp.tile([128, 16 * BL], bf16, tag="xT")
            nc.gpsimd.dma_start(
                xT_sb[:].rearrange("p (k b) -> p k b", b=BL),
                xT[:].rearrange("(k p) b -> p k b", p=128))
            for kc in range(1, 4):
                nc.gpsimd.dma_start(
                    fc1w_sb[:, kc * 4 * 512:(kc + 1) * 4 * 512],
                    fc1wt[:, kc * 4 * 512:(kc + 1) * 4 * 512])
            f32_sb = constp.tile([128, F32PACK_N], f32, tag="f32pack")
            nc.sync.dma_start(f32_sb[:], f32pack[:])
            selfd_sb = [f32_sb[:, F32_SEL[li]:F32_SEL[li] + _BN_F[li]]
                        for li in range(3)]
            fc1b_sb = f32_sb[:, F32_FC1B:F32_FC1B + 4]

            # ---- small constants (no DMA) ----
            ident_b = constp.tile([128, 128], bf16, tag="identb")
            make_identity(nc, ident_b[:])
            ident_f = constp.tile([1, 1], f32, tag="identf")
            nc.gpsimd.memset(ident_f[:], 1.0)
            eps_t = constp.tile([1, 1], f32, tag="eps")
            nc.gpsimd.memset(eps_t[:], EPS)
            onesn = constp.tile([8, 3], f32, tag="onesn")
            for li in range(3):
                nc.gpsimd.memset(onesn[:, li:li + 1], 1.0 / _BN_NG[li])
            onesv = constp.tile([1, 128], bf16, tag="onesv")
            nc.gpsimd.memset(onesv[:], 1.0)
            sq_warm = constp.tile([1, 1], f32, tag="sqwarm")
            nc.scalar.activation(sq_warm[:], eps_t[:], AF.Sqrt, bias=eps_t[:])

            # ================= FC head (bf16, fp32 psum) =================
            # kt-outer so matmuls start as soon as the first fc1w chunk lands
            h1T = miscp.tile([128, 4 * BL], bf16, tag="h1T")
            ps1 = pslin.tile([128, 4 * BL], f32, tag="lin")
            for mt in range(4):
                for kt in range(16):
                    nc.tensor.matmul(
                        ps1[:, mt * BL:(mt + 1) * BL],
                        fc1w_sb[:, kt * 512 + mt * 128: kt * 512 + (mt + 1) * 128],
                        xT_sb[:, kt * BL:(kt + 1) * BL],
                        start=(kt == 0), stop=(kt == 15))
                nc.scalar.activation(
                    h1T[:, mt * BL:(mt + 1) * BL], ps1[:, mt * BL:(mt + 1) * BL],
                    AF.Relu, bias=fc1b_sb[:, mt:mt + 1])

            # ---- mid-priority loads (small; needed for c0/c1 + BN) ----
            gb_all = constp.tile([1, 256], f32, tag="gbp")
            nc.sync.dma_start(gb_all[:], gbpack[:])
            gb_sb = [gb_all[:, 0:128], gb_all[:, 128:192], gb_all[:, 192:256]]
            sT_sb = {64: constp.tile([128, 256], f32, tag="sT64", name="sT64sb"),
                     32: constp.tile([128, 256], f32, tag="sT32", name="sT32sb")}
            nc.sync.dma_start(sT_sb[64][:], selT64[:])
            nc.sync.dma_start(sT_sb[32][:], selT32[:])
            b3_sb = constp.tile([1, 96], bf16, tag="b3row")
            nc.sync.dma_start(b3_sb[:], b3row[:])

            LUT, LTd = {}, {}
            t = wpool.tile([128, 320], bf16, tag="LU0")
            nc.sync.dma_start(t[:], LU0[:])
            LUT["c0"] = t
            t = wpool.tile([128, 3 * 320], bf16, tag="LT1")
            nc.sync.dma_start(t[:], LT1[:])
            LTd["c0"] = LTd["c1"] = LUT["c1"] = t
            wall_sb = wpool.tile([128, WPACK_N], bf16, tag="wall")
            nc.sync.dma_start(wall_sb[:], wall[:])
            W_sb = {}
            for cfg in CFGS:
                for w in "ABC":
                    o, n = _WOFF[f"{w}{cfg.name}"]
                    W_sb[f"{w}{cfg.name}"] = wall_sb[:, o:o + n]

            # ================= fc2 (streamed in 4 column-chunks) =========
            # psum partition = (v0%2)*64+f, col = mi*BL+b ; channels c = v0*64+f.
            # dest: XF0[(b%2)*64+f, (b//2)*80 + v0],  v0 = 2*(mc*10+mi)+p0
            XF0 = poolC.tile([128, 16 * 80], bf16, tag="XF0")
            cfg0 = CFGS[0]
            XFrep0 = poolA.tile([128, cfg0.nG * cfg0.V], bf16, tag="A")
            s_r0 = XF0[:].rearrange("p (g w) -> p g w", w=80)
            d_r0 = XFrep0[:].rearrange("p (g w r) -> p g w r", w=80, r=4)
            for mc in range(4):
                wch = poolB.tile([128, 4 * 1280], bf16, tag="B")
                nc.gpsimd.dma_start(
                    wch[:], fc2wt[:, mc * 4 * 1280:(mc + 1) * 4 * 1280])
                ps2 = psbig.tile([128, 10 * BL], f32, tag="big")
                for mi in range(10):
                    for kt in range(4):
                        nc.tensor.matmul(
                            ps2[:, mi * BL:(mi + 1) * BL],
                            wch[:, kt * 1280 + mi * 128: kt * 1280 + (mi + 1) * 128],
                            h1T[:, kt * BL:(kt + 1) * BL],
                            start=(kt == 0), stop=(kt == 3))
                src4 = ps2[:].rearrange("p (i g j) -> p i g j", g=16, j=2)
                dst4 = XF0[:].rearrange("p (g u q) -> p g u q", u=40, q=2)
                for p0 in range(2):
                    for j in range(2):
                        nc.scalar.activation(
                            dst4[j * 64:(j + 1) * 64, :,
                                 mc * 10:(mc + 1) * 10, p0]
                            .rearrange("p g i -> p i g"),
                            src4[p0 * 64:(p0 + 1) * 64, :, :, j],
                            AF.Copy)
                # up4-replicate this chunk's w-range (w = v0 in [20mc, 20mc+20))
                for r in range(4):
                    nc.vector.tensor_copy(
                        d_r0[:, :, 20 * mc:20 * (mc + 1), r],
                        s_r0[:, :, 20 * mc:20 * (mc + 1)])

            # ---- big late loads (needed at c2; stream during c0/c1) ----
            t = wpool.tile([128, 3 * 1280], fp8, tag="LU2")
            nc.gpsimd.dma_start(t[:], LU2[:])
            LUT["c2"] = t
            t = wpool.tile([128, 10 * 1280], fp8, tag="LT2")
            nc.gpsimd.dma_start(t[:, :5 * 1280], LT2[:, :5 * 1280])
            nc.gpsimd.dma_start(t[:, 5 * 1280:], LT2[:, 5 * 1280:])
            LTd["c2"] = LTd["c3"] = LUT["c3"] = t

            # ================= cheby layers =================
            if USE_RDMA:
                rsem = nc.alloc_semaphore("bn_rsem")
                lsem = nc.alloc_semaphore("bn_lsem")
                rbufs = [constp.tile([128, 16], f32, tag=f"rbuf{i}",
                                     name=f"rbuf{i}")
                         for i in range(3)]
            XF_cur = XF0
            XFrep_cur = XFrep0
            ar_idx = 0

            for li, cfg in enumerate(CFGS):
                V, Vsp, F = cfg.V, cfg.Vsp, cfg.Fout
                BF = cfg.BF
                last = cfg.name == "c3"
                XFrep = XFrep_cur if cfg.up4 else XF_cur

                fp8sp = cfg.name in ("c2", "c3")
                sp_dt = fp8 if fp8sp else bf16
                # --- C linear (in Vsp space) ---
                XC = poolC.tile([128, cfg.nVsp * BL * F], sp_dt, tag="XC")
                gpack = max(1, 512 // cfg.GF)
                for s in range(cfg.nVsp):
                    ssz = cfg.sps(s)
                    for g0 in range(0, cfg.nG, gpack):
                        gn = min(gpack, cfg.nG - g0)
                        pc = pslin.tile([128, 512], f32, tag="lin")
                        for gi in range(gn):
                            g = g0 + gi
                            nc.tensor.matmul(
                                pc[:ssz, gi * cfg.GF:(gi + 1) * cfg.GF],
                                XF_cur[:, g * Vsp + s * 128:
                                       g * Vsp + s * 128 + ssz],
                                W_sb[f"C{cfg.name}"][:],
                                start=True, stop=True)
                        nc.scalar.activation(
                            XC[:ssz, s * BL * F + g0 * cfg.GF:
                               s * BL * F + (g0 + gn) * cfg.GF],
                            pc[:ssz, :gn * cfg.GF], AF.Copy)

                # --- inner = LU @ (2C) + B ;  y = L @ inner + A ---
                Xin = poolB.tile([128, cfg.nVt * BF], sp_dt, tag="B")
                ytile = poolC.tile([128, cfg.nVt * BF], bf16, tag="YT")
                for phase in range(2):
                    srcL = LUT[cfg.name] if phase == 0 else LTd[cfg.name]
                    nS = cfg.nVsp if phase == 0 else cfg.nVt
                    ssizes = ([cfg.sps(s) for s in range(nS)] if phase == 0
                              else [cfg.vts(s) for s in range(nS)])
                    rhs = XC if phase == 0 else Xin
                    rhs_w = BL * F if phase == 0 else BF
                    Wacc = W_sb[f"B{cfg.name}" if phase == 0 else f"A{cfg.name}"]
                    dst = Xin if phase == 0 else ytile
                    for t in range(cfg.nVt):
                        vsz = cfg.vts(t)
                        for pc0 in range(0, BF, 1024):
                            pw = min(1024, BF - pc0)
                            pi = psbig.tile([128, max(pw, 512)], f32, tag="big")
                            for nk in range(0, pw, 512):
                                n0 = pc0 + nk
                                n1 = min(n0 + 512, pc0 + pw)
                                if fp8sp:
                                    # fp8 DoubleRow: contract 2 s-tiles/pass
                                    srcr = srcL[:].rearrange(
                                        "p (s v) -> p s v", v=V)
                                    rhsr = rhs[:].rearrange(
                                        "p (s n) -> p s n", n=rhs_w)
                                    for s0 in range(0, nS, 2):
                                        if s0 + 1 < nS and \
                                                ssizes[s0 + 1] == 128:
                                            nc.tensor.matmul(
                                                pi[:vsz, n0 - pc0:n1 - pc0],
                                                srcr[:, s0:s0 + 2,
                                                     t * 128:t * 128 + vsz],
                                                rhsr[:, s0:s0 + 2, n0:n1],
                                                start=(s0 == 0), stop=False,
                                                skip_group_check=True,
                                                perf_mode=DR)
                                        else:
                                            for s in range(s0, min(s0 + 2,
                                                                   nS)):
                                                ssz = ssizes[s]
                                                nc.tensor.matmul(
                                                    pi[:vsz,
                                                       n0 - pc0:n1 - pc0],
                                                    srcL[:ssz, s * V + t * 128:
                                                         s * V + t * 128 + vsz],
                                                    rhs[:ssz, s * rhs_w + n0:
                                                        s * rhs_w + n1],
                                                    start=(s == 0), stop=False,
                                                    skip_group_check=True)
                                else:
                                    for s in range(nS):
                                        ssz = ssizes[s]
                                        nc.tensor.matmul(
                                            pi[:vsz, n0 - pc0:n1 - pc0],
                                            srcL[:ssz, s * V + t * 128:
                                                 s * V + t * 128 + vsz],
                                            rhs[:ssz, s * rhs_w + n0:
                                                s * rhs_w + n1],
                                            start=(s == 0), stop=False,
                                            skip_group_check=True)
                                for g in range(n0 // cfg.GF,
                                               (n1 + cfg.GF - 1) // cfg.GF):
                                    nc.tensor.matmul(
                                        pi[:vsz, g * cfg.GF - pc0:
                                           (g + 1) * cfg.GF - pc0],
                                        XFrep[:, g * V + t * 128:
                                              g * V + t * 128 + vsz],
                                        Wacc[:],
                                        start=False,
                                        stop=(not (last and phase == 1)),
                                        skip_group_check=True)
                                if last and phase == 1:
                                    # fold cl3 bias: += ones^T @ b3row
                                    nc.tensor.matmul(
                                        pi[:vsz, n0 - pc0:n1 - pc0],
                                        onesv[:1, :vsz],
                                        b3_sb[:1, n0:n1],
                                        start=False, stop=True,
                                        skip_group_check=True)
                            if last and phase == 1:
                                nc.vector.tensor_copy(
                                    dst[:vsz, t * BF + pc0: t * BF + pc0 + pw],
                                    pi[:vsz, :pw])
                            elif phase == 0:
                                nc.scalar.activation(
                                    dst[:vsz, t * BF + pc0: t * BF + pc0 + pw],
                                    pi[:vsz, :pw], AF.Copy)
                            else:
                                nc.vector.tensor_copy(
                                    dst[:vsz, t * BF + pc0: t * BF + pc0 + pw],
                                    pi[:vsz, :pw])

                if not last:
                    # --- back-transpose to packed F-layout; stats per group ---
                    Gp, nGp = cfg.Gp, cfg.nGp
                    nq = (nGp + 3) // 4
                    XFn = poolA.tile([128, nGp * V], bf16, tag="A")
                    dstv = XFn[:].rearrange("p (g v) -> p g v", v=V)
                    nch = cfg.nVt * nq + nGp  # worst case incl. partial tiles
                    bnst = miscp.tile([128, nch * 6], f32, tag="bnst")
                    chn = 0
                    for t in range(cfg.nVt):
                        vsz = cfg.vts(t)
                        for qi0 in range(nq):
                            q0 = qi0 * 4
                            qn = min(4, nGp - q0)
                            pt = pstr.tile([128, 512], bf16, tag="tr")
                            for qi in range(qn):
                                gp = q0 + qi
                                nc.tensor.transpose(
                                    pt[:, qi * 128: qi * 128 + vsz],
                                    ytile[:vsz, t * BF + gp * 128:
                                          t * BF + (gp + 1) * 128],
                                    ident_b[:vsz, :vsz])
                            reg = dstv[:, q0:q0 + qn, t * 128:t * 128 + vsz]
                            nc.scalar.activation(
                                reg,
                                pt[:].rearrange("p (q v) -> p q v", v=128)
                                [:, :qn, :vsz],
                                AF.Copy)
                            if vsz == 128:
                                # stats straight off the transpose PSUM tile
                                nc.vector.bn_stats(
                                    bnst[:, chn * 6:(chn + 1) * 6],
                                    pt[:, :qn * 128])
                                chn += 1
                            else:
                                for qi in range(qn):
                                    gp = q0 + qi
                                    nc.vector.bn_stats(
                                        bnst[:, chn * 6:(chn + 1) * 6],
                                        XFn[:, gp * V + t * 128:
                                            gp * V + t * 128 + vsz])
                                    chn += 1
                    aggr = miscp.tile([128, 2], f32, tag="aggr")
                    nc.vector.bn_aggr(
                        aggr[:], bnst[:, :chn * 6]
                        .rearrange("p (c s) -> p c s", s=6))
                    part = miscp.tile([128, 2], f32, tag="part")
                    if USE_RDMA and ar_idx > 0:
                        # prior layer's sends must have drained before reuse
                        nc.vector.wait_ge(lsem, 112 * ar_idx)
                    nc.vector.tensor_tensor(
                        out=part[:, 1:2], in0=aggr[:, 0:1], in1=aggr[:, 0:1],
                        op=ALU.mult)
                    nc.vector.tensor_tensor(
                        out=part[:, 1:2], in0=part[:, 1:2], in1=aggr[:, 1:2],
                        op=ALU.add)
                    if USE_RDMA:
                        nc.vector.tensor_copy(part[:, 0:1], aggr[:, 0:1])
                        rb = rbufs[ar_idx]
                        nc.vector.tensor_copy(rb[:, 0:2], part[:])
                        for k in range(1, 8):
                            nc.gpsimd.remote_dma_broadcast(
                                rb[:, 2 * k:2 * k + 2], part[:],
                                remote_sem=rsem, local_sem=lsem,
                                rdests=[(0, k) if i == k else None
                                        for i in range(8)])
                        nc.gpsimd.trigger_dma(count=None)
                        nc.vector.wait_ge(rsem, 14 * (ar_idx + 1))
                        nc.vector.tensor_tensor(
                            out=rb[:, 0:8], in0=rb[:, 0:8], in1=rb[:, 8:16],
                            op=ALU.add)
                        nc.vector.tensor_tensor(
                            out=rb[:, 0:4], in0=rb[:, 0:4], in1=rb[:, 4:8],
                            op=ALU.add)
                        nc.vector.tensor_tensor(
                            out=rb[:, 0:2], in0=rb[:, 0:2], in1=rb[:, 2:4],
                            op=ALU.add)
                        pst = pslin.tile([128, 512], f32, tag="lin")
                        nc.tensor.matmul(pst[:1, :F], rb[:, 0:1],
                                         selfd_sb[li], start=True, stop=True)
                        nc.tensor.matmul(pst[:1, F:2 * F], rb[:, 1:2],
                                         selfd_sb[li], start=True, stop=True)
                        stats_g = miscp.tile([1, 2 * F], f32, tag="statg")
                        nc.vector.tensor_copy(stats_g[:], pst[:1, :2 * F])
                    else:
                        pst = pslin.tile([128, 512], f32, tag="lin")
                        nc.tensor.matmul(pst[:1, :F], aggr[:, 0:1],
                                         selfd_sb[li], start=True, stop=True)
                        nc.tensor.matmul(pst[:1, F:2 * F], part[:, 1:2],
                                         selfd_sb[li], start=True, stop=True)
                        stats_l = miscp.tile([1, 2 * F], f32, tag="statl")
                        nc.vector.tensor_copy(stats_l[:], pst[:1, :2 * F])
                        bin_ = dramp.tile([1, 2 * F], f32, tag=f"arin{ar_idx}")
                        bout = dramp.tile([8, 2 * F], f32, tag=f"arout{ar_idx}")
                        nc.sync.dma_start(bin_[:], stats_l[:])
                        nc.gpsimd.collective_compute(
                            "AllGather", ALU.bypass,
                            replica_groups=[list(range(NCORES))],
                            ins=[bin_.opt()], outs=[bout.opt()])
                        sg8 = miscp.tile([8, 2 * F], f32, tag="sg8")
                        nc.sync.dma_start(sg8[:], bout[:])
                        psg = pslin.tile([128, 512], f32, tag="lin", name="psg")
                        nc.tensor.matmul(psg[:1, :2 * F], onesn[:, li:li + 1],
                                         sg8[:], start=True, stop=True)
                        stats_g = miscp.tile([1, 2 * F], f32, tag="statg")
                        nc.vector.tensor_copy(stats_g[:], psg[:1, :2 * F])
                    # stats_g = (mu, E[y^2]) ; st cols [0:F]=s, [F:2F]=t
                    st = miscp.tile([1, 2 * F], f32, tag="st")
                    tmp = miscp.tile([1, 2 * F], f32, tag="sttmp")
                    mu2 = miscp.tile([1, F], f32, tag="mu2")
                    nc.vector.tensor_tensor(out=mu2[:], in0=stats_g[:, 0:F],
                                            in1=stats_g[:, 0:F], op=ALU.mult)
                    nc.vector.tensor_tensor(out=tmp[:, F:2 * F],
                                            in0=stats_g[:, F:2 * F],
                                            in1=mu2[:], op=ALU.subtract)
                    nc.scalar.activation(tmp[:, F:2 * F], tmp[:, F:2 * F],
                                         AF.Sqrt, bias=eps_t[:])
                    nc.vector.reciprocal(tmp[:, F:2 * F], tmp[:, F:2 * F])
                    nc.vector.tensor_tensor(out=st[:, 0:F],
                                            in0=tmp[:, F:2 * F],
                                            in1=gb_sb[li][:, 0:F], op=ALU.mult)
                    nc.vector.tensor_tensor(out=mu2[:], in0=stats_g[:, 0:F],
                                            in1=st[:, 0:F], op=ALU.mult)
                    nc.vector.tensor_tensor(out=st[:, F:2 * F],
                                            in0=gb_sb[li][:, F:2 * F],
                                            in1=mu2[:], op=ALU.subtract)
                    pss = pslin.tile([128, 512], f32, tag="lin", name="pss")
                    nc.tensor.transpose(pss[:2 * F, 0:1], st[:],
                                        ident_f[:1, :1])
                    stv = miscp.tile([128, 1], f32, tag="stv")
                    nc.vector.tensor_copy(stv[:2 * F, :], pss[:2 * F, 0:1])
                    # broadcast (s,t) to all 128 partitions via selT matmuls
                    psc = pslin.tile([128, 512], f32, tag="lin", name="psc")
                    nc.tensor.matmul(psc[:, 0:1], sT_sb[F][:2 * F, 0:128],
                                     stv[:2 * F, :], start=True, stop=True)
                    nc.tensor.matmul(psc[:, 1:2], sT_sb[F][:2 * F, 128:256],
                                     stv[:2 * F, :], start=True, stop=True)
                    stc = miscp.tile([128, 2], f32, tag=f"stc{ar_idx}")
                    nc.vector.tensor_copy(stc[:], psc[:, 0:2])
                    ar_idx += 1
                    # chunked scale+relu (+ up4 replication for next layer)
                    ncfg = CFGS[li + 1]
                    if ncfg.up4:
                        XFrep_cur = poolA.tile(
                            [128, ncfg.nG * ncfg.V], bf16, tag="A")
                        s_r = XFn[:].rearrange("p (g w) -> p g w", w=V)
                        d_r = XFrep_cur[:].rearrange(
                            "p (g w r) -> p g w r", w=V, r=4)
                    nap = max(1, nGp // 4)
                    bnds = [0, 1] + list(range(1 + nap, nGp, nap)) + [nGp]
                    bnds = sorted(set(b for b in bnds if b <= nGp))
                    for q0, q1 in zip(bnds[:-1], bnds[1:]):
                        c0_, c1_ = q0 * V, q1 * V
                        cm = c0_ + (c1_ - c0_) * 5 // 9  # Act a bit slower
                        nc.scalar.activation(
                            XFn[:, c0_:cm], XFn[:, c0_:cm],
                            AF.Relu, scale=stc[:, 0:1], bias=stc[:, 1:2])
                        nc.vector.tensor_scalar(
                            out=XFn[:, cm:c1_], in0=XFn[:, cm:c1_],
                            scalar1=stc[:, 0:1], scalar2=stc[:, 1:2],
                            op0=ALU.mult, op1=ALU.add)
                        nc.vector.tensor_scalar_max(
                            XFn[:, cm:c1_], XFn[:, cm:c1_], 0.0)
                        if ncfg.up4:
                            for r in range(4):
                                nc.vector.tensor_copy(
                                    d_r[:, q0:q1, :, r], s_r[:, q0:q1, :])
                    XF_cur = XFn
                else:
                    # --- output: ship ytile [v-part, (b,c)] as-is; host
                    # untangles the (p, t, b, c) layout in numpy ---
                    for t in range(cfg.nVt):
                        nc.sync.dma_start(
                            ydram[:, t * BF:(t + 1) * BF],
                            ytile[:, t * BF:(t + 1) * BF])

    nc.compile()
    return nc


def kernel(**inputs):
    import sys
    for p in ("/opt/trn_rl_repo", "/opt/trn_rl_repo/concourse"):
        if p not in sys.path:
            sys.path.insert(0, p)
    from concourse.bass_utils import run_bass_kernel_spmd

    host = _build_host(inputs)

    if "nc" not in _CACHE:
        _CACHE["nc"] = _build_nc()
    nc = _CACHE["nc"]

    in_maps = []
    for c in range(NCORES):
        m = {k: v for k, v in host.items() if k != "xT"}
        m["xT"] = np.ascontiguousarray(host["xT"][:, c * BL:(c + 1) * BL])
        in_maps.append(m)
    res = run_bass_kernel_spmd(nc, in_maps, core_ids=list(range(NCORES)))
    # y[p, t*96 + b*3 + c] -> out[b, t*128+p, c]
    outs = []
    for r in res.results:
        y = np.asarray(r["y"], np.float32).reshape(128, 10, BL, 3)
        outs.append(y.transpose(2, 1, 0, 3).reshape(BL, 1280, 3))
    return np.concatenate(outs, axis=0).astype(np.float32)


if __name__ == "__main__":
    import reference as R
    inp = R.setup_inputs()
    inp = {k: np.asarray(v) for k, v in inp.items()}
    act = kernel(**inp)
    exp = np.asarray(R.reference(**inp))
    err = np.linalg.norm(act - exp) / np.linalg.norm(exp)
    print("Relative error:", err)


# revision 53
# speedup vs baseline: 1.6566x; 1.0380x over previous
"""Trainium2 Bass kernel for nn_Graph_CNN_Feat_Mesh (Chebyshev GNN decoder).

Strategy (per-core, data-parallel over batch B=256 -> 32/core):
  - All spmms are dense matmuls on the tensor engine (PE) in bf16:
      y = A + L @ (B + L @ (2C)),  A/B/C = feature-space linears of the input.
    L is densified on host; for up4-preceded layers the replication is folded
    into LU = L @ U (contracting the small pre-upsample vertex space).
  - B and A linear terms accumulate directly into the spmm PSUM.
  - Activations live in packed F-layout [(j,Fin) partitions, (b//G)*Vsp + v]
    between layers; the per-layer linear emits V-layout directly; one PE
    transpose per layer returns to F-layout.
  - BatchNorm (training mode, global batch stats) is exact: per-core partial
    sums are AllGather'd across the 8 cores in-kernel (cheaper than
    AllReduce) and summed locally with a K=8 ones-matmul; stats are taken
    per transpose-group so they finish with the last transpose; scale+relu
    is chunked so the next layer starts on early chunks.
  - Weights are host-pre-tiled into [128, *] monoliths and streamed with a
    handful of large DMAs on the gpsimd queue (25ns issue) in consumption
    order; the FC head runs in bf16 with fp32 PSUM.
"""

import numpy as np

B = 256
NCORES = 8
BL = B // NCORES  # 32
EPS = 1e-5
USE_RDMA = False  # remote-DMA BN exchange: unsupported by the timing sim

_CACHE = {}


def _split_W(W):
    W = np.asarray(W, np.float32)
    return W[:, 0::3], W[:, 1::3], W[:, 2::3]


def _dense_L(rows, cols, vals, V):
    L = np.zeros((V, V), np.float32)
    np.add.at(L, (np.asarray(rows), np.asarray(cols)), np.asarray(vals, np.float32))
    return L


def _tile128(a):
    """[S*128, N] -> [128, S*N] block-column layout (pad rows to mult of 128)."""
    a = np.asarray(a)
    S = (a.shape[0] + 127) // 128
    if a.shape[0] != S * 128:
        a = np.concatenate(
            [a, np.zeros((S * 128 - a.shape[0], a.shape[1]), a.dtype)], 0)
    return np.ascontiguousarray(
        a.reshape(S, 128, a.shape[1]).transpose(1, 0, 2).reshape(128, -1))


class _LCfg:
    def __init__(self, name, Vsp, V, Fin, Fout, up4, bn):
        self.name = name
        self.Vsp = Vsp      # source vertex space of C-linear (pre-up4)
        self.V = V          # output vertex count
        self.Fin = Fin
        self.Fout = Fout
        self.G = 128 // Fin          # batches packed on partitions at input
        self.nG = BL // self.G
        self.GF = self.G * Fout      # N of one B/C/A-linear matmul
        self.Gp = 128 // Fout if Fout in (32, 64) else None
        self.nGp = BL // self.Gp if self.Gp else None
        self.up4 = up4
        self.bn = bn
        self.nVt = (V + 127) // 128
        self.nVsp = (Vsp + 127) // 128
        self.BF = BL * Fout          # free width of V-layout per vtile

    def vts(self, t):
        return min(128, self.V - t * 128)

    def sps(self, s):
        return min(128, self.Vsp - s * 128)


CFGS = [
    _LCfg("c0", 80, 320, 64, 64, True, True),
    _LCfg("c1", 320, 320, 64, 32, False, True),
    _LCfg("c2", 320, 1280, 32, 32, True, True),
    _LCfg("c3", 1280, 1280, 32, 3, False, False),
]


def _wbd(W, G, Fin, Fout, which):
    """Block-diagonal rhs weight [128, G*Fout] for the fused linear.
    which: 'A' -> W0 - W2, 'B' -> W1, 'C' -> 2*W2.  col = j*Fout + c."""
    W0, W1, W2 = _split_W(W)
    M = {"A": W0 - W2, "B": W1, "C": 2.0 * W2}[which]  # [Fout, Fin]
    out = np.zeros((128, G * Fout), np.float32)
    for j in range(G):
        out[j * Fin:(j + 1) * Fin, j * Fout:(j + 1) * Fout] = M.T
    return out


# column offsets inside the packed weight blobs
_WOFF = {}
_off = 0
for _cfg in CFGS:
    for _w in "ABC":
        _WOFF[f"{_w}{_cfg.name}"] = (_off, _cfg.GF)
        _off += _cfg.GF
WPACK_N = _off  # bf16 pack cols

# f32 pack: fc1b | per-layer FD-scaled sel blocks (BN partial-sum reduce)
F32_FC1B = 0
F32_SEL = [4, 68, 100]   # selFD for bn layers 0,1,2 (widths 64,32,32)
F32PACK_N = 132
_BN_F = [64, 32, 32]
_BN_FD = [16 * 320, 8 * 320, 8 * 1280]
_BN_NG = [256 * 320, 256 * 320, 256 * 1280]
# with equal per-core/per-group counts, global mu = sum of partition means
# scaled by FD/NG; same factor turns summed (mean^2+var) into E[y^2]
_BN_SCL = [fd / ng for fd, ng in zip(_BN_FD, _BN_NG)]


def _build_host(inputs):
    import ml_dtypes
    bf = ml_dtypes.bfloat16
    f32 = np.float32
    d = {}
    d["xT"] = np.ascontiguousarray(np.asarray(inputs["x"], f32).T).astype(bf)
    d["fc1wt"] = _tile128(
        np.asarray(inputs["fc1_w"], f32).T).astype(bf)           # [128, 16*512]
    d["fc1b"] = np.ascontiguousarray(
        np.asarray(inputs["fc1_b"], f32).reshape(4, 128).T)      # [128,4]
    # fc2: chunk-major (mc), then k-tile: [128, 16*1280]
    w2 = np.asarray(inputs["fc2_w"], f32).T                      # [512, 5120]
    blk = [w2[kt * 128:(kt + 1) * 128, mc * 1280:(mc + 1) * 1280]
           for mc in range(4) for kt in range(4)]
    d["fc2wt"] = np.ascontiguousarray(np.concatenate(blk, 1)).astype(bf)

    L1 = _dense_L(inputs["L1_rows"], inputs["L1_cols"], inputs["L1_vals"], 320)
    L2 = _dense_L(inputs["L2_rows"], inputs["L2_cols"], inputs["L2_vals"], 1280)
    U1 = np.repeat(np.eye(80, dtype=f32), 4, axis=0)    # [320, 80]
    U2 = np.repeat(np.eye(320, dtype=f32), 4, axis=0)   # [1280, 320]
    f8 = ml_dtypes.float8_e4m3
    d["LU0"] = _tile128((L1 @ U1).T).astype(bf)         # [128, 320]
    d["LT1"] = _tile128(L1.T).astype(bf)                # [128, 3*320]
    d["LU2"] = _tile128((L2 @ U2).T).astype(f8)         # [128, 3*1280] fp8
    d["LT2"] = _tile128(L2.T).astype(f8)                # [128, 10*1280] fp8

    Wn = {"c0": "cl0_w", "c1": "cl1_w", "c2": "cl2_w", "c3": "cl3_w"}
    wall = np.zeros((128, WPACK_N), f32)
    for cfg in CFGS:
        W = np.asarray(inputs[Wn[cfg.name]], f32)
        for which in "ABC":
            o, n = _WOFF[f"{which}{cfg.name}"]
            wall[:, o:o + n] = _wbd(W, cfg.G, cfg.Fin, cfg.Fout, which)
    d["wall"] = wall.astype(bf)
    # b3 tiled over the (b, c) column layout of the last-layer PSUM: col = b*3+c
    d["b3row"] = np.ascontiguousarray(
        np.tile(np.asarray(inputs["cl3_b"], f32), BL)[None, :]).astype(bf)

    gbc = np.zeros((128, 6), f32)
    for li, (g, b) in enumerate([("bn0_g", "bn0_b"), ("bn1_g", "bn1_b"),
                                 ("bn2_g", "bn2_b")]):
        F = _BN_F[li]
        gbc[0:F, 2 * li] = np.asarray(inputs[g], f32)
        gbc[0:F, 2 * li + 1] = np.asarray(inputs[b], f32)
    d["gbcol"] = gbc

    f32p = np.zeros((128, F32PACK_N), f32)
    for li in range(3):
        F, o = _BN_F[li], F32_SEL[li]
        v = _BN_SCL[li] if USE_RDMA else _BN_FD[li]
        for j in range(128 // F):
            f32p[j * F:(j + 1) * F, o:o + F] += v * np.eye(F, dtype=f32)
    f32p[:, F32_FC1B:F32_FC1B + 4] = d.pop("fc1b")
    d["f32pack"] = f32p
    # selT_s/selT_t [2F rows, 128]: stc[p,:] = (s[p%F], t[p%F]) via 2 matmuls
    stp = np.zeros((128, 2 * 128), f32)
    for F, ro in [(64, 0), (32, 0)]:
        pass
    sT = np.zeros((128, 256), f32)   # rows k (2F<=128), cols: [0:128]=s-map, [128:256]=t-map
    # build per-F maps stacked by row-offset: F=64 uses rows 0:128, F=32 uses rows 0:64
    sT64 = np.zeros((128, 128), f32)
    sT32 = np.zeros((128, 128), f32)
    for p in range(128):
        sT64[p % 64, p] = 1.0
        sT32[p % 32, p] = 1.0
    d["selT64"] = sT64
    d["selT32"] = sT32
    return d


def _build_nc():
    import sys
    for p in ("/opt/trn_rl_repo", "/opt/trn_rl_repo/concourse"):
        if p not in sys.path:
            sys.path.insert(0, p)
    import concourse.bass as bass  # noqa
    import concourse.mybir as mybir
    import concourse.tile as tile
    from concourse import bacc
    from concourse.masks import make_identity

    f32 = mybir.dt.float32
    bf16 = mybir.dt.bfloat16
    fp8 = mybir.dt.float8e4
    DR = mybir.MatmulPerfMode.DoubleRow
    AF = mybir.ActivationFunctionType
    ALU = mybir.AluOpType

    nc = bacc.Bacc(None, target_bir_lowering=False)

    xT = nc.dram_tensor("xT", [2048, BL], bf16, kind="ExternalInput")
    fc1wt = nc.dram_tensor("fc1wt", [128, 16 * 512], bf16, kind="ExternalInput")
    fc2wt = nc.dram_tensor("fc2wt", [128, 16 * 1280], bf16, kind="ExternalInput")
    LU0 = nc.dram_tensor("LU0", [128, 320], bf16, kind="ExternalInput")
    LT1 = nc.dram_tensor("LT1", [128, 3 * 320], bf16, kind="ExternalInput")
    LU2 = nc.dram_tensor("LU2", [128, 3 * 1280], fp8, kind="ExternalInput")
    LT2 = nc.dram_tensor("LT2", [128, 10 * 1280], fp8, kind="ExternalInput")
    wall = nc.dram_tensor("wall", [128, WPACK_N], bf16, kind="ExternalInput")
    gbcol = nc.dram_tensor("gbcol", [128, 6], f32, kind="ExternalInput")
    f32pack = nc.dram_tensor("f32pack", [128, F32PACK_N], f32, kind="ExternalInput")
    selT64 = nc.dram_tensor("selT64", [128, 128], f32, kind="ExternalInput")
    selT32 = nc.dram_tensor("selT32", [128, 128], f32, kind="ExternalInput")
    b3row = nc.dram_tensor("b3row", [1, 96], bf16, kind="ExternalInput")
    ydram = nc.dram_tensor("y", [128, 960], bf16, kind="ExternalOutput")

    with tile.TileContext(nc) as tc:
        with (
            tc.tile_pool(name="const", bufs=1) as constp,
            tc.tile_pool(name="wpool", bufs=1) as wpool,
            tc.tile_pool(name="poolA", bufs=2) as poolA,
            tc.tile_pool(name="poolB", bufs=2) as poolB,
            tc.tile_pool(name="poolC", bufs=1) as poolC,
            tc.tile_pool(name="misc", bufs=1) as miscp,
            tc.tile_pool(name="outp", bufs=3) as outp,
            tc.tile_pool(name="pslin", bufs=2, space="PSUM") as pslin,
            tc.tile_pool(name="psbig", bufs=2, space="PSUM") as psbig,
            tc.tile_pool(name="pstr", bufs=2, space="PSUM") as pstr,
            tc.tile_pool(name="dram", bufs=1, space="DRAM") as dramp,
        ):
            # ---- fc1 inputs first: these DMAs gate the first matmul ----
            fc1w_sb = poolA.tile([128, 16 * 512], bf16, tag="A")
            nc.gpsimd.dma_start(fc1w_sb[:, :4 * 512], fc1wt[:, :4 * 512])
            xT_sb = miscp.tile([128, 16 * BL], bf16, tag="xT")
            nc.gpsimd.dma_start(
                xT_sb[:].rearrange("p (k b) -> p k b", b=BL),
                xT[:].rearrange("(k p) b -> p k b", p=128))
            for kc in range(1, 4):
                nc.gpsimd.dma_start(
                    fc1w_sb[:, kc * 4 * 512:(kc + 1) * 4 * 512],
                    fc1wt[:, kc * 4 * 512:(kc + 1) * 4 * 512])
            f32_sb = constp.tile([128, F32PACK_N], f32, tag="f32pack")
            nc.sync.dma_start(f32_sb[:], f32pack[:])
            selfd_sb = [f32_sb[:, F32_SEL[li]:F32_SEL[li] + _BN_F[li]]
                        for li in range(3)]
            fc1b_sb = f32_sb[:, F32_FC1B:F32_FC1B + 4]

            # ---- small constants (no DMA) ----
            ident_b = constp.tile([128, 128], bf16, tag="identb")
            make_identity(nc, ident_b[:])
            ident_f = constp.tile([1, 1], f32, tag="identf")
            nc.gpsimd.memset(ident_f[:], 1.0)
            eps_t = constp.tile([1, 1], f32, tag="eps")
            nc.gpsimd.memset(eps_t[:], EPS)
            onesn = constp.tile([8, 3], f32, tag="onesn")
            for li in range(3):
                nc.gpsimd.memset(onesn[:, li:li + 1], 1.0 / _BN_NG[li])
            onesv = constp.tile([1, 128], bf16, tag="onesv")
            nc.gpsimd.memset(onesv[:], 1.0)
            sq_warm = constp.tile([1, 1], f32, tag="sqwarm")
            nc.scalar.activation(sq_warm[:], eps_t[:], AF.Sqrt, bias=eps_t[:])

            # ================= FC head (bf16, fp32 psum) =================
            # kt-outer so matmuls start as soon as the first fc1w chunk lands
            h1T = miscp.tile([128, 4 * BL], bf16, tag="h1T")
            ps1 = pslin.tile([128, 4 * BL], f32, tag="lin")
            for mt in range(4):
                for kt in range(16):
                    nc.tensor.matmul(
                        ps1[:, mt * BL:(mt + 1) * BL],
                        fc1w_sb[:, kt * 512 + mt * 128: kt * 512 + (mt + 1) * 128],
                        xT_sb[:, kt * BL:(kt + 1) * BL],
                        start=(kt == 0), stop=(kt == 15))
                nc.scalar.activation(
                    h1T[:, mt * BL:(mt + 1) * BL], ps1[:, mt * BL:(mt + 1) * BL],
                    AF.Relu, bias=fc1b_sb[:, mt:mt + 1])

            # ---- mid-priority loads (small; needed for c0/c1 + BN) ----
            gbc_sb = constp.tile([128, 6], f32, tag="gbc")
            nc.sync.dma_start(gbc_sb[:], gbcol[:])
            epsc = constp.tile([128, 1], f32, tag="epsc")
            nc.gpsimd.memset(epsc[:], EPS)
            sT_sb = {64: constp.tile([128, 128], f32, tag="sT64", name="sT64sb"),
                     32: constp.tile([128, 128], f32, tag="sT32", name="sT32sb")}
            nc.sync.dma_start(sT_sb[64][:], selT64[:])
            nc.sync.dma_start(sT_sb[32][:], selT32[:])
            b3_sb = constp.tile([1, 96], bf16, tag="b3row")
            nc.sync.dma_start(b3_sb[:], b3row[:])

            LUT, LTd = {}, {}
            t = wpool.tile([128, 320], bf16, tag="LU0")
            nc.sync.dma_start(t[:], LU0[:])
            LUT["c0"] = t
            t = wpool.tile([128, 3 * 320], bf16, tag="LT1")
            nc.sync.dma_start(t[:], LT1[:])
            LTd["c0"] = LTd["c1"] = LUT["c1"] = t
            wall_sb = wpool.tile([128, WPACK_N], bf16, tag="wall")
            nc.sync.dma_start(wall_sb[:], wall[:])
            W_sb = {}
            for cfg in CFGS:
                for w in "ABC":
                    o, n = _WOFF[f"{w}{cfg.name}"]
                    W_sb[f"{w}{cfg.name}"] = wall_sb[:, o:o + n]

            # ================= fc2 (streamed in 4 column-chunks) =========
            # psum partition = (v0%2)*64+f, col = mi*BL+b ; channels c = v0*64+f.
            # dest: XF0[(b%2)*64+f, (b//2)*80 + v0],  v0 = 2*(mc*10+mi)+p0
            XF0 = poolC.tile([128, 16 * 80], bf16, tag="XF0")
            cfg0 = CFGS[0]
            XFrep0 = poolA.tile([128, cfg0.nG * cfg0.V], bf16, tag="A")
            s_r0 = XF0[:].rearrange("p (g w) -> p g w", w=80)
            d_r0 = XFrep0[:].rearrange("p (g w r) -> p g w r", w=80, r=4)
            for mc in range(4):
                wch = poolB.tile([128, 4 * 1280], bf16, tag="B")
                nc.gpsimd.dma_start(
                    wch[:], fc2wt[:, mc * 4 * 1280:(mc + 1) * 4 * 1280])
                ps2 = psbig.tile([128, 10 * BL], f32, tag="big")
                for mi in range(10):
                    for kt in range(4):
                        nc.tensor.matmul(
                            ps2[:, mi * BL:(mi + 1) * BL],
                            wch[:, kt * 1280 + mi * 128: kt * 1280 + (mi + 1) * 128],
                            h1T[:, kt * BL:(kt + 1) * BL],
                            start=(kt == 0), stop=(kt == 3))
                src4 = ps2[:].rearrange("p (i g j) -> p i g j", g=16, j=2)
                dst4 = XF0[:].rearrange("p (g u q) -> p g u q", u=40, q=2)
                for p0 in range(2):
                    for j in range(2):
                        nc.scalar.activation(
                            dst4[j * 64:(j + 1) * 64, :,
                                 mc * 10:(mc + 1) * 10, p0]
                            .rearrange("p g i -> p i g"),
                            src4[p0 * 64:(p0 + 1) * 64, :, :, j],
                            AF.Copy)
                # up4-replicate this chunk's w-range (w = v0 in [20mc, 20mc+20))
                for r in range(4):
                    nc.vector.tensor_copy(
                        d_r0[:, :, 20 * mc:20 * (mc + 1), r],
                        s_r0[:, :, 20 * mc:20 * (mc + 1)])

            # ---- big late loads (needed at c2; stream during c0/c1) ----
            t = wpool.tile([128, 3 * 1280], fp8, tag="LU2")
            nc.gpsimd.dma_start(t[:], LU2[:])
            LUT["c2"] = t
            t = wpool.tile([128, 10 * 1280], fp8, tag="LT2")
            nc.gpsimd.dma_start(t[:, :5 * 1280], LT2[:, :5 * 1280])
            nc.gpsimd.dma_start(t[:, 5 * 1280:], LT2[:, 5 * 1280:])
            LTd["c2"] = LTd["c3"] = LUT["c3"] = t

            # ================= cheby layers =================
            if USE_RDMA:
                rsem = nc.alloc_semaphore("bn_rsem")
                lsem = nc.alloc_semaphore("bn_lsem")
                rbufs = [constp.tile([128, 16], f32, tag=f"rbuf{i}",
                                     name=f"rbuf{i}")
                         for i in range(3)]
            XF_cur = XF0
            XFrep_cur = XFrep0
            ar_idx = 0

            for li, cfg in enumerate(CFGS):
                V, Vsp, F = cfg.V, cfg.Vsp, cfg.Fout
                BF = cfg.BF
                last = cfg.name == "c3"
                XFrep = XFrep_cur if cfg.up4 else XF_cur

                fp8sp = cfg.name in ("c2", "c3")
                sp_dt = fp8 if fp8sp else bf16
                # --- C linear (in Vsp space) ---
                XC = poolC.tile([128, cfg.nVsp * BL * F], sp_dt, tag="XC")
                gpack = max(1, 512 // cfg.GF)
                for s in range(cfg.nVsp):
                    ssz = cfg.sps(s)
                    for g0 in range(0, cfg.nG, gpack):
                        gn = min(gpack, cfg.nG - g0)
                        pc = pslin.tile([128, 512], f32, tag="lin")
                        for gi in range(gn):
                            g = g0 + gi
                            nc.tensor.matmul(
                                pc[:ssz, gi * cfg.GF:(gi + 1) * cfg.GF],
                                XF_cur[:, g * Vsp + s * 128:
                                       g * Vsp + s * 128 + ssz],
                                W_sb[f"C{cfg.name}"][:],
                                start=True, stop=True)
                        nc.scalar.activation(
                            XC[:ssz, s * BL * F + g0 * cfg.GF:
                               s * BL * F + (g0 + gn) * cfg.GF],
                            pc[:ssz, :gn * cfg.GF], AF.Copy)

                # --- inner = LU @ (2C) + B ;  y = L @ inner + A ---
                Xin = poolB.tile([128, cfg.nVt * BF], sp_dt, tag="B")
                ytile = poolC.tile([128, cfg.nVt * BF], bf16, tag="YT")
                for phase in range(2):
                    srcL = LUT[cfg.name] if phase == 0 else LTd[cfg.name]
                    nS = cfg.nVsp if phase == 0 else cfg.nVt
                    ssizes = ([cfg.sps(s) for s in range(nS)] if phase == 0
                              else [cfg.vts(s) for s in range(nS)])
                    rhs = XC if phase == 0 else Xin
                    rhs_w = BL * F if phase == 0 else BF
                    Wacc = W_sb[f"B{cfg.name}" if phase == 0 else f"A{cfg.name}"]
                    dst = Xin if phase == 0 else ytile
                    for t in range(cfg.nVt):
                        vsz = cfg.vts(t)
                        for pc0 in range(0, BF, 1024):
                            pw = min(1024, BF - pc0)
                            pi = psbig.tile([128, max(pw, 512)], f32, tag="big")
                            for nk in range(0, pw, 512):
                                n0 = pc0 + nk
                                n1 = min(n0 + 512, pc0 + pw)
                                if fp8sp:
                                    # fp8 DoubleRow: contract 2 s-tiles/pass
                                    srcr = srcL[:].rearrange(
                                        "p (s v) -> p s v", v=V)
                                    rhsr = rhs[:].rearrange(
                                        "p (s n) -> p s n", n=rhs_w)
                                    for s0 in range(0, nS, 2):
                                        if s0 + 1 < nS and \
                                                ssizes[s0 + 1] == 128:
                                            nc.tensor.matmul(
                                                pi[:vsz, n0 - pc0:n1 - pc0],
                                                srcr[:, s0:s0 + 2,
                                                     t * 128:t * 128 + vsz],
                                                rhsr[:, s0:s0 + 2, n0:n1],
                                                start=(s0 == 0), stop=False,
                                                skip_group_check=True,
                                                perf_mode=DR)
                                        else:
                                            for s in range(s0, min(s0 + 2,
                                                                   nS)):
                                                ssz = ssizes[s]
                                                nc.tensor.matmul(
                                                    pi[:vsz,
                                                       n0 - pc0:n1 - pc0],
                                                    srcL[:ssz, s * V + t * 128:
                                                         s * V + t * 128 + vsz],
                                                    rhs[:ssz, s * rhs_w + n0:
                                                        s * rhs_w + n1],
                                                    start=(s == 0), stop=False,
                                                    skip_group_check=True)
                                else:
                                    for s in range(nS):
                                        ssz = ssizes[s]
                                        nc.tensor.matmul(
                                            pi[:vsz, n0 - pc0:n1 - pc0],
                                            srcL[:ssz, s * V + t * 128:
                                                 s * V + t * 128 + vsz],
                                            rhs[:ssz, s * rhs_w + n0:
                                                s * rhs_w + n1],
                                            start=(s == 0), stop=False,
                                            skip_group_check=True)
                                for g in range(n0 // cfg.GF,
                                               (n1 + cfg.GF - 1) // cfg.GF):
                                    nc.tensor.matmul(
                                        pi[:vsz, g * cfg.GF - pc0:
                                           (g + 1) * cfg.GF - pc0],
                                        XFrep[:, g * V + t * 128:
                                              g * V + t * 128 + vsz],
                                        Wacc[:],
                                        start=False,
                                        stop=(not (last and phase == 1)),
                                        skip_group_check=True)
                                if last and phase == 1:
                                    # fold cl3 bias: += ones^T @ b3row
                                    nc.tensor.matmul(
                                        pi[:vsz, n0 - pc0:n1 - pc0],
                                        onesv[:1, :vsz],
                                        b3_sb[:1, n0:n1],
                                        start=False, stop=True,
                                        skip_group_check=True)
                            if last and phase == 1:
                                nc.vector.tensor_copy(
                                    dst[:vsz, t * BF + pc0: t * BF + pc0 + pw],
                                    pi[:vsz, :pw])
                            elif phase == 0:
                                nc.scalar.activation(
                                    dst[:vsz, t * BF + pc0: t * BF + pc0 + pw],
                                    pi[:vsz, :pw], AF.Copy)
                            else:
                                nc.vector.tensor_copy(
                                    dst[:vsz, t * BF + pc0: t * BF + pc0 + pw],
                                    pi[:vsz, :pw])

                if not last:
                    # --- back-transpose to packed F-layout; stats per group ---
                    Gp, nGp = cfg.Gp, cfg.nGp
                    nq = (nGp + 3) // 4
                    XFn = poolA.tile([128, nGp * V], bf16, tag="A")
                    dstv = XFn[:].rearrange("p (g v) -> p g v", v=V)
                    nch = cfg.nVt * nq + nGp  # worst case incl. partial tiles
                    bnst = miscp.tile([128, nch * 6], f32, tag="bnst")
                    chn = 0
                    for t in range(cfg.nVt):
                        vsz = cfg.vts(t)
                        for qi0 in range(nq):
                            q0 = qi0 * 4
                            qn = min(4, nGp - q0)
                            pt = pstr.tile([128, 512], bf16, tag="tr")
                            for qi in range(qn):
                                gp = q0 + qi
                                nc.tensor.transpose(
                                    pt[:, qi * 128: qi * 128 + vsz],
                                    ytile[:vsz, t * BF + gp * 128:
                                          t * BF + (gp + 1) * 128],
                                    ident_b[:vsz, :vsz])
                            reg = dstv[:, q0:q0 + qn, t * 128:t * 128 + vsz]
                            nc.scalar.activation(
                                reg,
                                pt[:].rearrange("p (q v) -> p q v", v=128)
                                [:, :qn, :vsz],
                                AF.Copy)
                            if vsz == 128:
                                # stats straight off the transpose PSUM tile
                                nc.vector.bn_stats(
                                    bnst[:, chn * 6:(chn + 1) * 6],
                                    pt[:, :qn * 128])
                                chn += 1
                            else:
                                for qi in range(qn):
                                    gp = q0 + qi
                                    nc.vector.bn_stats(
                                        bnst[:, chn * 6:(chn + 1) * 6],
                                        XFn[:, gp * V + t * 128:
                                            gp * V + t * 128 + vsz])
                                    chn += 1
                    aggr = miscp.tile([128, 2], f32, tag="aggr")
                    nc.vector.bn_aggr(
                        aggr[:], bnst[:, :chn * 6]
                        .rearrange("p (c s) -> p c s", s=6))
                    part = miscp.tile([128, 2], f32, tag="part")
                    if USE_RDMA and ar_idx > 0:
                        # prior layer's sends must have drained before reuse
                        nc.vector.wait_ge(lsem, 112 * ar_idx)
                    nc.vector.tensor_tensor(
                        out=part[:, 1:2], in0=aggr[:, 0:1], in1=aggr[:, 0:1],
                        op=ALU.mult)
                    nc.vector.tensor_tensor(
                        out=part[:, 1:2], in0=part[:, 1:2], in1=aggr[:, 1:2],
                        op=ALU.add)
                    if USE_RDMA:
                        nc.vector.tensor_copy(part[:, 0:1], aggr[:, 0:1])
                        rb = rbufs[ar_idx]
                        nc.vector.tensor_copy(rb[:, 0:2], part[:])
                        for k in range(1, 8):
                            nc.gpsimd.remote_dma_broadcast(
                                rb[:, 2 * k:2 * k + 2], part[:],
                                remote_sem=rsem, local_sem=lsem,
                                rdests=[(0, k) if i == k else None
                                        for i in range(8)])
                        nc.gpsimd.trigger_dma(count=None)
                        nc.vector.wait_ge(rsem, 14 * (ar_idx + 1))
                        nc.vector.tensor_tensor(
                            out=rb[:, 0:8], in0=rb[:, 0:8], in1=rb[:, 8:16],
                            op=ALU.add)
                        nc.vector.tensor_tensor(
                            out=rb[:, 0:4], in0=rb[:, 0:4], in1=rb[:, 4:8],
                            op=ALU.add)
                        nc.vector.tensor_tensor(
                            out=rb[:, 0:2], in0=rb[:, 0:2], in1=rb[:, 2:4],
                            op=ALU.add)
                        pst = pslin.tile([128, 512], f32, tag="lin")
                        nc.tensor.matmul(pst[:1, :F], rb[:, 0:1],
                                         selfd_sb[li], start=True, stop=True)
                        nc.tensor.matmul(pst[:1, F:2 * F], rb[:, 1:2],
                                         selfd_sb[li], start=True, stop=True)
                        stats_g = miscp.tile([1, 2 * F], f32, tag="statg")
                        nc.vector.tensor_copy(stats_g[:], pst[:1, :2 * F])
                    else:
                        pst = pslin.tile([128, 512], f32, tag="lin")
                        nc.tensor.matmul(pst[:1, :F], aggr[:, 0:1],
                                         selfd_sb[li], start=True, stop=True)
                        nc.tensor.matmul(pst[:1, F:2 * F], part[:, 1:2],
                                         selfd_sb[li], start=True, stop=True)
                        stats_l = miscp.tile([1, 2 * F], f32, tag="statl")
                        nc.vector.tensor_copy(stats_l[:], pst[:1, :2 * F])
                        bin_ = dramp.tile([1, 2 * F], f32, tag=f"arin{ar_idx}")
                        bout = dramp.tile([8, 2 * F], f32, tag=f"arout{ar_idx}")
                        nc.sync.dma_start(bin_[:], stats_l[:])
                        nc.gpsimd.collective_compute(
                            "AllGather", ALU.bypass,
                            replica_groups=[list(range(NCORES))],
                            ins=[bin_.opt()], outs=[bout.opt()])
                        sg8 = miscp.tile([8, 2 * F], f32, tag="sg8")
                        nc.sync.dma_start(sg8[:], bout[:])
                        psg = pslin.tile([128, 512], f32, tag="lin", name="psg")
                        nc.tensor.matmul(psg[:2 * F, 0:1], sg8[:, :2 * F],
                                         onesn[:, li:li + 1],
                                         start=True, stop=True)
                    # column form, all vars at partitions [0:F]
                    # stg col0 rows [0:F]=mu, [F:2F]=E[y^2]
                    stg = miscp.tile([128, 1], f32, tag="stg")
                    nc.vector.tensor_copy(stg[:2 * F, :], psg[:2 * F, 0:1])
                    w = miscp.tile([128, 2], f32, tag="bnw")
                    st = miscp.tile([128, 2], f32, tag="st")
                    nc.vector.tensor_copy(w[0:F, 0:1], stg[F:2 * F, :])
                    nc.vector.tensor_tensor(out=w[0:F, 1:2],
                                            in0=stg[0:F, :], in1=stg[0:F, :],
                                            op=ALU.mult)
                    nc.vector.tensor_tensor(out=w[0:F, 1:2],
                                            in0=w[0:F, 0:1], in1=w[0:F, 1:2],
                                            op=ALU.subtract)
                    nc.scalar.activation(w[0:F, 1:2], w[0:F, 1:2],
                                         AF.Sqrt, bias=epsc[0:F, :])
                    nc.vector.reciprocal(w[0:F, 1:2], w[0:F, 1:2])
                    nc.vector.tensor_tensor(out=st[0:F, 0:1],
                                            in0=w[0:F, 1:2],
                                            in1=gbc_sb[0:F, 2 * li:2 * li + 1],
                                            op=ALU.mult)
                    nc.vector.tensor_tensor(out=w[0:F, 0:1], in0=stg[0:F, :],
                                            in1=st[0:F, 0:1], op=ALU.mult)
                    nc.vector.tensor_tensor(
                        out=st[0:F, 1:2],
                        in0=gbc_sb[0:F, 2 * li + 1:2 * li + 2],
                        in1=w[0:F, 0:1], op=ALU.subtract)
                    # broadcast (s,t) to all 128 partitions via selT matmuls
                    psc = pslin.tile([128, 512], f32, tag="lin", name="psc")
                    nc.tensor.matmul(psc[:, 0:2], sT_sb[F][:F, 0:128],
                                     st[:F, 0:2], start=True, stop=True)
                    stc = miscp.tile([128, 2], f32, tag=f"stc{ar_idx}")
                    nc.vector.tensor_copy(stc[:], psc[:, 0:2])
                    ar_idx += 1
                    # chunked scale+relu (+ up4 replication for next layer)
                    ncfg = CFGS[li + 1]
                    if ncfg.up4:
                        XFrep_cur = poolA.tile(
                            [128, ncfg.nG * ncfg.V], bf16, tag="A")
                        s_r = XFn[:].rearrange("p (g w) -> p g w", w=V)
                        d_r = XFrep_cur[:].rearrange(
                            "p (g w r) -> p g w r", w=V, r=4)
                    nap = max(1, nGp // 4)
                    bnds = [0, 1] + list(range(1 + nap, nGp, nap)) + [nGp]
                    bnds = sorted(set(b for b in bnds if b <= nGp))
                    for q0, q1 in zip(bnds[:-1], bnds[1:]):
                        c0_, c1_ = q0 * V, q1 * V
                        cm = c0_ + (c1_ - c0_) * 5 // 9  # Act a bit slower
                        nc.scalar.activation(
                            XFn[:, c0_:cm], XFn[:, c0_:cm],
                            AF.Relu, scale=stc[:, 0:1], bias=stc[:, 1:2])
                        nc.vector.tensor_scalar(
                            out=XFn[:, cm:c1_], in0=XFn[:, cm:c1_],
                            scalar1=stc[:, 0:1], scalar2=stc[:, 1:2],
                            op0=ALU.mult, op1=ALU.add)
                        nc.vector.tensor_scalar_max(
                            XFn[:, cm:c1_], XFn[:, cm:c1_], 0.0)
                        if ncfg.up4:
                            for r in range(4):
                                nc.vector.tensor_copy(
                                    d_r[:, q0:q1, :, r], s_r[:, q0:q1, :])
                    XF_cur = XFn
                else:
                    # --- output: ship ytile [v-part, (b,c)] as-is; host
                    # untangles the (p, t, b, c) layout in numpy ---
                    for t0, t1 in ((0, 4), (4, 8), (8, 10)):
                        nc.sync.dma_start(
                            ydram[:, t0 * BF:t1 * BF],
                            ytile[:, t0 * BF:t1 * BF])

    nc.compile()
    return nc


def kernel(**inputs):
    import sys
    for p in ("/opt/trn_rl_repo", "/opt/trn_rl_repo/concourse"):
        if p not in sys.path:
            sys.path.insert(0, p)
    from concourse.bass_utils import run_bass_kernel_spmd

    host = _build_host(inputs)

    if "nc" not in _CACHE:
        _CACHE["nc"] = _build_nc()
    nc = _CACHE["nc"]

    in_maps = []
    for c in range(NCORES):
        m = {k: v for k, v in host.items() if k != "xT"}
        m["xT"] = np.ascontiguousarray(host["xT"][:, c * BL:(c + 1) * BL])
        in_maps.append(m)
    res = run_bass_kernel_spmd(nc, in_maps, core_ids=list(range(NCORES)))
    # y[p, t*96 + b*3 + c] -> out[b, t*128+p, c]
    outs = []
    for r in res.results:
        y = np.asarray(r["y"], np.float32).reshape(128, 10, BL, 3)
        outs.append(y.transpose(2, 1, 0, 3).reshape(BL, 1280, 3))
    return np.concatenate(outs, axis=0).astype(np.float32)


if __name__ == "__main__":
    import reference as R
    inp = R.setup_inputs()
    inp = {k: np.asarray(v) for k, v in inp.items()}
    act = kernel(**inp)
    exp = np.asarray(R.reference(**inp))
    err = np.linalg.norm(act - exp) / np.linalg.norm(exp)
    print("Relative error:", err)


# revision 54
# speedup vs baseline: 1.6679x; 1.0068x over previous
"""Trainium2 Bass kernel for nn_Graph_CNN_Feat_Mesh (Chebyshev GNN decoder).

Strategy (per-core, data-parallel over batch B=256 -> 32/core):
  - All spmms are dense matmuls on the tensor engine (PE) in bf16:
      y = A + L @ (B + L @ (2C)),  A/B/C = feature-space linears of the input.
    L is densified on host; for up4-preceded layers the replication is folded
    into LU = L @ U (contracting the small pre-upsample vertex space).
  - B and A linear terms accumulate directly into the spmm PSUM.
  - Activations live in packed F-layout [(j,Fin) partitions, (b//G)*Vsp + v]
    between layers; the per-layer linear emits V-layout directly; one PE
    transpose per layer returns to F-layout.
  - BatchNorm (training mode, global batch stats) is exact: per-core partial
    sums are AllGather'd across the 8 cores in-kernel (cheaper than
    AllReduce) and summed locally with a K=8 ones-matmul; stats are taken
    per transpose-group so they finish with the last transpose; scale+relu
    is chunked so the next layer starts on early chunks.
  - Weights are host-pre-tiled into [128, *] monoliths and streamed with a
    handful of large DMAs on the gpsimd queue (25ns issue) in consumption
    order; the FC head runs in bf16 with fp32 PSUM.
"""

import numpy as np

B = 256
NCORES = 8
BL = B // NCORES  # 32
EPS = 1e-5
USE_RDMA = False  # remote-DMA BN exchange: unsupported by the timing sim

_CACHE = {}


def _split_W(W):
    W = np.asarray(W, np.float32)
    return W[:, 0::3], W[:, 1::3], W[:, 2::3]


def _dense_L(rows, cols, vals, V):
    L = np.zeros((V, V), np.float32)
    np.add.at(L, (np.asarray(rows), np.asarray(cols)), np.asarray(vals, np.float32))
    return L


def _tile128(a):
    """[S*128, N] -> [128, S*N] block-column layout (pad rows to mult of 128)."""
    a = np.asarray(a)
    S = (a.shape[0] + 127) // 128
    if a.shape[0] != S * 128:
        a = np.concatenate(
            [a, np.zeros((S * 128 - a.shape[0], a.shape[1]), a.dtype)], 0)
    return np.ascontiguousarray(
        a.reshape(S, 128, a.shape[1]).transpose(1, 0, 2).reshape(128, -1))


class _LCfg:
    def __init__(self, name, Vsp, V, Fin, Fout, up4, bn):
        self.name = name
        self.Vsp = Vsp      # source vertex space of C-linear (pre-up4)
        self.V = V          # output vertex count
        self.Fin = Fin
        self.Fout = Fout
        self.G = 128 // Fin          # batches packed on partitions at input
        self.nG = BL // self.G
        self.GF = self.G * Fout      # N of one B/C/A-linear matmul
        self.Gp = 128 // Fout if Fout in (32, 64) else None
        self.nGp = BL // self.Gp if self.Gp else None
        self.up4 = up4
        self.bn = bn
        self.nVt = (V + 127) // 128
        self.nVsp = (Vsp + 127) // 128
        self.BF = BL * Fout          # free width of V-layout per vtile

    def vts(self, t):
        return min(128, self.V - t * 128)

    def sps(self, s):
        return min(128, self.Vsp - s * 128)


CFGS = [
    _LCfg("c0", 80, 320, 64, 64, True, True),
    _LCfg("c1", 320, 320, 64, 32, False, True),
    _LCfg("c2", 320, 1280, 32, 32, True, True),
    _LCfg("c3", 1280, 1280, 32, 3, False, False),
]


def _wbd(W, G, Fin, Fout, which):
    """Block-diagonal rhs weight [128, G*Fout] for the fused linear.
    which: 'A' -> W0 - W2, 'B' -> W1, 'C' -> 2*W2.  col = j*Fout + c."""
    W0, W1, W2 = _split_W(W)
    M = {"A": W0 - W2, "B": W1, "C": 2.0 * W2}[which]  # [Fout, Fin]
    out = np.zeros((128, G * Fout), np.float32)
    for j in range(G):
        out[j * Fin:(j + 1) * Fin, j * Fout:(j + 1) * Fout] = M.T
    return out


# column offsets inside the packed weight blobs
_WOFF = {}
_off = 0
for _cfg in CFGS:
    for _w in "ABC":
        _WOFF[f"{_w}{_cfg.name}"] = (_off, _cfg.GF)
        _off += _cfg.GF
WPACK_N = _off  # bf16 pack cols

# f32 pack: fc1b | per-layer FD-scaled sel blocks (BN partial-sum reduce)
F32_FC1B = 0
F32_SEL = [4, 68, 100]   # selFD for bn layers 0,1,2 (widths 64,32,32)
F32PACK_N = 132
_BN_F = [64, 32, 32]
_BN_FD = [16 * 320, 8 * 320, 8 * 1280]
_BN_NG = [256 * 320, 256 * 320, 256 * 1280]
# with equal per-core/per-group counts, global mu = sum of partition means
# scaled by FD/NG; same factor turns summed (mean^2+var) into E[y^2]
_BN_SCL = [fd / ng for fd, ng in zip(_BN_FD, _BN_NG)]


def _build_host(inputs):
    import ml_dtypes
    bf = ml_dtypes.bfloat16
    f32 = np.float32
    d = {}
    d["xT"] = np.ascontiguousarray(np.asarray(inputs["x"], f32).T).astype(bf)
    d["fc1wt"] = _tile128(
        np.asarray(inputs["fc1_w"], f32).T).astype(bf)           # [128, 16*512]
    d["fc1b"] = np.ascontiguousarray(
        np.asarray(inputs["fc1_b"], f32).reshape(4, 128).T)      # [128,4]
    # fc2: chunk-major (mc), then k-tile: [128, 16*1280]
    w2 = np.asarray(inputs["fc2_w"], f32).T                      # [512, 5120]
    blk = [w2[kt * 128:(kt + 1) * 128, mc * 1280:(mc + 1) * 1280]
           for mc in range(4) for kt in range(4)]
    d["fc2wt"] = np.ascontiguousarray(np.concatenate(blk, 1)).astype(bf)

    L1 = _dense_L(inputs["L1_rows"], inputs["L1_cols"], inputs["L1_vals"], 320)
    L2 = _dense_L(inputs["L2_rows"], inputs["L2_cols"], inputs["L2_vals"], 1280)
    U1 = np.repeat(np.eye(80, dtype=f32), 4, axis=0)    # [320, 80]
    U2 = np.repeat(np.eye(320, dtype=f32), 4, axis=0)   # [1280, 320]
    f8 = ml_dtypes.float8_e4m3
    d["LU0"] = _tile128((L1 @ U1).T).astype(bf)         # [128, 320]
    d["LT1"] = _tile128(L1.T).astype(bf)                # [128, 3*320]
    d["LT1f8"] = _tile128(L1.T).astype(f8)              # fp8 copy for c1
    d["LU2"] = _tile128((L2 @ U2).T).astype(f8)         # [128, 3*1280] fp8
    d["LT2"] = _tile128(L2.T).astype(f8)                # [128, 10*1280] fp8

    Wn = {"c0": "cl0_w", "c1": "cl1_w", "c2": "cl2_w", "c3": "cl3_w"}
    wall = np.zeros((128, WPACK_N), f32)
    for cfg in CFGS:
        W = np.asarray(inputs[Wn[cfg.name]], f32)
        for which in "ABC":
            o, n = _WOFF[f"{which}{cfg.name}"]
            wall[:, o:o + n] = _wbd(W, cfg.G, cfg.Fin, cfg.Fout, which)
    d["wall"] = wall.astype(bf)
    # b3 tiled over the (b, c) column layout of the last-layer PSUM: col = b*3+c
    d["b3row"] = np.ascontiguousarray(
        np.tile(np.asarray(inputs["cl3_b"], f32), BL)[None, :]).astype(bf)

    gbc = np.zeros((128, 6), f32)
    for li, (g, b) in enumerate([("bn0_g", "bn0_b"), ("bn1_g", "bn1_b"),
                                 ("bn2_g", "bn2_b")]):
        F = _BN_F[li]
        gbc[0:F, 2 * li] = np.asarray(inputs[g], f32)
        gbc[0:F, 2 * li + 1] = np.asarray(inputs[b], f32)
    d["gbcol"] = gbc

    f32p = np.zeros((128, F32PACK_N), f32)
    for li in range(3):
        F, o = _BN_F[li], F32_SEL[li]
        v = _BN_SCL[li] if USE_RDMA else _BN_FD[li]
        for j in range(128 // F):
            f32p[j * F:(j + 1) * F, o:o + F] += v * np.eye(F, dtype=f32)
    f32p[:, F32_FC1B:F32_FC1B + 4] = d.pop("fc1b")
    d["f32pack"] = f32p
    # selT_s/selT_t [2F rows, 128]: stc[p,:] = (s[p%F], t[p%F]) via 2 matmuls
    stp = np.zeros((128, 2 * 128), f32)
    for F, ro in [(64, 0), (32, 0)]:
        pass
    sT = np.zeros((128, 256), f32)   # rows k (2F<=128), cols: [0:128]=s-map, [128:256]=t-map
    # build per-F maps stacked by row-offset: F=64 uses rows 0:128, F=32 uses rows 0:64
    sT64 = np.zeros((128, 128), f32)
    sT32 = np.zeros((128, 128), f32)
    for p in range(128):
        sT64[p % 64, p] = 1.0
        sT32[p % 32, p] = 1.0
    d["selT64"] = sT64
    d["selT32"] = sT32
    return d


def _build_nc():
    import sys
    for p in ("/opt/trn_rl_repo", "/opt/trn_rl_repo/concourse"):
        if p not in sys.path:
            sys.path.insert(0, p)
    import concourse.bass as bass  # noqa
    import concourse.mybir as mybir
    import concourse.tile as tile
    from concourse import bacc
    from concourse.masks import make_identity

    f32 = mybir.dt.float32
    bf16 = mybir.dt.bfloat16
    fp8 = mybir.dt.float8e4
    DR = mybir.MatmulPerfMode.DoubleRow
    AF = mybir.ActivationFunctionType
    ALU = mybir.AluOpType

    nc = bacc.Bacc(None, target_bir_lowering=False)

    xT = nc.dram_tensor("xT", [2048, BL], bf16, kind="ExternalInput")
    fc1wt = nc.dram_tensor("fc1wt", [128, 16 * 512], bf16, kind="ExternalInput")
    fc2wt = nc.dram_tensor("fc2wt", [128, 16 * 1280], bf16, kind="ExternalInput")
    LU0 = nc.dram_tensor("LU0", [128, 320], bf16, kind="ExternalInput")
    LT1 = nc.dram_tensor("LT1", [128, 3 * 320], bf16, kind="ExternalInput")
    LT1f8 = nc.dram_tensor("LT1f8", [128, 3 * 320], fp8, kind="ExternalInput")
    LU2 = nc.dram_tensor("LU2", [128, 3 * 1280], fp8, kind="ExternalInput")
    LT2 = nc.dram_tensor("LT2", [128, 10 * 1280], fp8, kind="ExternalInput")
    wall = nc.dram_tensor("wall", [128, WPACK_N], bf16, kind="ExternalInput")
    gbcol = nc.dram_tensor("gbcol", [128, 6], f32, kind="ExternalInput")
    f32pack = nc.dram_tensor("f32pack", [128, F32PACK_N], f32, kind="ExternalInput")
    selT64 = nc.dram_tensor("selT64", [128, 128], f32, kind="ExternalInput")
    selT32 = nc.dram_tensor("selT32", [128, 128], f32, kind="ExternalInput")
    b3row = nc.dram_tensor("b3row", [1, 96], bf16, kind="ExternalInput")
    ydram = nc.dram_tensor("y", [128, 960], bf16, kind="ExternalOutput")

    with tile.TileContext(nc) as tc:
        with (
            tc.tile_pool(name="const", bufs=1) as constp,
            tc.tile_pool(name="wpool", bufs=1) as wpool,
            tc.tile_pool(name="poolA", bufs=2) as poolA,
            tc.tile_pool(name="poolB", bufs=2) as poolB,
            tc.tile_pool(name="poolC", bufs=1) as poolC,
            tc.tile_pool(name="misc", bufs=1) as miscp,
            tc.tile_pool(name="outp", bufs=3) as outp,
            tc.tile_pool(name="pslin", bufs=2, space="PSUM") as pslin,
            tc.tile_pool(name="psbig", bufs=2, space="PSUM") as psbig,
            tc.tile_pool(name="pstr", bufs=2, space="PSUM") as pstr,
            tc.tile_pool(name="dram", bufs=1, space="DRAM") as dramp,
        ):
            # ---- fc1 inputs first: these DMAs gate the first matmul ----
            fc1w_sb = poolA.tile([128, 16 * 512], bf16, tag="A")
            nc.gpsimd.dma_start(fc1w_sb[:, :4 * 512], fc1wt[:, :4 * 512])
            xT_sb = miscp.tile([128, 16 * BL], bf16, tag="xT")
            nc.gpsimd.dma_start(
                xT_sb[:].rearrange("p (k b) -> p k b", b=BL),
                xT[:].rearrange("(k p) b -> p k b", p=128))
            for kc in range(1, 4):
                nc.gpsimd.dma_start(
                    fc1w_sb[:, kc * 4 * 512:(kc + 1) * 4 * 512],
                    fc1wt[:, kc * 4 * 512:(kc + 1) * 4 * 512])
            f32_sb = constp.tile([128, F32PACK_N], f32, tag="f32pack")
            nc.sync.dma_start(f32_sb[:], f32pack[:])
            selfd_sb = [f32_sb[:, F32_SEL[li]:F32_SEL[li] + _BN_F[li]]
                        for li in range(3)]
            fc1b_sb = f32_sb[:, F32_FC1B:F32_FC1B + 4]

            # ---- small constants (no DMA) ----
            ident_b = constp.tile([128, 128], bf16, tag="identb")
            make_identity(nc, ident_b[:])
            ident_f = constp.tile([1, 1], f32, tag="identf")
            nc.gpsimd.memset(ident_f[:], 1.0)
            eps_t = constp.tile([1, 1], f32, tag="eps")
            nc.gpsimd.memset(eps_t[:], EPS)
            onesn = constp.tile([8, 3], f32, tag="onesn")
            for li in range(3):
                nc.gpsimd.memset(onesn[:, li:li + 1], 1.0 / _BN_NG[li])
            onesv = constp.tile([1, 128], bf16, tag="onesv")
            nc.gpsimd.memset(onesv[:], 1.0)
            sq_warm = constp.tile([1, 1], f32, tag="sqwarm")
            nc.scalar.activation(sq_warm[:], eps_t[:], AF.Sqrt, bias=eps_t[:])

            # ================= FC head (bf16, fp32 psum) =================
            # kt-outer so matmuls start as soon as the first fc1w chunk lands
            h1T = miscp.tile([128, 4 * BL], bf16, tag="h1T")
            ps1 = pslin.tile([128, 4 * BL], f32, tag="lin")
            for mt in range(4):
                for kt in range(16):
                    nc.tensor.matmul(
                        ps1[:, mt * BL:(mt + 1) * BL],
                        fc1w_sb[:, kt * 512 + mt * 128: kt * 512 + (mt + 1) * 128],
                        xT_sb[:, kt * BL:(kt + 1) * BL],
                        start=(kt == 0), stop=(kt == 15))
                nc.scalar.activation(
                    h1T[:, mt * BL:(mt + 1) * BL], ps1[:, mt * BL:(mt + 1) * BL],
                    AF.Relu, bias=fc1b_sb[:, mt:mt + 1])

            # ---- mid-priority loads (small; needed for c0/c1 + BN) ----
            gbc_sb = constp.tile([128, 6], f32, tag="gbc")
            nc.sync.dma_start(gbc_sb[:], gbcol[:])
            epsc = constp.tile([128, 1], f32, tag="epsc")
            nc.gpsimd.memset(epsc[:], EPS)
            sT_sb = {64: constp.tile([128, 128], f32, tag="sT64", name="sT64sb"),
                     32: constp.tile([128, 128], f32, tag="sT32", name="sT32sb")}
            nc.sync.dma_start(sT_sb[64][:], selT64[:])
            nc.sync.dma_start(sT_sb[32][:], selT32[:])
            b3_sb = constp.tile([1, 96], bf16, tag="b3row")
            nc.sync.dma_start(b3_sb[:], b3row[:])

            LUT, LTd = {}, {}
            t = wpool.tile([128, 320], bf16, tag="LU0")
            nc.sync.dma_start(t[:], LU0[:])
            LUT["c0"] = t
            t = wpool.tile([128, 3 * 320], bf16, tag="LT1")
            nc.sync.dma_start(t[:], LT1[:])
            LTd["c0"] = t
            t = wpool.tile([128, 3 * 320], fp8, tag="LT1f8")
            nc.sync.dma_start(t[:], LT1f8[:])
            LTd["c1"] = LUT["c1"] = t
            wall_sb = wpool.tile([128, WPACK_N], bf16, tag="wall")
            nc.sync.dma_start(wall_sb[:], wall[:])
            W_sb = {}
            for cfg in CFGS:
                for w in "ABC":
                    o, n = _WOFF[f"{w}{cfg.name}"]
                    W_sb[f"{w}{cfg.name}"] = wall_sb[:, o:o + n]

            # ================= fc2 (streamed in 4 column-chunks) =========
            # psum partition = (v0%2)*64+f, col = mi*BL+b ; channels c = v0*64+f.
            # dest: XF0[(b%2)*64+f, (b//2)*80 + v0],  v0 = 2*(mc*10+mi)+p0
            XF0 = poolC.tile([128, 16 * 80], bf16, tag="XF0")
            cfg0 = CFGS[0]
            XFrep0 = poolA.tile([128, cfg0.nG * cfg0.V], bf16, tag="A")
            s_r0 = XF0[:].rearrange("p (g w) -> p g w", w=80)
            d_r0 = XFrep0[:].rearrange("p (g w r) -> p g w r", w=80, r=4)
            for mc in range(4):
                wch = poolB.tile([128, 4 * 1280], bf16, tag="B")
                nc.gpsimd.dma_start(
                    wch[:], fc2wt[:, mc * 4 * 1280:(mc + 1) * 4 * 1280])
                ps2 = psbig.tile([128, 10 * BL], f32, tag="big")
                for mi in range(10):
                    for kt in range(4):
                        nc.tensor.matmul(
                            ps2[:, mi * BL:(mi + 1) * BL],
                            wch[:, kt * 1280 + mi * 128: kt * 1280 + (mi + 1) * 128],
                            h1T[:, kt * BL:(kt + 1) * BL],
                            start=(kt == 0), stop=(kt == 3))
                src4 = ps2[:].rearrange("p (i g j) -> p i g j", g=16, j=2)
                dst4 = XF0[:].rearrange("p (g u q) -> p g u q", u=40, q=2)
                for p0 in range(2):
                    for j in range(2):
                        nc.scalar.activation(
                            dst4[j * 64:(j + 1) * 64, :,
                                 mc * 10:(mc + 1) * 10, p0]
                            .rearrange("p g i -> p i g"),
                            src4[p0 * 64:(p0 + 1) * 64, :, :, j],
                            AF.Copy)
                # up4-replicate this chunk's w-range (w = v0 in [20mc, 20mc+20))
                for r in range(4):
                    nc.vector.tensor_copy(
                        d_r0[:, :, 20 * mc:20 * (mc + 1), r],
                        s_r0[:, :, 20 * mc:20 * (mc + 1)])

            # ---- big late loads (needed at c2; stream during c0/c1) ----
            t = wpool.tile([128, 3 * 1280], fp8, tag="LU2")
            nc.gpsimd.dma_start(t[:], LU2[:])
            LUT["c2"] = t
            t = wpool.tile([128, 10 * 1280], fp8, tag="LT2")
            nc.gpsimd.dma_start(t[:, :5 * 1280], LT2[:, :5 * 1280])
            nc.gpsimd.dma_start(t[:, 5 * 1280:], LT2[:, 5 * 1280:])
            LTd["c2"] = LTd["c3"] = LUT["c3"] = t

            # ================= cheby layers =================
            if USE_RDMA:
                rsem = nc.alloc_semaphore("bn_rsem")
                lsem = nc.alloc_semaphore("bn_lsem")
                rbufs = [constp.tile([128, 16], f32, tag=f"rbuf{i}",
                                     name=f"rbuf{i}")
                         for i in range(3)]
            XF_cur = XF0
            XFrep_cur = XFrep0
            ar_idx = 0

            for li, cfg in enumerate(CFGS):
                V, Vsp, F = cfg.V, cfg.Vsp, cfg.Fout
                BF = cfg.BF
                last = cfg.name == "c3"
                XFrep = XFrep_cur if cfg.up4 else XF_cur

                fp8sp = cfg.name in ("c1", "c2", "c3")
                sp_dt = fp8 if fp8sp else bf16
                # --- C linear (in Vsp space) ---
                XC = poolC.tile([128, cfg.nVsp * BL * F], sp_dt, tag="XC")
                gpack = max(1, 512 // cfg.GF)
                for s in range(cfg.nVsp):
                    ssz = cfg.sps(s)
                    for g0 in range(0, cfg.nG, gpack):
                        gn = min(gpack, cfg.nG - g0)
                        pc = pslin.tile([128, 512], f32, tag="lin")
                        for gi in range(gn):
                            g = g0 + gi
                            nc.tensor.matmul(
                                pc[:ssz, gi * cfg.GF:(gi + 1) * cfg.GF],
                                XF_cur[:, g * Vsp + s * 128:
                                       g * Vsp + s * 128 + ssz],
                                W_sb[f"C{cfg.name}"][:],
                                start=True, stop=True)
                        nc.scalar.activation(
                            XC[:ssz, s * BL * F + g0 * cfg.GF:
                               s * BL * F + (g0 + gn) * cfg.GF],
                            pc[:ssz, :gn * cfg.GF], AF.Copy)

                # --- inner = LU @ (2C) + B ;  y = L @ inner + A ---
                Xin = poolB.tile([128, cfg.nVt * BF], sp_dt, tag="B")
                ytile = poolC.tile([128, cfg.nVt * BF], bf16, tag="YT")
                for phase in range(2):
                    srcL = LUT[cfg.name] if phase == 0 else LTd[cfg.name]
                    nS = cfg.nVsp if phase == 0 else cfg.nVt
                    ssizes = ([cfg.sps(s) for s in range(nS)] if phase == 0
                              else [cfg.vts(s) for s in range(nS)])
                    rhs = XC if phase == 0 else Xin
                    rhs_w = BL * F if phase == 0 else BF
                    Wacc = W_sb[f"B{cfg.name}" if phase == 0 else f"A{cfg.name}"]
                    dst = Xin if phase == 0 else ytile
                    for t in range(cfg.nVt):
                        vsz = cfg.vts(t)
                        for pc0 in range(0, BF, 1024):
                            pw = min(1024, BF - pc0)
                            pi = psbig.tile([128, max(pw, 512)], f32, tag="big")
                            for nk in range(0, pw, 512):
                                n0 = pc0 + nk
                                n1 = min(n0 + 512, pc0 + pw)
                                if fp8sp:
                                    # fp8 DoubleRow: contract 2 s-tiles/pass
                                    srcr = srcL[:].rearrange(
                                        "p (s v) -> p s v", v=V)
                                    rhsr = rhs[:].rearrange(
                                        "p (s n) -> p s n", n=rhs_w)
                                    for s0 in range(0, nS, 2):
                                        if s0 + 1 < nS and \
                                                ssizes[s0 + 1] == 128:
                                            nc.tensor.matmul(
                                                pi[:vsz, n0 - pc0:n1 - pc0],
                                                srcr[:, s0:s0 + 2,
                                                     t * 128:t * 128 + vsz],
                                                rhsr[:, s0:s0 + 2, n0:n1],
                                                start=(s0 == 0), stop=False,
                                                skip_group_check=True,
                                                perf_mode=DR)
                                        else:
                                            for s in range(s0, min(s0 + 2,
                                                                   nS)):
                                                ssz = ssizes[s]
                                                nc.tensor.matmul(
                                                    pi[:vsz,
                                                       n0 - pc0:n1 - pc0],
                                                    srcL[:ssz, s * V + t * 128:
                                                         s * V + t * 128 + vsz],
                                                    rhs[:ssz, s * rhs_w + n0:
                                                        s * rhs_w + n1],
                                                    start=(s == 0), stop=False,
                                                    skip_group_check=True)
                                else:
                                    for s in range(nS):
                                        ssz = ssizes[s]
                                        nc.tensor.matmul(
                                            pi[:vsz, n0 - pc0:n1 - pc0],
                                            srcL[:ssz, s * V + t * 128:
                                                 s * V + t * 128 + vsz],
                                            rhs[:ssz, s * rhs_w + n0:
                                                s * rhs_w + n1],
                                            start=(s == 0), stop=False,
                                            skip_group_check=True)
                                for g in range(n0 // cfg.GF,
                                               (n1 + cfg.GF - 1) // cfg.GF):
                                    nc.tensor.matmul(
                                        pi[:vsz, g * cfg.GF - pc0:
                                           (g + 1) * cfg.GF - pc0],
                                        XFrep[:, g * V + t * 128:
                                              g * V + t * 128 + vsz],
                                        Wacc[:],
                                        start=False,
                                        stop=(not (last and phase == 1)),
                                        skip_group_check=True)
                                if last and phase == 1:
                                    # fold cl3 bias: += ones^T @ b3row
                                    nc.tensor.matmul(
                                        pi[:vsz, n0 - pc0:n1 - pc0],
                                        onesv[:1, :vsz],
                                        b3_sb[:1, n0:n1],
                                        start=False, stop=True,
                                        skip_group_check=True)
                            if last and phase == 1:
                                nc.vector.tensor_copy(
                                    dst[:vsz, t * BF + pc0: t * BF + pc0 + pw],
                                    pi[:vsz, :pw])
                            elif phase == 0:
                                nc.scalar.activation(
                                    dst[:vsz, t * BF + pc0: t * BF + pc0 + pw],
                                    pi[:vsz, :pw], AF.Copy)
                            else:
                                nc.vector.tensor_copy(
                                    dst[:vsz, t * BF + pc0: t * BF + pc0 + pw],
                                    pi[:vsz, :pw])

                if not last:
                    # --- back-transpose to packed F-layout; stats per group ---
                    Gp, nGp = cfg.Gp, cfg.nGp
                    nq = (nGp + 3) // 4
                    XFn = poolA.tile([128, nGp * V], bf16, tag="A")
                    dstv = XFn[:].rearrange("p (g v) -> p g v", v=V)
                    nch = cfg.nVt * nq + nGp  # worst case incl. partial tiles
                    bnst = miscp.tile([128, nch * 6], f32, tag="bnst")
                    chn = 0
                    for t in range(cfg.nVt):
                        vsz = cfg.vts(t)
                        for qi0 in range(nq):
                            q0 = qi0 * 4
                            qn = min(4, nGp - q0)
                            pt = pstr.tile([128, 512], bf16, tag="tr")
                            for qi in range(qn):
                                gp = q0 + qi
                                nc.tensor.transpose(
                                    pt[:, qi * 128: qi * 128 + vsz],
                                    ytile[:vsz, t * BF + gp * 128:
                                          t * BF + (gp + 1) * 128],
                                    ident_b[:vsz, :vsz])
                            reg = dstv[:, q0:q0 + qn, t * 128:t * 128 + vsz]
                            nc.scalar.activation(
                                reg,
                                pt[:].rearrange("p (q v) -> p q v", v=128)
                                [:, :qn, :vsz],
                                AF.Copy)
                            if vsz == 128:
                                # stats straight off the transpose PSUM tile
                                nc.vector.bn_stats(
                                    bnst[:, chn * 6:(chn + 1) * 6],
                                    pt[:, :qn * 128])
                                chn += 1
                            else:
                                for qi in range(qn):
                                    gp = q0 + qi
                                    nc.vector.bn_stats(
                                        bnst[:, chn * 6:(chn + 1) * 6],
                                        XFn[:, gp * V + t * 128:
                                            gp * V + t * 128 + vsz])
                                    chn += 1
                    aggr = miscp.tile([128, 2], f32, tag="aggr")
                    nc.vector.bn_aggr(
                        aggr[:], bnst[:, :chn * 6]
                        .rearrange("p (c s) -> p c s", s=6))
                    part = miscp.tile([128, 2], f32, tag="part")
                    if USE_RDMA and ar_idx > 0:
                        # prior layer's sends must have drained before reuse
                        nc.vector.wait_ge(lsem, 112 * ar_idx)
                    nc.vector.tensor_tensor(
                        out=part[:, 1:2], in0=aggr[:, 0:1], in1=aggr[:, 0:1],
                        op=ALU.mult)
                    nc.vector.tensor_tensor(
                        out=part[:, 1:2], in0=part[:, 1:2], in1=aggr[:, 1:2],
                        op=ALU.add)
                    if USE_RDMA:
                        nc.vector.tensor_copy(part[:, 0:1], aggr[:, 0:1])
                        rb = rbufs[ar_idx]
                        nc.vector.tensor_copy(rb[:, 0:2], part[:])
                        for k in range(1, 8):
                            nc.gpsimd.remote_dma_broadcast(
                                rb[:, 2 * k:2 * k + 2], part[:],
                                remote_sem=rsem, local_sem=lsem,
                                rdests=[(0, k) if i == k else None
                                        for i in range(8)])
                        nc.gpsimd.trigger_dma(count=None)
                        nc.vector.wait_ge(rsem, 14 * (ar_idx + 1))
                        nc.vector.tensor_tensor(
                            out=rb[:, 0:8], in0=rb[:, 0:8], in1=rb[:, 8:16],
                            op=ALU.add)
                        nc.vector.tensor_tensor(
                            out=rb[:, 0:4], in0=rb[:, 0:4], in1=rb[:, 4:8],
                            op=ALU.add)
                        nc.vector.tensor_tensor(
                            out=rb[:, 0:2], in0=rb[:, 0:2], in1=rb[:, 2:4],
                            op=ALU.add)
                        pst = pslin.tile([128, 512], f32, tag="lin")
                        nc.tensor.matmul(pst[:1, :F], rb[:, 0:1],
                                         selfd_sb[li], start=True, stop=True)
                        nc.tensor.matmul(pst[:1, F:2 * F], rb[:, 1:2],
                                         selfd_sb[li], start=True, stop=True)
                        stats_g = miscp.tile([1, 2 * F], f32, tag="statg")
                        nc.vector.tensor_copy(stats_g[:], pst[:1, :2 * F])
                    else:
                        pst = pslin.tile([128, 512], f32, tag="lin")
                        nc.tensor.matmul(pst[:1, :F], aggr[:, 0:1],
                                         selfd_sb[li], start=True, stop=True)
                        nc.tensor.matmul(pst[:1, F:2 * F], part[:, 1:2],
                                         selfd_sb[li], start=True, stop=True)
                        stats_l = miscp.tile([1, 2 * F], f32, tag="statl")
                        nc.vector.tensor_copy(stats_l[:], pst[:1, :2 * F])
                        bin_ = dramp.tile([1, 2 * F], f32, tag=f"arin{ar_idx}")
                        bout = dramp.tile([8, 2 * F], f32, tag=f"arout{ar_idx}")
                        nc.sync.dma_start(bin_[:], stats_l[:])
                        nc.gpsimd.collective_compute(
                            "AllGather", ALU.bypass,
                            replica_groups=[list(range(NCORES))],
                            ins=[bin_.opt()], outs=[bout.opt()])
                        sg8 = miscp.tile([8, 2 * F], f32, tag="sg8")
                        nc.sync.dma_start(sg8[:], bout[:])
                        psg = pslin.tile([128, 512], f32, tag="lin", name="psg")
                        nc.tensor.matmul(psg[:2 * F, 0:1], sg8[:, :2 * F],
                                         onesn[:, li:li + 1],
                                         start=True, stop=True)
                    # column form, all vars at partitions [0:F]
                    # stg col0 rows [0:F]=mu, [F:2F]=E[y^2]
                    stg = miscp.tile([128, 1], f32, tag="stg")
                    nc.vector.tensor_copy(stg[:2 * F, :], psg[:2 * F, 0:1])
                    w = miscp.tile([128, 2], f32, tag="bnw")
                    st = miscp.tile([128, 2], f32, tag="st")
                    nc.vector.tensor_copy(w[0:F, 0:1], stg[F:2 * F, :])
                    nc.vector.tensor_tensor(out=w[0:F, 1:2],
                                            in0=stg[0:F, :], in1=stg[0:F, :],
                                            op=ALU.mult)
                    nc.vector.tensor_tensor(out=w[0:F, 1:2],
                                            in0=w[0:F, 0:1], in1=w[0:F, 1:2],
                                            op=ALU.subtract)
                    nc.scalar.activation(w[0:F, 1:2], w[0:F, 1:2],
                                         AF.Sqrt, bias=epsc[0:F, :])
                    nc.vector.reciprocal(w[0:F, 1:2], w[0:F, 1:2])
                    nc.vector.tensor_tensor(out=st[0:F, 0:1],
                                            in0=w[0:F, 1:2],
                                            in1=gbc_sb[0:F, 2 * li:2 * li + 1],
                                            op=ALU.mult)
                    nc.vector.tensor_tensor(out=w[0:F, 0:1], in0=stg[0:F, :],
                                            in1=st[0:F, 0:1], op=ALU.mult)
                    nc.vector.tensor_tensor(
                        out=st[0:F, 1:2],
                        in0=gbc_sb[0:F, 2 * li + 1:2 * li + 2],
                        in1=w[0:F, 0:1], op=ALU.subtract)
                    # broadcast (s,t) to all 128 partitions via selT matmuls
                    psc = pslin.tile([128, 512], f32, tag="lin", name="psc")
                    nc.tensor.matmul(psc[:, 0:2], sT_sb[F][:F, 0:128],
                                     st[:F, 0:2], start=True, stop=True)
                    stc = miscp.tile([128, 2], f32, tag=f"stc{ar_idx}")
                    nc.vector.tensor_copy(stc[:], psc[:, 0:2])
                    ar_idx += 1
                    # chunked scale+relu (+ up4 replication for next layer)
                    ncfg = CFGS[li + 1]
                    if ncfg.up4:
                        XFrep_cur = poolA.tile(
                            [128, ncfg.nG * ncfg.V], bf16, tag="A")
                        s_r = XFn[:].rearrange("p (g w) -> p g w", w=V)
                        d_r = XFrep_cur[:].rearrange(
                            "p (g w r) -> p g w r", w=V, r=4)
                    nap = max(1, nGp // 4)
                    bnds = [0, 1] + list(range(1 + nap, nGp, nap)) + [nGp]
                    bnds = sorted(set(b for b in bnds if b <= nGp))
                    for q0, q1 in zip(bnds[:-1], bnds[1:]):
                        c0_, c1_ = q0 * V, q1 * V
                        cm = c0_ + (c1_ - c0_) * 5 // 9  # Act a bit slower
                        nc.scalar.activation(
                            XFn[:, c0_:cm], XFn[:, c0_:cm],
                            AF.Relu, scale=stc[:, 0:1], bias=stc[:, 1:2])
                        nc.vector.tensor_scalar(
                            out=XFn[:, cm:c1_], in0=XFn[:, cm:c1_],
                            scalar1=stc[:, 0:1], scalar2=stc[:, 1:2],
                            op0=ALU.mult, op1=ALU.add)
                        nc.vector.tensor_scalar_max(
                            XFn[:, cm:c1_], XFn[:, cm:c1_], 0.0)
                        if ncfg.up4:
                            for r in range(4):
                                nc.vector.tensor_copy(
                                    d_r[:, q0:q1, :, r], s_r[:, q0:q1, :])
                    XF_cur = XFn
                else:
                    # --- output: ship ytile [v-part, (b,c)] as-is; host
                    # untangles the (p, t, b, c) layout in numpy ---
                    for t0, t1 in ((0, 4), (4, 8), (8, 10)):
                        nc.sync.dma_start(
                            ydram[:, t0 * BF:t1 * BF],
                            ytile[:, t0 * BF:t1 * BF])

    nc.compile()
    return nc


def kernel(**inputs):
    import sys
    for p in ("/opt/trn_rl_repo", "/opt/trn_rl_repo/concourse"):
        if p not in sys.path:
            sys.path.insert(0, p)
    from concourse.bass_utils import run_bass_kernel_spmd

    host = _build_host(inputs)

    if "nc" not in _CACHE:
        _CACHE["nc"] = _build_nc()
    nc = _CACHE["nc"]

    in_maps = []
    for c in range(NCORES):
        m = {k: v for k, v in host.items() if k != "xT"}
        m["xT"] = np.ascontiguousarray(host["xT"][:, c * BL:(c + 1) * BL])
        in_maps.append(m)
    res = run_bass_kernel_spmd(nc, in_maps, core_ids=list(range(NCORES)))
    # y[p, t*96 + b*3 + c] -> out[b, t*128+p, c]
    outs = []
    for r in res.results:
        y = np.asarray(r["y"], np.float32).reshape(128, 10, BL, 3)
        outs.append(y.transpose(2, 1, 0, 3).reshape(BL, 1280, 3))
    return np.concatenate(outs, axis=0).astype(np.float32)


if __name__ == "__main__":
    import reference as R
    inp = R.setup_inputs()
    inp = {k: np.asarray(v) for k, v in inp.items()}
    act = kernel(**inp)
    exp = np.asarray(R.reference(**inp))
    err = np.linalg.norm(act - exp) / np.linalg.norm(exp)
    print("Relative error:", err)
